# revision 1
# baseline (speedup 1.0000x reference)
"""Trainium2 Bass kernel: CANE FeatureEmbedding GNN message passing.

Strategy (node-range sharding, 8 cores):
  - Nodes are range-partitioned: core r owns nodes [r*6250, (r+1)*6250).
  - Edges are assigned to the core owning their DESTINATION (row = edge_index[1]).
  - Algebraic collapse of the peer branch (gather and scatter both use `row`):
        h_peer[i] = relu( sqrt(deg_i)*(W_px x_i + b_p) + deg_i^-1/2 * (W_pe S_ea[i]) )
    where S_ea[i] = segment_sum(edge_attrs, row). This removes the [E,160]x[160,96]
    per-edge MLP entirely.
  - Per-edge work that remains: h_e = relu(ea @ W_edge.T + b_edge), segment sums of
    ea and h_e over row, and M[i] = sum_{e: row=i} h_e_agg[col[e]].
  - Segment sums use a fixed "slot grid": each node gets C slots; edge k of node v
    goes to (block b = v//128, partition v%128, slot k). One matmul per slot tile
    accumulates S_ea directly in PSUM; h_e slots land in per-slot PSUM columns and
    are reduced after a single big ReLU. Nodes with deg > C spill to per-block
    overflow tiles handled with one-hot scatter matmuls.
  - One AllGather shares h_e_agg; M is built from an indirect-DMA gather of
    h_e_agg[col] in the same slot layout, then reduced along slots.
"""

import numpy as np

import concourse.bass as bass
import concourse.mybir as mybir
import concourse.tile as tile
from concourse import bacc
from concourse._compat import get_trn_type
from concourse.bass import IndirectOffsetOnAxis
from concourse.bass_utils import run_bass_kernel_spmd

F32 = mybir.dt.float32
I32 = mybir.dt.int32
AX = mybir.AxisListType
OP = mybir.AluOpType
ACT = mybir.ActivationFunctionType


class Cfg:
    def __init__(self, N=50000, E=800000, ncores=8, C=15, t_ov=3):
        self.N = N
        self.E = E
        self.ncores = ncores
        self.NPC = N // ncores            # nodes per core
        self.NBLK = (self.NPC + 127) // 128
        self.PADN = self.NBLK * 128       # padded nodes per core
        self.C = C                        # grid slots per node
        self.t_ov = t_ov                  # overflow tiles per block
        self.TPB = C + t_ov               # tiles per block
        self.ND = 128                     # node feature dim
        self.ED = 32                      # edge feature dim
        self.OUTD = 224

    def key(self):
        return (self.N, self.E, self.ncores, self.C, self.t_ov)


def build_program(cfg, skip=()):
    """Build the SPMD Bass program (same NEFF for all cores).

    skip: iterable of {"gather", "slotmm", "overflow", "phasec", "collective"} —
    timing-experiment knobs only (results are wrong when used)."""
    skip = set(skip)
    nc = bacc.Bacc(get_trn_type() or "TRN2", target_bir_lowering=False, debug=True)
    NBLK, TPB, C, t_ov, PADN = cfg.NBLK, cfg.TPB, cfg.C, cfg.t_ov, cfg.PADN
    TOT = NBLK * TPB

    eat = nc.declare_dram_parameter("eat", [33, TOT * 128], F32, isOutput=False)
    gidx = nc.declare_dram_parameter("gidx", [128, NBLK * TPB], I32, isOutput=False)
    rlov = nc.declare_dram_parameter("rlov", [128, NBLK * t_ov], F32, isOutput=False)
    xt = nc.declare_dram_parameter("xt", [128, PADN], F32, isOutput=False)
    dinvp = nc.declare_dram_parameter("dinv", [128, NBLK], F32, isOutput=False)
    sqdp = nc.declare_dram_parameter("sqd", [128, NBLK], F32, isOutput=False)
    sdrow = nc.declare_dram_parameter("sdrow", [1, PADN], F32, isOutput=False)
    rconst = nc.declare_dram_parameter("rconst", [33, 64], F32, isOutput=False)
    wegot = nc.declare_dram_parameter("wegot", [128, 64], F32, isOutput=False)
    wpxt = nc.declare_dram_parameter("wpxt", [128, 96], F32, isOutput=False)
    wpet = nc.declare_dram_parameter("wpet", [32, 96], F32, isOutput=False)
    bego = nc.declare_dram_parameter("bego", [1, 64], F32, isOutput=False)
    bpeer = nc.declare_dram_parameter("bpeer", [1, 96], F32, isOutput=False)
    iota = nc.declare_dram_parameter("iota", [128, 128], F32, isOutput=False)
    ident = nc.declare_dram_parameter("ident", [128, 128], F32, isOutput=False)
    outp = nc.declare_dram_parameter("out", [PADN, cfg.OUTD], F32, isOutput=True)

    with tile.TileContext(nc) as tc:
        with (
            tc.tile_pool(name="const", bufs=1) as cp,
            tc.tile_pool(name="resident", bufs=1) as rp,
            tc.tile_pool(name="dram", bufs=1, space="DRAM") as dp,
        ):
            rc = cp.tile([33, 64], F32)
            nc.sync.dma_start(rc[:], rconst[:])
            wego_sb = cp.tile([128, 64], F32)
            nc.sync.dma_start(wego_sb[:], wegot[:])
            wpx_sb = cp.tile([128, 96], F32)
            nc.sync.dma_start(wpx_sb[:], wpxt[:])
            wpe_sb = cp.tile([32, 96], F32)
            nc.sync.dma_start(wpe_sb[:], wpet[:])
            bego_sb = cp.tile([1, 64], F32)
            nc.sync.dma_start(bego_sb[:], bego[:])
            bpeer_sb = cp.tile([1, 96], F32)
            nc.sync.dma_start(bpeer_sb[:], bpeer[:])
            iota_sb = cp.tile([128, 128], F32)
            nc.sync.dma_start(iota_sb[:], iota[:])
            ident_sb = cp.tile([128, 128], F32)
            nc.sync.dma_start(ident_sb[:], ident[:])
            ones_sb = cp.tile([1, 128], F32)
            nc.gpsimd.memset(ones_sb[:], 1.0)

            xt_sb = rp.tile([128, PADN], F32)
            nc.sync.dma_start(xt_sb[:], xt[:])
            gidx_sb = rp.tile([128, NBLK * TPB], I32)
            nc.sync.dma_start(gidx_sb[:], gidx[:])
            rlov_sb = rp.tile([128, NBLK * t_ov], F32)
            nc.sync.dma_start(rlov_sb[:], rlov[:])
            dinv_sb = rp.tile([128, NBLK], F32)
            nc.sync.dma_start(dinv_sb[:], dinvp[:])
            sqd_sb = rp.tile([128, NBLK], F32)
            nc.sync.dma_start(sqd_sb[:], sqdp[:])
            sdrow_sb = rp.tile([1, PADN], F32)
            nc.sync.dma_start(sdrow_sb[:], sdrow[:])

            sea_sb = rp.tile([128, NBLK * 32], F32)
            heagg_sb = rp.tile([128, NBLK * 32], F32)

            zloc = dp.tile([PADN, 32], F32)
            zag = dp.tile([cfg.ncores * PADN, 32], F32, addr_space="Shared")

            # ---------------- Phase A: per-edge MLP + segment sums ----------
            with (
                tc.tile_pool(name="eatp", bufs=int(getattr(cfg, "eat_bufs", 3))) as eatp,
                tc.tile_pool(name="workA", bufs=int(getattr(cfg, "wa_bufs", 4))) as wp,
                tc.tile_pool(name="psA", bufs=2, space="PSUM") as psA,
                tc.tile_pool(name="psB", bufs=int(getattr(cfg, "psb_bufs", 2)), space="PSUM") as psB,
                tc.tile_pool(name="psP", bufs=int(getattr(cfg, "psp_bufs", 2)), space="PSUM") as psP,
                tc.tile_pool(name="psO", bufs=2, space="PSUM") as psO,
            ):
                for b in range(NBLK):
                    ech = eatp.tile([33, TPB * 128], F32, tag="ech")
                    nc.sync.dma_start(ech[:], eat[:, b * TPB * 128:(b + 1) * TPB * 128])
                    bankA = psA.tile([128, 32], F32, tag="bankA")
                    bankB = psB.tile([128, C * 32], F32, tag="bankB")
                    for j in range(C):
                        if "slotmm" in skip:
                            break
                        lh = ech[:, j * 128:(j + 1) * 128]
                        nc.tensor.matmul(bankA[:], lh, rc[:, 0:32],
                                         start=(j == 0), stop=(j == C - 1))
                        nc.tensor.matmul(bankB[:, j * 32:(j + 1) * 32], lh, rc[:, 32:64],
                                         start=(j == 0), stop=(j == C - 1))
                    bankO = psO.tile([128, 64], F32, tag="bankO")
                    for o in ([] if "overflow" in skip else range(t_ov)):
                        lh = ech[:, (C + o) * 128:(C + o + 1) * 128]
                        pc = psP.tile([128, 64], F32, tag="pc")
                        nc.tensor.matmul(pc[:], lh, rc[:, 0:64], start=True, stop=True)
                        ov = wp.tile([128, 64], F32, tag="ov")
                        nc.vector.tensor_copy(ov[:, 0:32], pc[:, 0:32])
                        nc.vector.tensor_scalar_max(ov[:, 32:64], pc[:, 32:64], 0.0)
                        oh = wp.tile([128, 128], F32, tag="oh")
                        k = b * t_ov + o
                        nc.vector.tensor_scalar(
                            out=oh[:], in0=iota_sb[:],
                            scalar1=rlov_sb[:, k:k + 1], scalar2=None,
                            op0=OP.is_equal,
                        )
                        nc.tensor.matmul(bankO[:], oh[:], ov[:],
                                         start=(o == 0), stop=(o == t_ov - 1))
                    relu_st = wp.tile([128, C * 32], F32, tag="relu")
                    nc.scalar.activation(relu_st[:], bankB[:], ACT.Relu)
                    t_he = wp.tile([128, 32], F32, tag="the")
                    nc.vector.tensor_reduce(
                        t_he[:],
                        relu_st[:].rearrange("p (j c) -> p c j", j=C),
                        axis=AX.X, op=OP.add,
                    )
                    ovsb = wp.tile([128, 64], F32, tag="ovsb")
                    nc.vector.tensor_copy(ovsb[:], bankO[:])
                    nc.vector.tensor_tensor(
                        out=heagg_sb[:, b * 32:(b + 1) * 32],
                        in0=t_he[:], in1=ovsb[:, 32:64], op=OP.add)
                    nc.vector.tensor_tensor(
                        out=sea_sb[:, b * 32:(b + 1) * 32],
                        in0=ovsb[:, 0:32], in1=bankA[:], op=OP.add)

            # h_e_agg -> DRAM -> AllGather
            nc.sync.dma_start(
                zloc[:].rearrange("(b p) c -> p b c", p=128),
                heagg_sb[:].rearrange("p (b c) -> p b c", c=32),
            )
            if "collective" not in skip:
                for _rep in range(int(getattr(cfg, "ag_rep", 1))):
                    nc.gpsimd.collective_compute(
                        "AllGather", OP.bypass,
                        ins=[zloc.opt()], outs=[zag.opt()],
                        replica_groups=[list(range(cfg.ncores))],
                    )

            # ------------- Phase B+C: gather/M + node-level MLPs ------------
            with (
                tc.tile_pool(name="workB", bufs=int(getattr(cfg, "wb_bufs", 4))) as wb,
                tc.tile_pool(name="outp_pool", bufs=int(getattr(cfg, "op_bufs", 3))) as op_pool,
                tc.tile_pool(name="psM", bufs=2, space="PSUM") as psM,
                tc.tile_pool(name="ps1", bufs=2, space="PSUM") as ps1,
                tc.tile_pool(name="ps2", bufs=2, space="PSUM") as ps2,
                tc.tile_pool(name="ps3", bufs=1, space="PSUM") as ps3,
                tc.tile_pool(name="psT", bufs=1, space="PSUM") as psT,
            ):
                for b in range(NBLK):
                    g = wb.tile([128, TPB * 32], F32, tag="g", bufs=8)
                    for t in ([] if "gather" in skip else range(TPB)):
                        for _rep in range(int(getattr(cfg, "gather_rep", 1))):
                            nc.gpsimd.indirect_dma_start(
                                out=g[:, t * 32:(t + 1) * 32],
                                out_offset=None,
                                in_=zag[:],
                                in_offset=IndirectOffsetOnAxis(
                                    ap=gidx_sb[:, b * TPB + t:b * TPB + t + 1], axis=0),
                            )
                    m_main = wb.tile([128, 32], F32, tag="mmain")
                    nc.vector.tensor_reduce(
                        m_main[:],
                        g[:, 0:C * 32].rearrange("p (j c) -> p c j", j=C),
                        axis=AX.X, op=OP.add,
                    )
                    pm = psM.tile([128, 32], F32, tag="pm")
                    for o in range(t_ov):
                        oh = wb.tile([128, 128], F32, tag="oh2")
                        k = b * t_ov + o
                        nc.vector.tensor_scalar(
                            out=oh[:], in0=iota_sb[:],
                            scalar1=rlov_sb[:, k:k + 1], scalar2=None,
                            op0=OP.is_equal,
                        )
                        nc.tensor.matmul(pm[:], oh[:], g[:, (C + o) * 32:(C + o + 1) * 32],
                                         start=(o == 0), stop=(o == t_ov - 1))
                    outst = op_pool.tile([128, cfg.OUTD], F32, tag="outst")
                    t_m = wb.tile([128, 32], F32, tag="tm")
                    nc.vector.tensor_tensor(out=t_m[:], in0=m_main[:], in1=pm[:], op=OP.add)
                    nc.vector.tensor_scalar_mul(outst[:, 96:128], t_m[:], dinv_sb[:, b:b + 1])
                    nc.vector.tensor_copy(outst[:, 64:96], heagg_sb[:, b * 32:(b + 1) * 32])

                    # h_ego = relu(x W_ego^T + b_ego)
                    if "phasec" in skip:
                        nc.sync.dma_start(outp[b * 128:(b + 1) * 128, :], outst[:])
                        continue
                    p1 = ps1.tile([128, 64], F32, tag="p1")
                    nc.tensor.matmul(p1[:], ones_sb[:], bego_sb[:], start=True, stop=False)
                    nc.tensor.matmul(p1[:], xt_sb[:, b * 128:(b + 1) * 128], wego_sb[:],
                                     start=False, stop=True)
                    nc.vector.tensor_scalar_max(outst[:, 0:64], p1[:], 0.0)

                    # h_peer = relu(sqd*(W_px x) + sqd*b_p + W_pe (dinv*S_ea))
                    p2 = ps2.tile([128, 96], F32, tag="p2")
                    nc.tensor.matmul(p2[:], xt_sb[:, b * 128:(b + 1) * 128], wpx_sb[:],
                                     start=True, stop=True)
                    p3 = ps3.tile([128, 96], F32, tag="p3")
                    nc.tensor.matmul(p3[:], sdrow_sb[:, b * 128:(b + 1) * 128], bpeer_sb[:],
                                     start=True, stop=False)
                    t_s = wb.tile([128, 32], F32, tag="ts")
                    nc.vector.tensor_scalar_mul(t_s[:], sea_sb[:, b * 32:(b + 1) * 32],
                                                dinv_sb[:, b:b + 1])
                    pt = psT.tile([32, 128], F32, tag="pt")
                    nc.tensor.matmul(pt[:], t_s[:], ident_sb[:], is_transpose=True,
                                     start=True, stop=True)
                    seat = wb.tile([32, 128], F32, tag="seat")
                    nc.vector.tensor_copy(seat[:], pt[:])
                    nc.tensor.matmul(p3[:], seat[:], wpe_sb[:], start=False, stop=True)
                    t_u = wb.tile([128, 96], F32, tag="tu")
                    nc.vector.tensor_scalar_mul(t_u[:], p2[:], sqd_sb[:, b:b + 1])
                    nc.vector.tensor_tensor(out=t_u[:], in0=t_u[:], in1=p3[:], op=OP.add)
                    nc.vector.tensor_scalar_max(outst[:, 128:224], t_u[:], 0.0)

                    nc.sync.dma_start(outp[b * 128:(b + 1) * 128, :], outst[:])
    nc.compile()
    return nc


def host_prep(cfg, x, edge_attrs, edge_index):
    """Shard + lay out inputs for the slot-grid kernel. Pure index work + O(N)
    scalar prep (degree normalizers); all O(E*H)/O(N*H) FP math runs on device."""
    N, E, C, NBLK, TPB, t_ov, NPC, PADN = (cfg.N, cfg.E, cfg.C, cfg.NBLK,
                                           cfg.TPB, cfg.t_ov, cfg.NPC, cfg.PADN)
    row = np.asarray(edge_index[1]).astype(np.int64)
    col = np.asarray(edge_index[0]).astype(np.int64)
    ea = np.asarray(edge_attrs, dtype=np.float32)
    xf = np.asarray(x, dtype=np.float32)

    deg = np.bincount(row, minlength=N)
    degf = np.maximum(deg, 1).astype(np.float64)
    dinv = np.where(deg > 0, degf ** -0.5, 0.0).astype(np.float32)
    sqd = np.sqrt(deg.astype(np.float64)).astype(np.float32)

    core = row // NPC
    lrow = row - core * NPC
    blk = lrow // 128
    part = lrow % 128

    # rank of each edge within its destination node
    order = np.argsort(row, kind="stable")
    sorted_row = row[order]
    starts = np.searchsorted(sorted_row, np.arange(N), side="left")
    rank = np.empty(E, np.int64)
    rank[order] = np.arange(E) - starts[sorted_row]

    is_grid = rank < C
    ovsel = ~is_grid
    ove = np.where(ovsel)[0]
    ovkey = core[ove] * NBLK + blk[ove]
    o_order = np.argsort(ovkey, kind="stable")
    ove = ove[o_order]
    okey_sorted = ovkey[o_order]
    ostarts = np.searchsorted(okey_sorted, np.arange(NBLK * cfg.ncores), side="left")
    opos = np.arange(ove.size) - ostarts[okey_sorted]
    otile = C + opos // 128
    opart = opos % 128
    if ove.size and otile.max() >= TPB:
        raise ValueError("overflow tiles exceeded; raise t_ov")

    # tile index + within-tile partition for every edge
    tile_idx = np.empty(E, np.int64)
    tpart = np.empty(E, np.int64)
    ge = np.where(is_grid)[0]
    tile_idx[ge] = blk[ge] * TPB + rank[ge]
    tpart[ge] = part[ge]
    tile_idx[ove] = blk[ove] * TPB + otile
    tpart[ove] = opart

    zrow = (col // NPC) * PADN + (col % NPC)     # row in allgathered z table
    assert NPC < PADN, "pad-slot gathers need a guaranteed-zero dummy row"
    ZPAD = PADN - 1                               # core0 dummy node -> zeros

    TOTC = NBLK * TPB
    in_maps = []
    # constants shared by all cores are built once
    consts = None
    for r in range(cfg.ncores):
        sel = core == r
        e_idx = np.where(sel)[0]
        t_i = tile_idx[e_idx]
        t_p = tpart[e_idx]
        colpos = t_i * 128 + t_p

        EAT = np.zeros((33, TOTC * 128), np.float32)
        EAT[:32, colpos] = ea[e_idx].T
        EAT[32, colpos] = 1.0

        GIDX = np.full((128, TOTC), ZPAD, np.int32)
        GIDX[t_p, t_i] = zrow[e_idx].astype(np.int32)

        RLOV = np.full((128, NBLK * t_ov), 200.0, np.float32)
        ovm = sel[ove] if False else None
        ov_r = ove[core[ove] == r]
        op_r = opart[core[ove] == r]
        ot_r = otile[core[ove] == r]
        ob_r = blk[ov_r]
        RLOV[op_r, ob_r * t_ov + (ot_r - C)] = part[ov_r].astype(np.float32)

        lo, hi = r * NPC, (r + 1) * NPC
        XT = np.zeros((128, PADN), np.float32)
        XT[:, :NPC] = xf[lo:hi].T
        dl = np.zeros(PADN, np.float32)
        dl[:NPC] = dinv[lo:hi]
        sl = np.zeros(PADN, np.float32)
        sl[:NPC] = sqd[lo:hi]
        DINV = dl.reshape(NBLK, 128).T.copy()
        SQD = sl.reshape(NBLK, 128).T.copy()
        SDROW = sl.reshape(1, PADN)

        m = {
            "eat": EAT, "gidx": GIDX, "rlov": RLOV, "xt": XT,
            "dinv": DINV, "sqd": SQD, "sdrow": SDROW,
        }
        in_maps.append(m)
    return in_maps


def make_consts(cfg, W_peer, b_peer, W_ego, b_ego, W_edge, b_edge):
    RCONST = np.zeros((33, 64), np.float32)
    RCONST[:32, :32] = np.eye(32, dtype=np.float32)
    RCONST[:32, 32:64] = np.asarray(W_edge, np.float32).T
    RCONST[32, 32:64] = np.asarray(b_edge, np.float32)
    consts = {
        "rconst": RCONST,
        "wegot": np.ascontiguousarray(np.asarray(W_ego, np.float32).T),
        "wpxt": np.ascontiguousarray(np.asarray(W_peer, np.float32)[:, :128].T),
        "wpet": np.ascontiguousarray(np.asarray(W_peer, np.float32)[:, 128:].T),
        "bego": np.asarray(b_ego, np.float32).reshape(1, 64),
        "bpeer": np.asarray(b_peer, np.float32).reshape(1, 96),
        "iota": np.broadcast_to(np.arange(128, dtype=np.float32), (128, 128)).copy(),
        "ident": np.eye(128, dtype=np.float32),
    }
    return consts


_CACHE = {}
RUN_KWARGS = {}


def kernel(x, edge_attrs, W_peer, b_peer, W_ego, b_ego, W_edge, b_edge, edge_index):
    x = np.asarray(x)
    edge_attrs = np.asarray(edge_attrs)
    edge_index = np.asarray(edge_index)
    N, E = x.shape[0], edge_attrs.shape[0]

    # pick t_ov from the actual degree distribution (>=3 keeps NEFF cache warm
    # for the expected data)
    row = edge_index[1].astype(np.int64)
    C = 15
    ncores = 8
    NPC = N // ncores
    NBLK = (NPC + 127) // 128
    deg = np.bincount(row, minlength=N)
    ovn = np.maximum(deg - C, 0)
    nodes = np.arange(N)
    bkey = (nodes // NPC) * NBLK + (nodes % NPC) // 128
    ovblk = np.bincount(bkey, weights=ovn.astype(np.float64), minlength=NBLK * ncores)
    t_ov = max(3, int(np.ceil(ovblk.max() / 128.0)))

    cfg = Cfg(N=N, E=E, ncores=ncores, C=C, t_ov=t_ov)
    key = cfg.key()
    if key not in _CACHE:
        _CACHE[key] = build_program(cfg)
    nc = _CACHE[key]

    in_maps = host_prep(cfg, x, edge_attrs, edge_index)
    consts = make_consts(cfg, W_peer, b_peer, W_ego, b_ego, W_edge, b_edge)
    for m in in_maps:
        m.update(consts)

    res = run_bass_kernel_spmd(nc, in_maps, core_ids=list(range(cfg.ncores)),
                               **RUN_KWARGS)
    out = np.empty((N, cfg.OUTD), np.float32)
    for r in range(cfg.ncores):
        out[r * cfg.NPC:(r + 1) * cfg.NPC] = res.results[r]["out"][:cfg.NPC]
    if RUN_KWARGS:
        kernel.last_result = res
    return out



# revision 35
# speedup vs baseline: 2.8026x; 2.8026x over previous
"""Trainium2 Bass kernel: CANE FeatureEmbedding GNN message passing.

Strategy (node-range sharding, 8 cores), v2:
  - Nodes range-partitioned; edges assigned to the core owning their
    DESTINATION (row = edge_index[1]).
  - Algebraic collapse of the peer branch (gather and scatter both use `row`):
        h_peer[i] = relu( sqrt(deg_i)*(W_px x_i + b_p) + deg_i^-1/2 * (W_pe S_ea[i]) )
    removing the [E,160]x[160,96] per-edge MLP.
  - Remaining per-edge work: h_e = relu(ea @ W_edge.T + b_edge), segment sums
    of ea and h_e over row, and M[i] = sum_{e: row=i} h_e_agg[col[e]].
  - Slot grid: node v gets C slots; edge k of v goes to (block v//128,
    partition v%128, slot k). One bf16 matmul per slot tile accumulates S_ea
    in PSUM; h_e pre-acts land per-slot, relu on the Act engine, slot-sum on
    DVE straight into PSUM, overflow (deg>C) via one-hot scatter matmuls.
  - h_e_agg shared across cores in bf16 (AllGather or remote-DMA + barrier
    collective); M built from batched multi-index indirect-DMA gathers of
    h_e_agg[col] in slot layout, reduced along slots.
  - Everything streams in bf16; PSUM accumulation in fp32; output written as
    [PADN, 256] bf16 rows (cols 224:256 pad) and upcast host-side.
"""

import numpy as np

import concourse.bass as bass
import concourse.mybir as mybir
import concourse.tile as tile
from concourse import bacc
from concourse._compat import get_trn_type
from concourse.bass import IndirectOffsetOnAxis
from concourse.bass_utils import run_bass_kernel_spmd

F32 = mybir.dt.float32
BF16 = mybir.dt.float16  # fp16: 4x finer mantissa than bf16, same perf
I32 = mybir.dt.int32
I16 = mybir.dt.int16
AX = mybir.AxisListType
OP = mybir.AluOpType
ACT = mybir.ActivationFunctionType


class Cfg:
    def __init__(self, N=50000, E=800000, ncores=8, C=15, t_ov=3, GB=4, EB=4,
                 CH=8, t_ov2=2):
        self.N = N
        self.E = E
        self.ncores = ncores
        self.NPC = N // ncores            # nodes per core
        self.NBLK = (self.NPC + 127) // 128
        self.PADN = self.NBLK * 128       # padded nodes per core
        self.C = C                        # grid slots per node
        self.t_ov = t_ov                  # overflow tiles per block
        self.TPB = C + t_ov               # tiles per block
        self.GB = GB                      # blocks per gather instruction
        self.EB = EB                      # blocks per eat-load DMA
        self.CH = CH                      # phase-B main slots per (node, half)
        self.t_ov2 = t_ov2                # phase-B overflow tiles per (block, half)
        self.THALF = CH + t_ov2           # phase-B tiles per (block, half)
        self.IW = (self.THALF * 128 + 15) // 16   # int16 idx cols per gather
        self.ND = 128
        self.ED = 32
        self.OUTD = 224
        self.OUTP = 256                   # padded out row

    def key(self):
        return (self.N, self.E, self.ncores, self.C, self.t_ov, self.GB,
                self.EB, self.CH, self.t_ov2)


def raw_dma_gather(nc, out_ap, in_ap, idxs_ap, num_idxs, elem_size, elem_step):
    """nc.gpsimd.dma_gather minus the %256 payload assert (the 256B rule
    applies to the row STRIDE, which elem_step satisfies; the ucode packetizes
    the payload at any size)."""
    gp = nc.gpsimd
    stride_bytes = elem_step * mybir.dt.size(in_ap.dtype)
    assert stride_bytes % 256 == 0 and stride_bytes // 256 < 256
    _in_ap = gp.lower_ap_dma(in_ap, for_custom_bir_dma=True)
    _idxs_ap = gp.lower_ap(idxs_ap)
    _out_ap = gp.lower_ap(out_ap)
    return gp.add_instruction(
        mybir.InstDMAGatherAnt(
            name=f"I-{nc.next_id()}",
            ins=[*_in_ap, _idxs_ap, gp.lower_val_access(gp.to_reg(num_idxs))],
            outs=[_out_ap],
            transpose=False,
            num_idxs=num_idxs,
            elem_size=elem_size,
            stride_bytes_256=stride_bytes // 256,
            gen_mode=0,
            single_packet=True,
            queue_num=0,
            sbuf_tokens_per_rank=0,
            sbuf_free_dim_per_rank=0,
            sbuf_free_dim_pad_per_rank=0,
            sbuf_byte_offset=0,
        )
    )


def build_program(cfg, skip=()):
    """Build the SPMD Bass program (same NEFF for all cores).

    skip: {"gather", "slotmm", "overflow", "phasec", "collective"} — timing
    experiment knobs only (results are wrong when used)."""
    skip = set(skip)
    nc = bacc.Bacc(get_trn_type() or "TRN2", target_bir_lowering=False, debug=True)
    NBLK, TPB, C, t_ov, PADN = cfg.NBLK, cfg.TPB, cfg.C, cfg.t_ov, cfg.PADN
    GB, EB, OUTP = cfg.GB, cfg.EB, cfg.OUTP
    TOT = NBLK * TPB

    eat = nc.declare_dram_parameter("eat", [33, TOT * 128], BF16, isOutput=False)
    gidx16 = nc.declare_dram_parameter(
        "gidx16", [128, NBLK * 2 * cfg.IW], I16, isOutput=False)
    rlov2 = nc.declare_dram_parameter("rlov2", [128, NBLK * 2 * cfg.t_ov2], F32,
                                      isOutput=False)
    rlov = nc.declare_dram_parameter("rlov", [128, NBLK * t_ov], F32, isOutput=False)
    xt = nc.declare_dram_parameter("xt", [128, PADN], BF16, isOutput=False)
    dinvp = nc.declare_dram_parameter("dinv", [128, NBLK], F32, isOutput=False)
    sqdp = nc.declare_dram_parameter("sqd", [128, NBLK], F32, isOutput=False)
    deginvp = nc.declare_dram_parameter("deginv", [128, NBLK], F32, isOutput=False)
    rconst = nc.declare_dram_parameter("rconst", [33, 64], BF16, isOutput=False)
    wegot = nc.declare_dram_parameter("wegot", [128, 64], BF16, isOutput=False)
    wpxt = nc.declare_dram_parameter("wpxt", [128, 96], BF16, isOutput=False)
    wpet = nc.declare_dram_parameter("wpet", [32, 96], BF16, isOutput=False)
    bego = nc.declare_dram_parameter("bego", [1, 64], BF16, isOutput=False)
    bpeer = nc.declare_dram_parameter("bpeer", [1, 96], BF16, isOutput=False)
    iota = nc.declare_dram_parameter("iota", [128, 128], BF16, isOutput=False)
    ident = nc.declare_dram_parameter("ident", [128, 128], BF16, isOutput=False)
    outp = nc.declare_dram_parameter("out", [PADN, OUTP], BF16, isOutput=True)

    with tile.TileContext(nc) as tc:
        with (
            tc.tile_pool(name="const", bufs=1) as cp,
            tc.tile_pool(name="resident", bufs=1) as rp,
            tc.tile_pool(name="dram", bufs=1, space="DRAM") as dp,
        ):
            rc = cp.tile([33, 64], BF16)
            nc.sync.dma_start(rc[:], rconst[:])
            wego_sb = cp.tile([128, 64], BF16)
            nc.sync.dma_start(wego_sb[:], wegot[:])
            wpx_sb = cp.tile([128, 96], BF16)
            nc.sync.dma_start(wpx_sb[:], wpxt[:])
            wpe_sb = cp.tile([32, 96], BF16)
            nc.sync.dma_start(wpe_sb[:], wpet[:])
            bego_sb = cp.tile([1, 64], BF16)
            nc.sync.dma_start(bego_sb[:], bego[:])
            bpeer_sb = cp.tile([1, 96], BF16)
            nc.sync.dma_start(bpeer_sb[:], bpeer[:])
            iota_sb = cp.tile([128, 128], BF16)
            nc.sync.dma_start(iota_sb[:], iota[:])
            ident_sb = cp.tile([128, 128], BF16)
            nc.sync.dma_start(ident_sb[:], ident[:])
            ones_sb = cp.tile([1, 128], BF16)
            nc.gpsimd.memset(ones_sb[:], 1.0)

            xt_sb = rp.tile([128, PADN], BF16)
            nc.sync.dma_start(xt_sb[:], xt[:])
            gidx16_sb = rp.tile([128, NBLK * 2 * cfg.IW], I16)
            nc.sync.dma_start(gidx16_sb[:], gidx16[:])
            rlov2_sb = rp.tile([128, NBLK * 2 * cfg.t_ov2], F32)
            nc.sync.dma_start(rlov2_sb[:], rlov2[:])
            rlov_sb = rp.tile([128, NBLK * t_ov], F32)
            nc.sync.dma_start(rlov_sb[:], rlov[:])
            dinv_sb = rp.tile([128, NBLK], F32)
            nc.sync.dma_start(dinv_sb[:], dinvp[:])
            sqd_sb = rp.tile([128, NBLK], F32)
            nc.sync.dma_start(sqd_sb[:], sqdp[:])
            deginv_sb = rp.tile([128, NBLK], F32)
            nc.sync.dma_start(deginv_sb[:], deginvp[:])

            heagg_bf = rp.tile([128, NBLK * 32], BF16)   # h_e_agg, bf16
            ts_cache = rp.tile([128, NBLK * 32], BF16)   # dinv * S_ea, bf16
            oh_cache = rp.tile([128, NBLK * t_ov * 128], BF16)  # one-hots
            outst = rp.tile([128, NBLK * OUTP], BF16)    # output staging

            # zag holds all cores' h_e_agg (p-major rows within each core
            # section); zagA/zagB are 256B-stride padded halves for dma_gather
            zloc = dp.tile([PADN, 32], BF16)
            zag = dp.tile([cfg.ncores * PADN, 32], BF16, addr_space="Shared")
            zagA = dp.tile([cfg.ncores // 2 * PADN, 128], BF16)
            zagB = dp.tile([cfg.ncores // 2 * PADN, 128], BF16)

            # ---------------- Phase A: per-edge MLP + segment sums ----------
            with (
                tc.tile_pool(name="eatp", bufs=3) as eatp,
                tc.tile_pool(name="workA", bufs=4) as wp,
                tc.tile_pool(name="psA", bufs=2, space="PSUM") as psA,
                tc.tile_pool(name="psB", bufs=2, space="PSUM") as psB,
                tc.tile_pool(name="psP", bufs=2, space="PSUM") as psP,
                tc.tile_pool(name="psO", bufs=2, space="PSUM") as psO,
            ):
                ech = None
                for b in range(NBLK):
                    if b % EB == 0:
                        nch = min(EB, NBLK - b)
                        ech = eatp.tile([33, nch * TPB * 128], BF16, tag="ech")
                        nc.sync.dma_start(
                            ech[:], eat[:, b * TPB * 128:(b + nch) * TPB * 128])
                    e0 = (b % EB) * TPB * 128
                    bankA = psA.tile([128, 32], F32, tag="bankA")
                    bankB = psB.tile([128, C * 32], F32, tag="bankB")
                    for j in range(C):
                        if "slotmm" in skip:
                            break
                        lh = ech[:, e0 + j * 128:e0 + (j + 1) * 128]
                        nc.tensor.matmul(bankA[:], lh, rc[:, 0:32],
                                         start=(j == 0), stop=False)
                        nc.tensor.matmul(bankB[:, j * 32:(j + 1) * 32], lh,
                                         rc[:, 32:64],
                                         start=(j == 0), stop=(j == C - 1))
                    # overflow: pre-act for t_ov tiles in one PSUM tile
                    pc = psP.tile([128, t_ov * 64], F32, tag="pc")
                    ov = wp.tile([128, t_ov * 64], BF16, tag="ov")
                    for o in range(t_ov):
                        lh = ech[:, e0 + (C + o) * 128:e0 + (C + o + 1) * 128]
                        nc.tensor.matmul(pc[:, o * 64:(o + 1) * 64], lh, rc[:, 0:64],
                                         start=(o == 0), stop=(o == t_ov - 1))
                    # ea part: copy (Act); h_e part: relu (DVE)
                    nc.scalar.copy(
                        ov[:].rearrange("p (t x) -> p t x", x=64)[:, :, 0:32],
                        pc[:].rearrange("p (t x) -> p t x", x=64)[:, :, 0:32])
                    nc.vector.tensor_scalar_max(
                        ov[:].rearrange("p (t x) -> p t x", x=64)[:, :, 32:64],
                        pc[:].rearrange("p (t x) -> p t x", x=64)[:, :, 32:64],
                        0.0)
                    # one-hot scatter matrices (cached for Phase B reuse)
                    for o in range(t_ov):
                        k = b * t_ov + o
                        nc.vector.tensor_scalar(
                            out=oh_cache[:, k * 128:(k + 1) * 128], in0=iota_sb[:],
                            scalar1=rlov_sb[:, k:k + 1], scalar2=None,
                            op0=OP.is_equal,
                        )
                        # fold overflow ea into bankA accumulation group
                        nc.tensor.matmul(bankA[:], oh_cache[:, k * 128:(k + 1) * 128],
                                         ov[:, o * 64:o * 64 + 32],
                                         start=False, stop=(o == t_ov - 1))
                    # t_s = (1/deg) * S_ea (kept bf16 for Phase C transpose;
                    # the final sqd scale rides the peer relu)
                    nc.vector.tensor_scalar_mul(
                        ts_cache[:, b * 32:(b + 1) * 32], bankA[:],
                        deginv_sb[:, b:b + 1])
                    # h_e slots: relu on Act, slot-sum on DVE into PSUM
                    relu_st = wp.tile([128, C * 32], BF16, tag="relu")
                    nc.scalar.activation(relu_st[:], bankB[:], ACT.Relu)
                    bankO = psO.tile([128, 32], F32, tag="bankO")
                    nc.vector.tensor_reduce(
                        bankO[:],
                        relu_st[:].rearrange("p (j c) -> p c j", j=C),
                        axis=AX.X, op=OP.add,
                    )
                    for o in range(t_ov):
                        k = b * t_ov + o
                        nc.tensor.matmul(bankO[:], oh_cache[:, k * 128:(k + 1) * 128],
                                         ov[:, o * 64 + 32:(o + 1) * 64],
                                         start=False, stop=(o == t_ov - 1))
                    nc.scalar.copy(heagg_bf[:, b * 32:(b + 1) * 32], bankO[:])
                    # outst[64:96] = h_e_agg; pad cols 224:256 need some value
                    nc.vector.tensor_copy(
                        outst[:, b * OUTP + 64:b * OUTP + 96],
                        heagg_bf[:, b * 32:(b + 1) * 32])
                    nc.vector.tensor_copy(
                        outst[:, b * OUTP + 224:b * OUTP + 256],
                        heagg_bf[:, b * 32:(b + 1) * 32])

            # Share h_e_agg via compact AllGather ("Shared" DRAM is NOT
            # coherently cross-core writable in this runtime, so direct
            # scatter-shares don't work); then locally re-pad each half into
            # a 256B-stride table for the int16 dma_gather.
            nc.sync.dma_start(
                zloc[:].rearrange("(p b) c -> p (b c)", p=128),
                heagg_bf[:],
            )
            if "collective" not in skip:
                nc.gpsimd.collective_compute(
                    "AllGather", OP.bypass,
                    ins=[zloc.opt()], outs=[zag.opt()],
                    replica_groups=[list(range(cfg.ncores))],
                )
            HROWS = cfg.ncores // 2 * PADN
            nc.sync.dma_start(zagA[:, 0:32], zag[0:HROWS, :])
            nc.sync.dma_start(zagB[:, 0:32], zag[HROWS:2 * HROWS, :])

            # ------------- Phase C: node MLPs (overlaps the collective) -----
            if "phasec" not in skip:
                with (
                    tc.tile_pool(name="workC", bufs=4) as wc,
                    tc.tile_pool(name="ps1", bufs=2, space="PSUM") as ps1,
                    tc.tile_pool(name="ps3", bufs=2, space="PSUM") as ps3,
                    tc.tile_pool(name="psT", bufs=2, space="PSUM") as psT,
                ):
                    for b in range(NBLK):
                        xb = xt_sb[:, b * 128:(b + 1) * 128]
                        # h_ego = relu(x W_ego^T + b_ego)
                        p1 = ps1.tile([128, 64], F32, tag="p1")
                        nc.tensor.matmul(p1[:], ones_sb[:], bego_sb[:],
                                         start=True, stop=False)
                        nc.tensor.matmul(p1[:], xb, wego_sb[:],
                                         start=False, stop=True)
                        nc.scalar.activation(
                            outst[:, b * OUTP:b * OUTP + 64], p1[:], ACT.Relu)
                        # h_peer = relu(sqd * (W_px x + b_p + W_pe ((1/deg) S_ea)))
                        pt = psT.tile([32, 128], BF16, tag="pt")
                        nc.tensor.matmul(pt[:], ts_cache[:, b * 32:(b + 1) * 32],
                                         ident_sb[:], is_transpose=True,
                                         start=True, stop=True)
                        seat = wc.tile([32, 128], BF16, tag="seat")
                        nc.scalar.copy(seat[:], pt[:])
                        p3 = ps3.tile([128, 96], F32, tag="p3")
                        nc.tensor.matmul(p3[:], ones_sb[:], bpeer_sb[:],
                                         start=True, stop=False)
                        nc.tensor.matmul(p3[:], xb, wpx_sb[:],
                                         start=False, stop=False)
                        nc.tensor.matmul(p3[:], seat[:], wpe_sb[:],
                                         start=False, stop=True)
                        nc.scalar.activation(
                            outst[:, b * OUTP + 128:b * OUTP + 224], p3[:],
                            ACT.Relu, scale=sqd_sb[:, b:b + 1])

            # ------------- Phase B: gather + M + output writes --------------
            # dma_gather is limited to 1024 indices (8 tiles) per instruction,
            # so each (block, half) span of THALF tiles is split at 8 tiles.
            CH2, tov2, THALF, IW = cfg.CH, cfg.t_ov2, cfg.THALF, cfg.IW
            with (
                tc.tile_pool(name="workB", bufs=4) as wb,
                tc.tile_pool(name="psM", bufs=3, space="PSUM") as psM,
            ):
                for g0 in range(0, NBLK, GB):
                    ng = min(GB, NBLK - g0)
                    g = wb.tile([128, ng * 2 * THALF * 32], BF16, tag="g")
                    if "gather" not in skip:
                        for br in range(ng):
                            b = g0 + br
                            for h, ztab in ((0, zagA), (1, zagB)):
                                cbase = (br * 2 + h) * THALF
                                ibase = (b * 2 + h) * IW
                                for k0 in range(0, THALF, 8):
                                    nt = min(8, THALF - k0)
                                    raw_dma_gather(
                                        nc,
                                        g[:, (cbase + k0) * 32:
                                          (cbase + k0 + nt) * 32].rearrange(
                                            "p (t e) -> p t e", e=32),
                                        ztab[:, 0:32],
                                        gidx16_sb[:, ibase + k0 * 8:
                                                  ibase + (k0 + nt) * 8],
                                        num_idxs=nt * 128, elem_size=32,
                                        elem_step=128)
                    for br in range(ng):
                        b = g0 + br
                        s0 = br * 2 * THALF * 32
                        pm = psM.tile([128, 32], F32, tag="pm")
                        nc.vector.tensor_reduce(
                            pm[:],
                            g[:, s0:s0 + 2 * THALF * 32].rearrange(
                                "p (h t c) -> p c h t", h=2, c=32)[:, :, :, 0:CH2],
                            axis=AX.XY, op=OP.add,
                        )
                        oh2 = wb.tile([128, 2 * tov2 * 128], BF16, tag="oh2")
                        nmm = 2 * tov2
                        for h in range(2):
                            for o in range(tov2):
                                k = h * tov2 + o
                                nc.vector.tensor_scalar(
                                    out=oh2[:, k * 128:(k + 1) * 128], in0=iota_sb[:],
                                    scalar1=rlov2_sb[:, (b * 2 + h) * tov2 + o:
                                                     (b * 2 + h) * tov2 + o + 1],
                                    scalar2=None, op0=OP.is_equal,
                                )
                                nc.tensor.matmul(
                                    pm[:], oh2[:, k * 128:(k + 1) * 128],
                                    g[:, ((br * 2 + h) * THALF + CH2 + o) * 32:
                                      ((br * 2 + h) * THALF + CH2 + o + 1) * 32],
                                    start=False, stop=(k == nmm - 1))
                        nc.scalar.activation(
                            outst[:, b * OUTP + 96:b * OUTP + 128], pm[:],
                            ACT.Copy, scale=dinv_sb[:, b:b + 1])
                    nc.sync.dma_start(
                        outp[g0 * 128:(g0 + ng) * 128, :].rearrange(
                            "(q p) c -> p q c", p=128),
                        outst[:, g0 * OUTP:(g0 + ng) * OUTP].rearrange(
                            "p (q c) -> p q c", c=OUTP),
                    )
    nc.compile()
    return nc


def host_prep(cfg, x, edge_attrs, edge_index):
    """Shard + lay out inputs for the slot-grid kernel. Pure index work + O(N)
    scalar prep (degree normalizers); all O(E*H)/O(N*H) FP math runs on device."""
    BF = np.float16
    N, E, C, NBLK, TPB, t_ov, NPC, PADN = (cfg.N, cfg.E, cfg.C, cfg.NBLK,
                                           cfg.TPB, cfg.t_ov, cfg.NPC, cfg.PADN)
    row = np.asarray(edge_index[1]).astype(np.int64)
    col = np.asarray(edge_index[0]).astype(np.int64)
    ea = np.asarray(edge_attrs, dtype=np.float32)
    xf = np.asarray(x, dtype=np.float32)

    deg = np.bincount(row, minlength=N)
    degf = np.maximum(deg, 1).astype(np.float64)
    dinv = np.where(deg > 0, degf ** -0.5, 0.0).astype(np.float32)
    sqd = np.sqrt(deg.astype(np.float64)).astype(np.float32)

    core = row // NPC
    lrow = row - core * NPC
    blk = lrow // 128
    part = lrow % 128

    # rank of each edge within its destination node
    order = np.argsort(row, kind="stable")
    sorted_row = row[order]
    starts = np.searchsorted(sorted_row, np.arange(N), side="left")
    rank = np.empty(E, np.int64)
    rank[order] = np.arange(E) - starts[sorted_row]

    is_grid = rank < C
    ove = np.where(~is_grid)[0]
    ovkey = core[ove] * NBLK + blk[ove]
    o_order = np.argsort(ovkey, kind="stable")
    ove = ove[o_order]
    okey_sorted = ovkey[o_order]
    ostarts = np.searchsorted(okey_sorted, np.arange(NBLK * cfg.ncores), side="left")
    opos = np.arange(ove.size) - ostarts[okey_sorted]
    otile = C + opos // 128
    opart = opos % 128
    if ove.size and otile.max() >= TPB:
        raise ValueError("overflow tiles exceeded; raise t_ov")

    tile_idx = np.empty(E, np.int64)
    tpart = np.empty(E, np.int64)
    ge = np.where(is_grid)[0]
    tile_idx[ge] = blk[ge] * TPB + rank[ge]
    tpart[ge] = part[ge]
    tile_idx[ove] = blk[ove] * TPB + otile
    tpart[ove] = opart

    # ---- Phase-B dual gather grid: edges split by source half ----------
    # half h = src_core // 4; within-half row (p-major per core section):
    #   (src_core % 4) * PADN + (l % 128) * NBLK + l // 128,  l = col % NPC
    CH, t_ov2, THALF, IW = cfg.CH, cfg.t_ov2, cfg.THALF, cfg.IW
    src_core = col // NPC
    lcol = col % NPC
    halfs = src_core // (cfg.ncores // 2)
    zrow16 = ((src_core % (cfg.ncores // 2)) * PADN
              + (lcol % 128) * NBLK + lcol // 128)
    ZPADH = 127 * NBLK + (NBLK - 1)   # half-core-0 pad node -> zeros

    # rank of each edge within (dest node, half)
    key2 = row * 2 + halfs
    order2 = np.argsort(key2, kind="stable")
    sk2 = key2[order2]
    starts2 = np.searchsorted(sk2, np.arange(2 * N), side="left")
    rank2 = np.empty(E, np.int64)
    rank2[order2] = np.arange(E) - starts2[sk2]

    is_grid2 = rank2 < CH
    ove2 = np.where(~is_grid2)[0]
    ovkey2 = (core[ove2] * NBLK + blk[ove2]) * 2 + halfs[ove2]
    o_order2 = np.argsort(ovkey2, kind="stable")
    ove2 = ove2[o_order2]
    ok2_sorted = ovkey2[o_order2]
    ostarts2 = np.searchsorted(ok2_sorted, np.arange(NBLK * cfg.ncores * 2),
                               side="left")
    opos2 = np.arange(ove2.size) - ostarts2[ok2_sorted]
    otile2 = CH + opos2 // 128
    opart2 = opos2 % 128
    if ove2.size and otile2.max() >= THALF:
        raise ValueError("phase-B overflow tiles exceeded; raise t_ov2")

    tile2 = np.empty(E, np.int64)
    tpart2 = np.empty(E, np.int64)
    ge2 = np.where(is_grid2)[0]
    tile2[ge2] = rank2[ge2]
    tpart2[ge2] = part[ge2]
    tile2[ove2] = otile2
    tpart2[ove2] = opart2

    TOTC = NBLK * TPB
    in_maps = []
    for r in range(cfg.ncores):
        sel = core == r
        e_idx = np.where(sel)[0]
        t_i = tile_idx[e_idx]
        t_p = tpart[e_idx]
        colpos = t_i * 128 + t_p

        EAT = np.zeros((33, TOTC * 128), np.float32)
        EAT[:32, colpos] = ea[e_idx].T
        EAT[32, colpos] = 1.0

        # int16 gather indices: list position i = tile*128 + dest_partition,
        # stored 16-partition-wrapped [16g + i%16, i//16], replicated x8
        ipos = ((blk[e_idx] * 2 + halfs[e_idx]) * (THALF * 128)
                + tile2[e_idx] * 128 + tpart2[e_idx])
        flat = np.full(NBLK * 2 * THALF * 128, ZPADH, np.int32)
        flat[ipos] = zrow16[e_idx]
        wrap = (flat.reshape(NBLK * 2, IW, 16).transpose(2, 0, 1)
                .reshape(16, NBLK * 2 * IW).astype(np.int16))
        GIDX16 = np.tile(wrap, (8, 1))

        RLOV = np.full((128, NBLK * t_ov), 200.0, np.float32)
        ov_r = ove[core[ove] == r]
        op_r = opart[core[ove] == r]
        ot_r = otile[core[ove] == r]
        ob_r = blk[ov_r]
        RLOV[op_r, ob_r * t_ov + (ot_r - C)] = part[ov_r].astype(np.float32)

        RLOV2 = np.full((128, NBLK * 2 * t_ov2), 200.0, np.float32)
        sel2 = core[ove2] == r
        ov2_r = ove2[sel2]
        RLOV2[opart2[sel2],
              (blk[ov2_r] * 2 + halfs[ov2_r]) * t_ov2
              + (otile2[sel2] - CH)] = part[ov2_r].astype(np.float32)

        lo, hi = r * NPC, (r + 1) * NPC
        XT = np.zeros((128, PADN), np.float32)
        XT[:, :NPC] = xf[lo:hi].T
        dl = np.zeros(PADN, np.float32)
        dl[:NPC] = dinv[lo:hi]
        sl = np.zeros(PADN, np.float32)
        sl[:NPC] = sqd[lo:hi]
        gl = np.zeros(PADN, np.float32)
        gl[:NPC] = dinv[lo:hi] ** 2          # 1/deg (0 for deg==0)
        DINV = dl.reshape(NBLK, 128).T.copy()
        SQD = sl.reshape(NBLK, 128).T.copy()
        DEGINV = gl.reshape(NBLK, 128).T.copy()

        m = {
            "eat": EAT.astype(BF), "gidx16": GIDX16, "rlov": RLOV,
            "rlov2": RLOV2, "xt": XT.astype(BF), "dinv": DINV, "sqd": SQD,
            "deginv": DEGINV,
        }
        in_maps.append(m)
    return in_maps


def make_consts(cfg, W_peer, b_peer, W_ego, b_ego, W_edge, b_edge):
    BF = np.float16
    RCONST = np.zeros((33, 64), np.float32)
    RCONST[:32, :32] = np.eye(32, dtype=np.float32)
    RCONST[:32, 32:64] = np.asarray(W_edge, np.float32).T
    RCONST[32, 32:64] = np.asarray(b_edge, np.float32)
    consts = {
        "rconst": RCONST.astype(BF),
        "wegot": np.ascontiguousarray(np.asarray(W_ego, np.float32).T).astype(BF),
        "wpxt": np.ascontiguousarray(np.asarray(W_peer, np.float32)[:, :128].T).astype(BF),
        "wpet": np.ascontiguousarray(np.asarray(W_peer, np.float32)[:, 128:].T).astype(BF),
        "bego": np.asarray(b_ego, np.float32).reshape(1, 64).astype(BF),
        "bpeer": np.asarray(b_peer, np.float32).reshape(1, 96).astype(BF),
        "iota": np.broadcast_to(np.arange(128, dtype=np.float32), (128, 128)).astype(BF).copy(),
        "ident": np.eye(128, dtype=np.float32).astype(BF),
    }
    return consts


_CACHE = {}
RUN_KWARGS = {}


def kernel(x, edge_attrs, W_peer, b_peer, W_ego, b_ego, W_edge, b_edge, edge_index):
    x = np.asarray(x)
    edge_attrs = np.asarray(edge_attrs)
    edge_index = np.asarray(edge_index)
    N, E = x.shape[0], edge_attrs.shape[0]

    row = edge_index[1].astype(np.int64)
    col = edge_index[0].astype(np.int64)
    C = 15
    CH = 8
    ncores = 8
    NPC = N // ncores
    NBLK = (NPC + 127) // 128
    deg = np.bincount(row, minlength=N)
    ovn = np.maximum(deg - C, 0)
    nodes = np.arange(N)
    bkey = (nodes // NPC) * NBLK + (nodes % NPC) // 128
    ovblk = np.bincount(bkey, weights=ovn.astype(np.float64), minlength=NBLK * ncores)
    t_ov = max(3, int(np.ceil(ovblk.max() / 128.0)))

    # phase-B dual-grid overflow sizing (per dest block x source half)
    halfs = (col // NPC) // (ncores // 2)
    degh = np.bincount(row * 2 + halfs, minlength=2 * N)
    ovn2 = np.maximum(degh - CH, 0)
    bkey2 = np.repeat(bkey, 2) * 2 + np.tile([0, 1], N)
    ovblk2 = np.bincount(bkey2, weights=ovn2.astype(np.float64),
                         minlength=2 * NBLK * ncores)
    t_ov2 = max(2, int(np.ceil(ovblk2.max() / 128.0)))

    cfg = Cfg(N=N, E=E, ncores=ncores, C=C, t_ov=t_ov, CH=CH, t_ov2=t_ov2)
    key = cfg.key()
    if key not in _CACHE:
        _CACHE[key] = build_program(cfg)
    nc = _CACHE[key]

    in_maps = host_prep(cfg, x, edge_attrs, edge_index)
    consts = make_consts(cfg, W_peer, b_peer, W_ego, b_ego, W_edge, b_edge)
    for m in in_maps:
        m.update(consts)

    res = run_bass_kernel_spmd(nc, in_maps, core_ids=list(range(cfg.ncores)),
                               **RUN_KWARGS)
    out = np.empty((N, cfg.OUTD), np.float32)
    for r in range(cfg.ncores):
        blk = np.asarray(res.results[r]["out"]).astype(np.float32)
        out[r * cfg.NPC:(r + 1) * cfg.NPC] = blk[:cfg.NPC, :cfg.OUTD]
    kernel.last_result = res
    return out


# revision 37
# speedup vs baseline: 2.8424x; 1.0142x over previous
"""Trainium2 Bass kernel: CANE FeatureEmbedding GNN message passing.

Strategy (node-range sharding, 8 cores), v2:
  - Nodes range-partitioned; edges assigned to the core owning their
    DESTINATION (row = edge_index[1]).
  - Algebraic collapse of the peer branch (gather and scatter both use `row`):
        h_peer[i] = relu( sqrt(deg_i)*(W_px x_i + b_p) + deg_i^-1/2 * (W_pe S_ea[i]) )
    removing the [E,160]x[160,96] per-edge MLP.
  - Remaining per-edge work: h_e = relu(ea @ W_edge.T + b_edge), segment sums
    of ea and h_e over row, and M[i] = sum_{e: row=i} h_e_agg[col[e]].
  - Slot grid: node v gets C slots; edge k of v goes to (block v//128,
    partition v%128, slot k). One bf16 matmul per slot tile accumulates S_ea
    in PSUM; h_e pre-acts land per-slot, relu on the Act engine, slot-sum on
    DVE straight into PSUM, overflow (deg>C) via one-hot scatter matmuls.
  - h_e_agg shared across cores via a compact fp16 AllGather, then locally
    re-padded into two 256B-stride half-tables (<=32767 rows each, int16
    indexable). M is built with dma_gather (64B payload / 256B stride, <=1024
    indices per instruction -- the SWDGE ring cap) over a second slot grid
    split by source half, reduced along (half, slot) on DVE into PSUM with
    one-hot matmul accumulation for overflow.
  - Everything streams in fp16 (4x finer mantissa than bf16 at identical
    cost); PSUM accumulation in fp32; output written as [PADN, 256] fp16 rows
    (cols 224:256 pad) and upcast host-side.
"""

import numpy as np

import concourse.bass as bass
import concourse.mybir as mybir
import concourse.tile as tile
from concourse import bacc
from concourse._compat import get_trn_type
from concourse.bass import IndirectOffsetOnAxis
from concourse.bass_utils import run_bass_kernel_spmd

F32 = mybir.dt.float32
BF16 = mybir.dt.float16  # fp16: 4x finer mantissa than bf16, same perf
I32 = mybir.dt.int32
I16 = mybir.dt.int16
AX = mybir.AxisListType
OP = mybir.AluOpType
ACT = mybir.ActivationFunctionType


class Cfg:
    def __init__(self, N=50000, E=800000, ncores=8, C=15, t_ov=3, GB=4, EB=4,
                 CH=8, t_ov2=2):
        self.N = N
        self.E = E
        self.ncores = ncores
        self.NPC = N // ncores            # nodes per core
        self.NBLK = (self.NPC + 127) // 128
        self.PADN = self.NBLK * 128       # padded nodes per core
        self.C = C                        # grid slots per node
        self.t_ov = t_ov                  # overflow tiles per block
        self.TPB = C + t_ov               # tiles per block
        self.GB = GB                      # blocks per gather instruction
        self.EB = EB                      # blocks per eat-load DMA
        self.CH = CH                      # phase-B main slots per (node, half)
        self.t_ov2 = t_ov2                # phase-B overflow tiles per (block, half)
        self.THALF = CH + t_ov2           # phase-B tiles per (block, half)
        self.IW = (self.THALF * 128 + 15) // 16   # int16 idx cols per gather
        self.ND = 128
        self.ED = 32
        self.OUTD = 224
        self.OUTP = 256                   # padded out row

    def key(self):
        return (self.N, self.E, self.ncores, self.C, self.t_ov, self.GB,
                self.EB, self.CH, self.t_ov2)


def raw_dma_gather(nc, out_ap, in_ap, idxs_ap, num_idxs, elem_size, elem_step):
    """nc.gpsimd.dma_gather minus the %256 payload assert (the 256B rule
    applies to the row STRIDE, which elem_step satisfies; the ucode packetizes
    the payload at any size)."""
    gp = nc.gpsimd
    stride_bytes = elem_step * mybir.dt.size(in_ap.dtype)
    assert stride_bytes % 256 == 0 and stride_bytes // 256 < 256
    _in_ap = gp.lower_ap_dma(in_ap, for_custom_bir_dma=True)
    _idxs_ap = gp.lower_ap(idxs_ap)
    _out_ap = gp.lower_ap(out_ap)
    return gp.add_instruction(
        mybir.InstDMAGatherAnt(
            name=f"I-{nc.next_id()}",
            ins=[*_in_ap, _idxs_ap, gp.lower_val_access(gp.to_reg(num_idxs))],
            outs=[_out_ap],
            transpose=False,
            num_idxs=num_idxs,
            elem_size=elem_size,
            stride_bytes_256=stride_bytes // 256,
            gen_mode=0,
            single_packet=True,
            queue_num=0,
            sbuf_tokens_per_rank=0,
            sbuf_free_dim_per_rank=0,
            sbuf_free_dim_pad_per_rank=0,
            sbuf_byte_offset=0,
        )
    )


def build_program(cfg, skip=()):
    """Build the SPMD Bass program (same NEFF for all cores).

    skip: {"gather", "slotmm", "overflow", "phasec", "collective"} — timing
    experiment knobs only (results are wrong when used)."""
    skip = set(skip)
    nc = bacc.Bacc(get_trn_type() or "TRN2", target_bir_lowering=False, debug=True)
    NBLK, TPB, C, t_ov, PADN = cfg.NBLK, cfg.TPB, cfg.C, cfg.t_ov, cfg.PADN
    GB, EB, OUTP = cfg.GB, cfg.EB, cfg.OUTP
    TOT = NBLK * TPB

    eat = nc.declare_dram_parameter("eat", [33, TOT * 128], BF16, isOutput=False)
    gidx16 = nc.declare_dram_parameter(
        "gidx16", [128, NBLK * 2 * cfg.IW], I16, isOutput=False)
    rlov2 = nc.declare_dram_parameter("rlov2", [128, NBLK * 2 * cfg.t_ov2], F32,
                                      isOutput=False)
    rlov = nc.declare_dram_parameter("rlov", [128, NBLK * t_ov], F32, isOutput=False)
    xt = nc.declare_dram_parameter("xt", [128, PADN], BF16, isOutput=False)
    dinvp = nc.declare_dram_parameter("dinv", [128, NBLK], F32, isOutput=False)
    sqdp = nc.declare_dram_parameter("sqd", [128, NBLK], F32, isOutput=False)
    deginvp = nc.declare_dram_parameter("deginv", [128, NBLK], F32, isOutput=False)
    rconst = nc.declare_dram_parameter("rconst", [33, 64], BF16, isOutput=False)
    wegot = nc.declare_dram_parameter("wegot", [128, 64], BF16, isOutput=False)
    wpxt = nc.declare_dram_parameter("wpxt", [128, 96], BF16, isOutput=False)
    wpet = nc.declare_dram_parameter("wpet", [32, 96], BF16, isOutput=False)
    bego = nc.declare_dram_parameter("bego", [1, 64], BF16, isOutput=False)
    bpeer = nc.declare_dram_parameter("bpeer", [1, 96], BF16, isOutput=False)
    iota = nc.declare_dram_parameter("iota", [128, 128], BF16, isOutput=False)
    ident = nc.declare_dram_parameter("ident", [128, 128], BF16, isOutput=False)
    outp = nc.declare_dram_parameter("out", [PADN, OUTP], BF16, isOutput=True)

    with tile.TileContext(nc) as tc:
        with (
            tc.tile_pool(name="const", bufs=1) as cp,
            tc.tile_pool(name="resident", bufs=1) as rp,
            tc.tile_pool(name="dram", bufs=1, space="DRAM") as dp,
        ):
            rc = cp.tile([33, 64], BF16)
            nc.sync.dma_start(rc[:], rconst[:])
            wego_sb = cp.tile([128, 64], BF16)
            nc.sync.dma_start(wego_sb[:], wegot[:])
            wpx_sb = cp.tile([128, 96], BF16)
            nc.sync.dma_start(wpx_sb[:], wpxt[:])
            wpe_sb = cp.tile([32, 96], BF16)
            nc.sync.dma_start(wpe_sb[:], wpet[:])
            bego_sb = cp.tile([1, 64], BF16)
            nc.sync.dma_start(bego_sb[:], bego[:])
            bpeer_sb = cp.tile([1, 96], BF16)
            nc.sync.dma_start(bpeer_sb[:], bpeer[:])
            iota_sb = cp.tile([128, 128], BF16)
            nc.sync.dma_start(iota_sb[:], iota[:])
            ident_sb = cp.tile([128, 128], BF16)
            nc.sync.dma_start(ident_sb[:], ident[:])
            ones_sb = cp.tile([1, 128], BF16)
            nc.gpsimd.memset(ones_sb[:], 1.0)

            xt_sb = rp.tile([128, PADN], BF16)
            nc.sync.dma_start(xt_sb[:], xt[:])
            gidx16_sb = rp.tile([128, NBLK * 2 * cfg.IW], I16)
            nc.sync.dma_start(gidx16_sb[:], gidx16[:])
            rlov2_sb = rp.tile([128, NBLK * 2 * cfg.t_ov2], F32)
            nc.sync.dma_start(rlov2_sb[:], rlov2[:])
            rlov_sb = rp.tile([128, NBLK * t_ov], F32)
            nc.sync.dma_start(rlov_sb[:], rlov[:])
            dinv_sb = rp.tile([128, NBLK], F32)
            nc.sync.dma_start(dinv_sb[:], dinvp[:])
            sqd_sb = rp.tile([128, NBLK], F32)
            nc.sync.dma_start(sqd_sb[:], sqdp[:])
            deginv_sb = rp.tile([128, NBLK], F32)
            nc.sync.dma_start(deginv_sb[:], deginvp[:])

            heagg_bf = rp.tile([128, NBLK * 32], BF16)   # h_e_agg, bf16
            ts_cache = rp.tile([128, NBLK * 32], BF16)   # dinv * S_ea, bf16
            oh_cache = rp.tile([128, NBLK * t_ov * 128], BF16)  # one-hots
            outst = rp.tile([128, NBLK * OUTP], BF16)    # output staging

            # zag holds all cores' h_e_agg (p-major rows within each core
            # section); zagA/zagB are 256B-stride padded halves for dma_gather
            zloc = dp.tile([PADN, 32], BF16)
            zag = dp.tile([cfg.ncores * PADN, 32], BF16, addr_space="Shared")
            zagA = dp.tile([cfg.ncores // 2 * PADN, 128], BF16)
            zagB = dp.tile([cfg.ncores // 2 * PADN, 128], BF16)

            # ---------------- Phase A: per-edge MLP + segment sums ----------
            with (
                tc.tile_pool(name="eatp", bufs=3) as eatp,
                tc.tile_pool(name="workA", bufs=4) as wp,
                tc.tile_pool(name="psA", bufs=2, space="PSUM") as psA,
                tc.tile_pool(name="psB", bufs=2, space="PSUM") as psB,
                tc.tile_pool(name="psP", bufs=2, space="PSUM") as psP,
                tc.tile_pool(name="psO", bufs=2, space="PSUM") as psO,
            ):
                ech = None
                for b in range(NBLK):
                    if b % EB == 0:
                        nch = min(EB, NBLK - b)
                        ech = eatp.tile([33, nch * TPB * 128], BF16, tag="ech")
                        nc.sync.dma_start(
                            ech[:], eat[:, b * TPB * 128:(b + nch) * TPB * 128])
                    e0 = (b % EB) * TPB * 128
                    bankA = psA.tile([128, 32], F32, tag="bankA")
                    bankB = psB.tile([128, C * 32], F32, tag="bankB")
                    for j in range(C):
                        if "slotmm" in skip:
                            break
                        lh = ech[:, e0 + j * 128:e0 + (j + 1) * 128]
                        nc.tensor.matmul(bankA[:], lh, rc[:, 0:32],
                                         start=(j == 0), stop=False)
                        nc.tensor.matmul(bankB[:, j * 32:(j + 1) * 32], lh,
                                         rc[:, 32:64],
                                         start=(j == 0), stop=(j == C - 1))
                    # overflow: pre-act for t_ov tiles in one PSUM tile
                    pc = psP.tile([128, t_ov * 64], F32, tag="pc")
                    ov = wp.tile([128, t_ov * 64], BF16, tag="ov")
                    for o in range(t_ov):
                        lh = ech[:, e0 + (C + o) * 128:e0 + (C + o + 1) * 128]
                        nc.tensor.matmul(pc[:, o * 64:(o + 1) * 64], lh, rc[:, 0:64],
                                         start=(o == 0), stop=(o == t_ov - 1))
                    # ea part: copy (Act); h_e part: relu (DVE)
                    nc.scalar.copy(
                        ov[:].rearrange("p (t x) -> p t x", x=64)[:, :, 0:32],
                        pc[:].rearrange("p (t x) -> p t x", x=64)[:, :, 0:32])
                    nc.vector.tensor_scalar_max(
                        ov[:].rearrange("p (t x) -> p t x", x=64)[:, :, 32:64],
                        pc[:].rearrange("p (t x) -> p t x", x=64)[:, :, 32:64],
                        0.0)
                    # one-hot scatter matrices (cached for Phase B reuse)
                    for o in range(t_ov):
                        k = b * t_ov + o
                        nc.vector.tensor_scalar(
                            out=oh_cache[:, k * 128:(k + 1) * 128], in0=iota_sb[:],
                            scalar1=rlov_sb[:, k:k + 1], scalar2=None,
                            op0=OP.is_equal,
                        )
                        # fold overflow ea into bankA accumulation group
                        nc.tensor.matmul(bankA[:], oh_cache[:, k * 128:(k + 1) * 128],
                                         ov[:, o * 64:o * 64 + 32],
                                         start=False, stop=(o == t_ov - 1))
                    # t_s = (1/deg) * S_ea (kept bf16 for Phase C transpose;
                    # the final sqd scale rides the peer relu)
                    nc.vector.tensor_scalar_mul(
                        ts_cache[:, b * 32:(b + 1) * 32], bankA[:],
                        deginv_sb[:, b:b + 1])
                    # h_e slots: relu on Act, slot-sum on DVE into PSUM
                    relu_st = wp.tile([128, C * 32], BF16, tag="relu")
                    nc.scalar.activation(relu_st[:], bankB[:], ACT.Relu)
                    t_he = wp.tile([128, 32], F32, tag="the")
                    nc.vector.tensor_reduce(
                        t_he[:],
                        relu_st[:].rearrange("p (j c) -> p c j", j=C),
                        axis=AX.X, op=OP.add,
                    )
                    bankO = psO.tile([128, 32], F32, tag="bankO")
                    for o in range(t_ov):
                        k = b * t_ov + o
                        nc.tensor.matmul(bankO[:], oh_cache[:, k * 128:(k + 1) * 128],
                                         ov[:, o * 64 + 32:(o + 1) * 64],
                                         start=(o == 0), stop=(o == t_ov - 1))
                    nc.vector.tensor_tensor(
                        out=heagg_bf[:, b * 32:(b + 1) * 32],
                        in0=t_he[:], in1=bankO[:], op=OP.add)
                    # outst[64:96] = h_e_agg; pad cols 224:256 need some value
                    nc.vector.tensor_copy(
                        outst[:, b * OUTP + 64:b * OUTP + 96],
                        heagg_bf[:, b * 32:(b + 1) * 32])
                    nc.vector.tensor_copy(
                        outst[:, b * OUTP + 224:b * OUTP + 256],
                        heagg_bf[:, b * 32:(b + 1) * 32])

            # Share h_e_agg via compact AllGather ("Shared" DRAM is NOT
            # coherently cross-core writable in this runtime, so direct
            # scatter-shares don't work); then locally re-pad each half into
            # a 256B-stride table for the int16 dma_gather.
            nc.sync.dma_start(
                zloc[:].rearrange("(p b) c -> p (b c)", p=128),
                heagg_bf[:],
            )
            if "collective" not in skip:
                nc.gpsimd.collective_compute(
                    "AllGather", OP.bypass,
                    ins=[zloc.opt()], outs=[zag.opt()],
                    replica_groups=[list(range(cfg.ncores))],
                )
            HROWS = cfg.ncores // 2 * PADN
            nc.sync.dma_start(zagA[:, 0:32], zag[0:HROWS, :])
            nc.sync.dma_start(zagB[:, 0:32], zag[HROWS:2 * HROWS, :])

            # ------------- Phase C: node MLPs (overlaps the collective) -----
            if "phasec" not in skip:
                with (
                    tc.tile_pool(name="workC", bufs=4) as wc,
                    tc.tile_pool(name="ps1", bufs=2, space="PSUM") as ps1,
                    tc.tile_pool(name="ps3", bufs=2, space="PSUM") as ps3,
                    tc.tile_pool(name="psT", bufs=2, space="PSUM") as psT,
                ):
                    for b in range(NBLK):
                        xb = xt_sb[:, b * 128:(b + 1) * 128]
                        # h_ego = relu(x W_ego^T + b_ego)
                        p1 = ps1.tile([128, 64], F32, tag="p1")
                        nc.tensor.matmul(p1[:], ones_sb[:], bego_sb[:],
                                         start=True, stop=False)
                        nc.tensor.matmul(p1[:], xb, wego_sb[:],
                                         start=False, stop=True)
                        nc.scalar.activation(
                            outst[:, b * OUTP:b * OUTP + 64], p1[:], ACT.Relu)
                        # h_peer = relu(sqd * (W_px x + b_p + W_pe ((1/deg) S_ea)))
                        pt = psT.tile([32, 128], BF16, tag="pt")
                        nc.tensor.matmul(pt[:], ts_cache[:, b * 32:(b + 1) * 32],
                                         ident_sb[:], is_transpose=True,
                                         start=True, stop=True)
                        seat = wc.tile([32, 128], BF16, tag="seat")
                        nc.scalar.copy(seat[:], pt[:])
                        p3 = ps3.tile([128, 96], F32, tag="p3")
                        nc.tensor.matmul(p3[:], ones_sb[:], bpeer_sb[:],
                                         start=True, stop=False)
                        nc.tensor.matmul(p3[:], xb, wpx_sb[:],
                                         start=False, stop=False)
                        nc.tensor.matmul(p3[:], seat[:], wpe_sb[:],
                                         start=False, stop=True)
                        nc.scalar.activation(
                            outst[:, b * OUTP + 128:b * OUTP + 224], p3[:],
                            ACT.Relu, scale=sqd_sb[:, b:b + 1])

            # ------------- Phase B: gather + M + output writes --------------
            # dma_gather is limited to 1024 indices (8 tiles) per instruction,
            # so each (block, half) span of THALF tiles is split at 8 tiles.
            CH2, tov2, THALF, IW = cfg.CH, cfg.t_ov2, cfg.THALF, cfg.IW
            with (
                tc.tile_pool(name="workB", bufs=4) as wb,
                tc.tile_pool(name="psM", bufs=3, space="PSUM") as psM,
            ):
                for g0 in range(0, NBLK, GB):
                    ng = min(GB, NBLK - g0)
                    g = wb.tile([128, ng * 2 * THALF * 32], BF16, tag="g")
                    if "gather" not in skip:
                        for br in range(ng):
                            b = g0 + br
                            for h, ztab in ((0, zagA), (1, zagB)):
                                cbase = (br * 2 + h) * THALF
                                ibase = (b * 2 + h) * IW
                                for k0 in range(0, THALF, 8):
                                    nt = min(8, THALF - k0)
                                    raw_dma_gather(
                                        nc,
                                        g[:, (cbase + k0) * 32:
                                          (cbase + k0 + nt) * 32].rearrange(
                                            "p (t e) -> p t e", e=32),
                                        ztab[:, 0:32],
                                        gidx16_sb[:, ibase + k0 * 8:
                                                  ibase + (k0 + nt) * 8],
                                        num_idxs=nt * 128, elem_size=32,
                                        elem_step=128)
                    for br in range(ng):
                        b = g0 + br
                        s0 = br * 2 * THALF * 32
                        m_sb = wb.tile([128, 32], F32, tag="msb")
                        nc.vector.tensor_reduce(
                            m_sb[:],
                            g[:, s0:s0 + 2 * THALF * 32].rearrange(
                                "p (h t c) -> p c h t", h=2, c=32)[:, :, :, 0:CH2],
                            axis=AX.XY, op=OP.add,
                        )
                        pm = psM.tile([128, 32], F32, tag="pm")
                        oh2 = wb.tile([128, 2 * tov2 * 128], BF16, tag="oh2")
                        nmm = 2 * tov2
                        for h in range(2):
                            for o in range(tov2):
                                k = h * tov2 + o
                                nc.vector.tensor_scalar(
                                    out=oh2[:, k * 128:(k + 1) * 128], in0=iota_sb[:],
                                    scalar1=rlov2_sb[:, (b * 2 + h) * tov2 + o:
                                                     (b * 2 + h) * tov2 + o + 1],
                                    scalar2=None, op0=OP.is_equal,
                                )
                                nc.tensor.matmul(
                                    pm[:], oh2[:, k * 128:(k + 1) * 128],
                                    g[:, ((br * 2 + h) * THALF + CH2 + o) * 32:
                                      ((br * 2 + h) * THALF + CH2 + o + 1) * 32],
                                    start=(k == 0), stop=(k == nmm - 1))
                        t_m = wb.tile([128, 32], F32, tag="tm")
                        nc.vector.tensor_tensor(
                            out=t_m[:], in0=m_sb[:], in1=pm[:], op=OP.add)
                        nc.scalar.activation(
                            outst[:, b * OUTP + 96:b * OUTP + 128], t_m[:],
                            ACT.Copy, scale=dinv_sb[:, b:b + 1])
                    nc.sync.dma_start(
                        outp[g0 * 128:(g0 + ng) * 128, :].rearrange(
                            "(q p) c -> p q c", p=128),
                        outst[:, g0 * OUTP:(g0 + ng) * OUTP].rearrange(
                            "p (q c) -> p q c", c=OUTP),
                    )
    nc.compile()
    return nc


def host_prep(cfg, x, edge_attrs, edge_index):
    """Shard + lay out inputs for the slot-grid kernel. Pure index work + O(N)
    scalar prep (degree normalizers); all O(E*H)/O(N*H) FP math runs on device."""
    BF = np.float16
    N, E, C, NBLK, TPB, t_ov, NPC, PADN = (cfg.N, cfg.E, cfg.C, cfg.NBLK,
                                           cfg.TPB, cfg.t_ov, cfg.NPC, cfg.PADN)
    row = np.asarray(edge_index[1]).astype(np.int64)
    col = np.asarray(edge_index[0]).astype(np.int64)
    ea = np.asarray(edge_attrs, dtype=np.float32)
    xf = np.asarray(x, dtype=np.float32)

    deg = np.bincount(row, minlength=N)
    degf = np.maximum(deg, 1).astype(np.float64)
    dinv = np.where(deg > 0, degf ** -0.5, 0.0).astype(np.float32)
    sqd = np.sqrt(deg.astype(np.float64)).astype(np.float32)

    core = row // NPC
    lrow = row - core * NPC
    blk = lrow // 128
    part = lrow % 128

    # rank of each edge within its destination node
    order = np.argsort(row, kind="stable")
    sorted_row = row[order]
    starts = np.searchsorted(sorted_row, np.arange(N), side="left")
    rank = np.empty(E, np.int64)
    rank[order] = np.arange(E) - starts[sorted_row]

    is_grid = rank < C
    ove = np.where(~is_grid)[0]
    ovkey = core[ove] * NBLK + blk[ove]
    o_order = np.argsort(ovkey, kind="stable")
    ove = ove[o_order]
    okey_sorted = ovkey[o_order]
    ostarts = np.searchsorted(okey_sorted, np.arange(NBLK * cfg.ncores), side="left")
    opos = np.arange(ove.size) - ostarts[okey_sorted]
    otile = C + opos // 128
    opart = opos % 128
    if ove.size and otile.max() >= TPB:
        raise ValueError("overflow tiles exceeded; raise t_ov")

    tile_idx = np.empty(E, np.int64)
    tpart = np.empty(E, np.int64)
    ge = np.where(is_grid)[0]
    tile_idx[ge] = blk[ge] * TPB + rank[ge]
    tpart[ge] = part[ge]
    tile_idx[ove] = blk[ove] * TPB + otile
    tpart[ove] = opart

    # ---- Phase-B dual gather grid: edges split by source half ----------
    # half h = src_core // 4; within-half row (p-major per core section):
    #   (src_core % 4) * PADN + (l % 128) * NBLK + l // 128,  l = col % NPC
    CH, t_ov2, THALF, IW = cfg.CH, cfg.t_ov2, cfg.THALF, cfg.IW
    src_core = col // NPC
    lcol = col % NPC
    halfs = src_core // (cfg.ncores // 2)
    zrow16 = ((src_core % (cfg.ncores // 2)) * PADN
              + (lcol % 128) * NBLK + lcol // 128)
    ZPADH = 127 * NBLK + (NBLK - 1)   # half-core-0 pad node -> zeros

    # rank of each edge within (dest node, half)
    key2 = row * 2 + halfs
    order2 = np.argsort(key2, kind="stable")
    sk2 = key2[order2]
    starts2 = np.searchsorted(sk2, np.arange(2 * N), side="left")
    rank2 = np.empty(E, np.int64)
    rank2[order2] = np.arange(E) - starts2[sk2]

    is_grid2 = rank2 < CH
    ove2 = np.where(~is_grid2)[0]
    ovkey2 = (core[ove2] * NBLK + blk[ove2]) * 2 + halfs[ove2]
    o_order2 = np.argsort(ovkey2, kind="stable")
    ove2 = ove2[o_order2]
    ok2_sorted = ovkey2[o_order2]
    ostarts2 = np.searchsorted(ok2_sorted, np.arange(NBLK * cfg.ncores * 2),
                               side="left")
    opos2 = np.arange(ove2.size) - ostarts2[ok2_sorted]
    otile2 = CH + opos2 // 128
    opart2 = opos2 % 128
    if ove2.size and otile2.max() >= THALF:
        raise ValueError("phase-B overflow tiles exceeded; raise t_ov2")

    tile2 = np.empty(E, np.int64)
    tpart2 = np.empty(E, np.int64)
    ge2 = np.where(is_grid2)[0]
    tile2[ge2] = rank2[ge2]
    tpart2[ge2] = part[ge2]
    tile2[ove2] = otile2
    tpart2[ove2] = opart2

    TOTC = NBLK * TPB
    in_maps = []
    for r in range(cfg.ncores):
        sel = core == r
        e_idx = np.where(sel)[0]
        t_i = tile_idx[e_idx]
        t_p = tpart[e_idx]
        colpos = t_i * 128 + t_p

        EAT = np.zeros((33, TOTC * 128), np.float32)
        EAT[:32, colpos] = ea[e_idx].T
        EAT[32, colpos] = 1.0

        # int16 gather indices: list position i = tile*128 + dest_partition,
        # stored 16-partition-wrapped [16g + i%16, i//16], replicated x8
        ipos = ((blk[e_idx] * 2 + halfs[e_idx]) * (THALF * 128)
                + tile2[e_idx] * 128 + tpart2[e_idx])
        flat = np.full(NBLK * 2 * THALF * 128, ZPADH, np.int32)
        flat[ipos] = zrow16[e_idx]
        wrap = (flat.reshape(NBLK * 2, IW, 16).transpose(2, 0, 1)
                .reshape(16, NBLK * 2 * IW).astype(np.int16))
        GIDX16 = np.tile(wrap, (8, 1))

        RLOV = np.full((128, NBLK * t_ov), 200.0, np.float32)
        ov_r = ove[core[ove] == r]
        op_r = opart[core[ove] == r]
        ot_r = otile[core[ove] == r]
        ob_r = blk[ov_r]
        RLOV[op_r, ob_r * t_ov + (ot_r - C)] = part[ov_r].astype(np.float32)

        RLOV2 = np.full((128, NBLK * 2 * t_ov2), 200.0, np.float32)
        sel2 = core[ove2] == r
        ov2_r = ove2[sel2]
        RLOV2[opart2[sel2],
              (blk[ov2_r] * 2 + halfs[ov2_r]) * t_ov2
              + (otile2[sel2] - CH)] = part[ov2_r].astype(np.float32)

        lo, hi = r * NPC, (r + 1) * NPC
        XT = np.zeros((128, PADN), np.float32)
        XT[:, :NPC] = xf[lo:hi].T
        dl = np.zeros(PADN, np.float32)
        dl[:NPC] = dinv[lo:hi]
        sl = np.zeros(PADN, np.float32)
        sl[:NPC] = sqd[lo:hi]
        gl = np.zeros(PADN, np.float32)
        gl[:NPC] = dinv[lo:hi] ** 2          # 1/deg (0 for deg==0)
        DINV = dl.reshape(NBLK, 128).T.copy()
        SQD = sl.reshape(NBLK, 128).T.copy()
        DEGINV = gl.reshape(NBLK, 128).T.copy()

        m = {
            "eat": EAT.astype(BF), "gidx16": GIDX16, "rlov": RLOV,
            "rlov2": RLOV2, "xt": XT.astype(BF), "dinv": DINV, "sqd": SQD,
            "deginv": DEGINV,
        }
        in_maps.append(m)
    return in_maps


def make_consts(cfg, W_peer, b_peer, W_ego, b_ego, W_edge, b_edge):
    BF = np.float16
    RCONST = np.zeros((33, 64), np.float32)
    RCONST[:32, :32] = np.eye(32, dtype=np.float32)
    RCONST[:32, 32:64] = np.asarray(W_edge, np.float32).T
    RCONST[32, 32:64] = np.asarray(b_edge, np.float32)
    consts = {
        "rconst": RCONST.astype(BF),
        "wegot": np.ascontiguousarray(np.asarray(W_ego, np.float32).T).astype(BF),
        "wpxt": np.ascontiguousarray(np.asarray(W_peer, np.float32)[:, :128].T).astype(BF),
        "wpet": np.ascontiguousarray(np.asarray(W_peer, np.float32)[:, 128:].T).astype(BF),
        "bego": np.asarray(b_ego, np.float32).reshape(1, 64).astype(BF),
        "bpeer": np.asarray(b_peer, np.float32).reshape(1, 96).astype(BF),
        "iota": np.broadcast_to(np.arange(128, dtype=np.float32), (128, 128)).astype(BF).copy(),
        "ident": np.eye(128, dtype=np.float32).astype(BF),
    }
    return consts


_CACHE = {}
RUN_KWARGS = {}


def kernel(x, edge_attrs, W_peer, b_peer, W_ego, b_ego, W_edge, b_edge, edge_index):
    x = np.asarray(x)
    edge_attrs = np.asarray(edge_attrs)
    edge_index = np.asarray(edge_index)
    N, E = x.shape[0], edge_attrs.shape[0]

    row = edge_index[1].astype(np.int64)
    col = edge_index[0].astype(np.int64)
    C = 15
    CH = 8
    ncores = 8
    NPC = N // ncores
    NBLK = (NPC + 127) // 128
    deg = np.bincount(row, minlength=N)
    ovn = np.maximum(deg - C, 0)
    nodes = np.arange(N)
    bkey = (nodes // NPC) * NBLK + (nodes % NPC) // 128
    ovblk = np.bincount(bkey, weights=ovn.astype(np.float64), minlength=NBLK * ncores)
    t_ov = max(3, int(np.ceil(ovblk.max() / 128.0)))

    # phase-B dual-grid overflow sizing (per dest block x source half)
    halfs = (col // NPC) // (ncores // 2)
    degh = np.bincount(row * 2 + halfs, minlength=2 * N)
    ovn2 = np.maximum(degh - CH, 0)
    bkey2 = np.repeat(bkey, 2) * 2 + np.tile([0, 1], N)
    ovblk2 = np.bincount(bkey2, weights=ovn2.astype(np.float64),
                         minlength=2 * NBLK * ncores)
    t_ov2 = max(2, int(np.ceil(ovblk2.max() / 128.0)))

    cfg = Cfg(N=N, E=E, ncores=ncores, C=C, t_ov=t_ov, CH=CH, t_ov2=t_ov2)
    key = cfg.key()
    if key not in _CACHE:
        _CACHE[key] = build_program(cfg)
    nc = _CACHE[key]

    in_maps = host_prep(cfg, x, edge_attrs, edge_index)
    consts = make_consts(cfg, W_peer, b_peer, W_ego, b_ego, W_edge, b_edge)
    for m in in_maps:
        m.update(consts)

    res = run_bass_kernel_spmd(nc, in_maps, core_ids=list(range(cfg.ncores)),
                               **RUN_KWARGS)
    out = np.empty((N, cfg.OUTD), np.float32)
    for r in range(cfg.ncores):
        blk = np.asarray(res.results[r]["out"]).astype(np.float32)
        out[r * cfg.NPC:(r + 1) * cfg.NPC] = blk[:cfg.NPC, :cfg.OUTD]
    kernel.last_result = res
    return out


# revision 39
# speedup vs baseline: 3.3965x; 1.1949x over previous
"""Trainium2 Bass kernel: CANE FeatureEmbedding GNN message passing.

Strategy (node-range sharding, 8 cores), v2:
  - Nodes range-partitioned; edges assigned to the core owning their
    DESTINATION (row = edge_index[1]).
  - Algebraic collapse of the peer branch (gather and scatter both use `row`):
        h_peer[i] = relu( sqrt(deg_i)*(W_px x_i + b_p) + deg_i^-1/2 * (W_pe S_ea[i]) )
    removing the [E,160]x[160,96] per-edge MLP.
  - Remaining per-edge work: h_e = relu(ea @ W_edge.T + b_edge), segment sums
    of ea and h_e over row, and M[i] = sum_{e: row=i} h_e_agg[col[e]].
  - Slot grid: node v gets C slots; edge k of v goes to (block v//128,
    partition v%128, slot k). One bf16 matmul per slot tile accumulates S_ea
    in PSUM; h_e pre-acts land per-slot, relu on the Act engine, slot-sum on
    DVE straight into PSUM, overflow (deg>C) via one-hot scatter matmuls.
  - h_e_agg shared across cores via a compact fp16 AllGather, then locally
    re-padded into two 256B-stride half-tables (<=32767 rows each, int16
    indexable). M is built with dma_gather (64B payload / 256B stride, <=1024
    indices per instruction -- the SWDGE ring cap) over a second slot grid
    split by source half, reduced along (half, slot) on DVE into PSUM with
    one-hot matmul accumulation for overflow.
  - Everything streams in fp16 (4x finer mantissa than bf16 at identical
    cost); PSUM accumulation in fp32; output written as [PADN, 256] fp16 rows
    (cols 224:256 pad) and upcast host-side.
"""

import numpy as np

import concourse.bass as bass
import concourse.mybir as mybir
import concourse.tile as tile
from concourse import bacc
from concourse._compat import get_trn_type
from concourse.bass import IndirectOffsetOnAxis
from concourse.bass_utils import run_bass_kernel_spmd

F32 = mybir.dt.float32
BF16 = mybir.dt.float16  # fp16: 4x finer mantissa than bf16, same perf
I32 = mybir.dt.int32
I16 = mybir.dt.int16
AX = mybir.AxisListType
OP = mybir.AluOpType
ACT = mybir.ActivationFunctionType


class Cfg:
    def __init__(self, N=50000, E=800000, ncores=8, C=15, t_ov=3, GB=4, EB=4,
                 CH=8, t_ov2=2):
        self.N = N
        self.E = E
        self.ncores = ncores
        self.NPC = N // ncores            # nodes per core
        self.NBLK = (self.NPC + 127) // 128
        self.PADN = self.NBLK * 128       # padded nodes per core
        self.C = C                        # grid slots per node
        self.t_ov = t_ov                  # overflow tiles per block
        self.TPB = C + t_ov               # tiles per block
        self.GB = GB                      # blocks per gather instruction
        self.EB = EB                      # blocks per eat-load DMA
        self.CH = CH                      # phase-B main slots per (node, half)
        self.t_ov2 = t_ov2                # phase-B overflow tiles per (block, half)
        self.THALF = CH + t_ov2           # phase-B tiles per (block, half)
        self.IW = (self.THALF * 128 + 15) // 16   # int16 idx cols per gather
        self.ND = 128
        self.ED = 32
        self.OUTD = 224
        self.OUTP = 256                   # padded out row

    def key(self):
        return (self.N, self.E, self.ncores, self.C, self.t_ov, self.GB,
                self.EB, self.CH, self.t_ov2)


def raw_dma_gather(nc, out_ap, in_ap, idxs_ap, num_idxs, elem_size, elem_step):
    """nc.gpsimd.dma_gather minus the %256 payload assert (the 256B rule
    applies to the row STRIDE, which elem_step satisfies; the ucode packetizes
    the payload at any size)."""
    gp = nc.gpsimd
    stride_bytes = elem_step * mybir.dt.size(in_ap.dtype)
    assert stride_bytes % 256 == 0 and stride_bytes // 256 < 256
    _in_ap = gp.lower_ap_dma(in_ap, for_custom_bir_dma=True)
    _idxs_ap = gp.lower_ap(idxs_ap)
    _out_ap = gp.lower_ap(out_ap)
    return gp.add_instruction(
        mybir.InstDMAGatherAnt(
            name=f"I-{nc.next_id()}",
            ins=[*_in_ap, _idxs_ap, gp.lower_val_access(gp.to_reg(num_idxs))],
            outs=[_out_ap],
            transpose=False,
            num_idxs=num_idxs,
            elem_size=elem_size,
            stride_bytes_256=stride_bytes // 256,
            gen_mode=0,
            single_packet=True,
            queue_num=0,
            sbuf_tokens_per_rank=0,
            sbuf_free_dim_per_rank=0,
            sbuf_free_dim_pad_per_rank=0,
            sbuf_byte_offset=0,
        )
    )


def build_program(cfg, skip=()):
    """Build the SPMD Bass program (same NEFF for all cores).

    skip: {"gather", "slotmm", "overflow", "phasec", "collective"} — timing
    experiment knobs only (results are wrong when used)."""
    skip = set(skip)
    nc = bacc.Bacc(get_trn_type() or "TRN2", target_bir_lowering=False, debug=True)
    NBLK, TPB, C, t_ov, PADN = cfg.NBLK, cfg.TPB, cfg.C, cfg.t_ov, cfg.PADN
    GB, EB, OUTP = cfg.GB, cfg.EB, cfg.OUTP
    TOT = NBLK * TPB

    eat = nc.declare_dram_parameter("eat", [33, TOT * 128], BF16, isOutput=False)
    NQ = (NBLK + GB - 1) // GB
    gidx16 = nc.declare_dram_parameter(
        "gidx16", [128, NQ * 2 * GB * cfg.THALF * 8], I16, isOutput=False)
    rlov2 = nc.declare_dram_parameter("rlov2", [128, NBLK * 2 * cfg.t_ov2], F32,
                                      isOutput=False)
    rlov = nc.declare_dram_parameter("rlov", [128, NBLK * t_ov], F32, isOutput=False)
    xt = nc.declare_dram_parameter("xt", [128, PADN], BF16, isOutput=False)
    dinvp = nc.declare_dram_parameter("dinv", [128, NBLK], F32, isOutput=False)
    sqdp = nc.declare_dram_parameter("sqd", [128, NBLK], F32, isOutput=False)
    deginvp = nc.declare_dram_parameter("deginv", [128, NBLK], F32, isOutput=False)
    rconst = nc.declare_dram_parameter("rconst", [33, 64], BF16, isOutput=False)
    wegot = nc.declare_dram_parameter("wegot", [128, 64], BF16, isOutput=False)
    wpxt = nc.declare_dram_parameter("wpxt", [128, 96], BF16, isOutput=False)
    wpet = nc.declare_dram_parameter("wpet", [32, 96], BF16, isOutput=False)
    bego = nc.declare_dram_parameter("bego", [1, 64], BF16, isOutput=False)
    bpeer = nc.declare_dram_parameter("bpeer", [1, 96], BF16, isOutput=False)
    iota = nc.declare_dram_parameter("iota", [128, 128], BF16, isOutput=False)
    ident = nc.declare_dram_parameter("ident", [128, 128], BF16, isOutput=False)
    outp = nc.declare_dram_parameter("out", [PADN, OUTP], BF16, isOutput=True)

    with tile.TileContext(nc) as tc:
        with (
            tc.tile_pool(name="const", bufs=1) as cp,
            tc.tile_pool(name="resident", bufs=1) as rp,
            tc.tile_pool(name="dram", bufs=1, space="DRAM") as dp,
        ):
            rc = cp.tile([33, 64], BF16)
            nc.sync.dma_start(rc[:], rconst[:])
            wego_sb = cp.tile([128, 64], BF16)
            nc.sync.dma_start(wego_sb[:], wegot[:])
            wpx_sb = cp.tile([128, 96], BF16)
            nc.sync.dma_start(wpx_sb[:], wpxt[:])
            wpe_sb = cp.tile([32, 96], BF16)
            nc.sync.dma_start(wpe_sb[:], wpet[:])
            bego_sb = cp.tile([1, 64], BF16)
            nc.sync.dma_start(bego_sb[:], bego[:])
            bpeer_sb = cp.tile([1, 96], BF16)
            nc.sync.dma_start(bpeer_sb[:], bpeer[:])
            iota_sb = cp.tile([128, 128], BF16)
            nc.sync.dma_start(iota_sb[:], iota[:])
            ident_sb = cp.tile([128, 128], BF16)
            nc.sync.dma_start(ident_sb[:], ident[:])
            ones_sb = cp.tile([1, 128], BF16)
            nc.gpsimd.memset(ones_sb[:], 1.0)

            xt_sb = rp.tile([128, PADN], BF16)
            nc.sync.dma_start(xt_sb[:], xt[:])
            gidx16_sb = rp.tile([128, NQ * 2 * GB * cfg.THALF * 8], I16)
            nc.sync.dma_start(gidx16_sb[:], gidx16[:])
            rlov2_sb = rp.tile([128, NBLK * 2 * cfg.t_ov2], F32)
            nc.sync.dma_start(rlov2_sb[:], rlov2[:])
            rlov_sb = rp.tile([128, NBLK * t_ov], F32)
            nc.sync.dma_start(rlov_sb[:], rlov[:])
            dinv_sb = rp.tile([128, NBLK], F32)
            nc.sync.dma_start(dinv_sb[:], dinvp[:])
            sqd_sb = rp.tile([128, NBLK], F32)
            nc.sync.dma_start(sqd_sb[:], sqdp[:])
            deginv_sb = rp.tile([128, NBLK], F32)
            nc.sync.dma_start(deginv_sb[:], deginvp[:])

            heagg_bf = rp.tile([128, NBLK * 32], BF16)   # h_e_agg, bf16
            ts_cache = rp.tile([128, NBLK * 32], BF16)   # dinv * S_ea, bf16
            oh_cache = rp.tile([128, NBLK * t_ov * 128], BF16)  # one-hots
            outst = rp.tile([128, NBLK * OUTP], BF16)    # output staging

            # zag holds all cores' h_e_agg (p-major rows within each core
            # section); zagA/zagB are 256B-stride padded halves for dma_gather
            zloc = dp.tile([PADN, 32], BF16)
            zag = dp.tile([cfg.ncores * PADN, 32], BF16, addr_space="Shared")
            zagA = dp.tile([cfg.ncores // 2 * PADN, 128], BF16)
            zagB = dp.tile([cfg.ncores // 2 * PADN, 128], BF16)

            # ---------------- Phase A: per-edge MLP + segment sums ----------
            with (
                tc.tile_pool(name="eatp", bufs=3) as eatp,
                tc.tile_pool(name="workA", bufs=4) as wp,
                tc.tile_pool(name="psA", bufs=2, space="PSUM") as psA,
                tc.tile_pool(name="psB", bufs=2, space="PSUM") as psB,
                tc.tile_pool(name="psP", bufs=2, space="PSUM") as psP,
                tc.tile_pool(name="psO", bufs=2, space="PSUM") as psO,
            ):
                ech = None
                for b in range(NBLK):
                    if b % EB == 0:
                        nch = min(EB, NBLK - b)
                        ech = eatp.tile([33, nch * TPB * 128], BF16, tag="ech")
                        nc.sync.dma_start(
                            ech[:], eat[:, b * TPB * 128:(b + nch) * TPB * 128])
                    e0 = (b % EB) * TPB * 128
                    bankA = psA.tile([128, 32], F32, tag="bankA")
                    bankB = psB.tile([128, C * 32], F32, tag="bankB")
                    for j in range(C):
                        if "slotmm" in skip:
                            break
                        lh = ech[:, e0 + j * 128:e0 + (j + 1) * 128]
                        nc.tensor.matmul(bankA[:], lh, rc[:, 0:32],
                                         start=(j == 0), stop=False)
                        nc.tensor.matmul(bankB[:, j * 32:(j + 1) * 32], lh,
                                         rc[:, 32:64],
                                         start=(j == 0), stop=(j == C - 1))
                    # overflow: pre-act for t_ov tiles in one PSUM tile
                    pc = psP.tile([128, t_ov * 64], F32, tag="pc")
                    ov = wp.tile([128, t_ov * 64], BF16, tag="ov")
                    for o in range(t_ov):
                        lh = ech[:, e0 + (C + o) * 128:e0 + (C + o + 1) * 128]
                        nc.tensor.matmul(pc[:, o * 64:(o + 1) * 64], lh, rc[:, 0:64],
                                         start=(o == 0), stop=(o == t_ov - 1))
                    # ea part: copy (Act); h_e part: relu (DVE)
                    nc.scalar.copy(
                        ov[:].rearrange("p (t x) -> p t x", x=64)[:, :, 0:32],
                        pc[:].rearrange("p (t x) -> p t x", x=64)[:, :, 0:32])
                    nc.vector.tensor_scalar_max(
                        ov[:].rearrange("p (t x) -> p t x", x=64)[:, :, 32:64],
                        pc[:].rearrange("p (t x) -> p t x", x=64)[:, :, 32:64],
                        0.0)
                    # one-hot scatter matrices (cached for Phase B reuse)
                    for o in range(t_ov):
                        k = b * t_ov + o
                        nc.vector.tensor_scalar(
                            out=oh_cache[:, k * 128:(k + 1) * 128], in0=iota_sb[:],
                            scalar1=rlov_sb[:, k:k + 1], scalar2=None,
                            op0=OP.is_equal,
                        )
                        # fold overflow ea into bankA accumulation group
                        nc.tensor.matmul(bankA[:], oh_cache[:, k * 128:(k + 1) * 128],
                                         ov[:, o * 64:o * 64 + 32],
                                         start=False, stop=(o == t_ov - 1))
                    # t_s = (1/deg) * S_ea (kept bf16 for Phase C transpose;
                    # the final sqd scale rides the peer relu)
                    nc.vector.tensor_scalar_mul(
                        ts_cache[:, b * 32:(b + 1) * 32], bankA[:],
                        deginv_sb[:, b:b + 1])
                    # h_e slots: relu on Act, slot-sum on DVE into PSUM
                    relu_st = wp.tile([128, C * 32], BF16, tag="relu")
                    nc.scalar.activation(relu_st[:], bankB[:], ACT.Relu)
                    t_he = wp.tile([128, 32], F32, tag="the")
                    nc.vector.tensor_reduce(
                        t_he[:],
                        relu_st[:].rearrange("p (j c) -> p c j", j=C),
                        axis=AX.X, op=OP.add,
                    )
                    bankO = psO.tile([128, 32], F32, tag="bankO")
                    for o in range(t_ov):
                        k = b * t_ov + o
                        nc.tensor.matmul(bankO[:], oh_cache[:, k * 128:(k + 1) * 128],
                                         ov[:, o * 64 + 32:(o + 1) * 64],
                                         start=(o == 0), stop=(o == t_ov - 1))
                    nc.vector.tensor_tensor(
                        out=heagg_bf[:, b * 32:(b + 1) * 32],
                        in0=t_he[:], in1=bankO[:], op=OP.add)
                    # outst[64:96] = h_e_agg; pad cols 224:256 need some value
                    nc.vector.tensor_copy(
                        outst[:, b * OUTP + 64:b * OUTP + 96],
                        heagg_bf[:, b * 32:(b + 1) * 32])
                    nc.vector.tensor_copy(
                        outst[:, b * OUTP + 224:b * OUTP + 256],
                        heagg_bf[:, b * 32:(b + 1) * 32])

            # Share h_e_agg via compact AllGather ("Shared" DRAM is NOT
            # coherently cross-core writable in this runtime, so direct
            # scatter-shares don't work); then locally re-pad each half into
            # a 256B-stride table for the int16 dma_gather.
            nc.sync.dma_start(
                zloc[:].rearrange("(p b) c -> p (b c)", p=128),
                heagg_bf[:],
            )
            if "collective" not in skip:
                nc.gpsimd.collective_compute(
                    "AllGather", OP.bypass,
                    ins=[zloc.opt()], outs=[zag.opt()],
                    replica_groups=[list(range(cfg.ncores))],
                )
            HROWS = cfg.ncores // 2 * PADN
            nc.sync.dma_start(zagA[:, 0:32], zag[0:HROWS, :])
            nc.sync.dma_start(zagB[:, 0:32], zag[HROWS:2 * HROWS, :])

            # ------------- Phase C: node MLPs (overlaps the collective) -----
            if "phasec" not in skip:
                with (
                    tc.tile_pool(name="workC", bufs=4) as wc,
                    tc.tile_pool(name="ps1", bufs=2, space="PSUM") as ps1,
                    tc.tile_pool(name="ps3", bufs=2, space="PSUM") as ps3,
                    tc.tile_pool(name="psT", bufs=2, space="PSUM") as psT,
                ):
                    for b in range(NBLK):
                        xb = xt_sb[:, b * 128:(b + 1) * 128]
                        # h_ego = relu(x W_ego^T + b_ego)
                        p1 = ps1.tile([128, 64], F32, tag="p1")
                        nc.tensor.matmul(p1[:], ones_sb[:], bego_sb[:],
                                         start=True, stop=False)
                        nc.tensor.matmul(p1[:], xb, wego_sb[:],
                                         start=False, stop=True)
                        nc.scalar.activation(
                            outst[:, b * OUTP:b * OUTP + 64], p1[:], ACT.Relu)
                        # h_peer = relu(sqd * (W_px x + b_p + W_pe ((1/deg) S_ea)))
                        pt = psT.tile([32, 128], BF16, tag="pt")
                        nc.tensor.matmul(pt[:], ts_cache[:, b * 32:(b + 1) * 32],
                                         ident_sb[:], is_transpose=True,
                                         start=True, stop=True)
                        seat = wc.tile([32, 128], BF16, tag="seat")
                        nc.scalar.copy(seat[:], pt[:])
                        p3 = ps3.tile([128, 96], F32, tag="p3")
                        nc.tensor.matmul(p3[:], ones_sb[:], bpeer_sb[:],
                                         start=True, stop=False)
                        nc.tensor.matmul(p3[:], xb, wpx_sb[:],
                                         start=False, stop=False)
                        nc.tensor.matmul(p3[:], seat[:], wpe_sb[:],
                                         start=False, stop=True)
                        nc.scalar.activation(
                            outst[:, b * OUTP + 128:b * OUTP + 224], p3[:],
                            ACT.Relu, scale=sqd_sb[:, b:b + 1])

            # ------------- Phase B: gather + M + output writes --------------
            # dma_gather caps at 1024 indices (8 tiles); CH=8 main tiles fill
            # one instruction per (block, half), and the per-block overflow
            # tiles of a whole chunk merge into ONE instruction per half.
            # g layout per chunk: [half][main: block-major 8 tiles][ov:
            # block-major t_ov2 tiles].
            CH2, tov2, THALF = cfg.CH, cfg.t_ov2, cfg.THALF
            assert CH2 == 8
            with (
                tc.tile_pool(name="workB", bufs=4) as wb,
                tc.tile_pool(name="psM", bufs=3, space="PSUM") as psM,
            ):
                for qi, g0 in enumerate(range(0, NBLK, GB)):
                    ng = min(GB, NBLK - g0)
                    span = ng * THALF
                    g = wb.tile([128, 2 * span * 32], BF16, tag="g")
                    if "gather" not in skip:
                        for h, ztab in ((0, zagA), (1, zagB)):
                            qh = (qi * 2 + h) * GB * THALF * 8
                            for br in range(ng):
                                nc_ = raw_dma_gather(
                                    nc,
                                    g[:, (h * span + br * 8) * 32:
                                      (h * span + br * 8 + 8) * 32].rearrange(
                                        "p (t e) -> p t e", e=32),
                                    ztab[:, 0:32],
                                    gidx16_sb[:, qh + br * 64:qh + br * 64 + 64],
                                    num_idxs=1024, elem_size=32, elem_step=128)
                            nov = ng * tov2
                            raw_dma_gather(
                                nc,
                                g[:, (h * span + ng * 8) * 32:
                                  (h * span + ng * 8 + nov) * 32].rearrange(
                                    "p (t e) -> p t e", e=32),
                                ztab[:, 0:32],
                                gidx16_sb[:, qh + GB * 64:qh + GB * 64 + nov * 8],
                                num_idxs=nov * 128, elem_size=32, elem_step=128)
                    gv = g[:].rearrange("p (h z c) -> p c h z", h=2, c=32)
                    for br in range(ng):
                        b = g0 + br
                        m_sb = wb.tile([128, 32], F32, tag="msb")
                        nc.vector.tensor_reduce(
                            m_sb[:],
                            gv[:, :, :, br * 8:br * 8 + 8],
                            axis=AX.XY, op=OP.add,
                        )
                        pm = psM.tile([128, 32], F32, tag="pm")
                        oh2 = wb.tile([128, 2 * tov2 * 128], BF16, tag="oh2")
                        nmm = 2 * tov2
                        for h in range(2):
                            for o in range(tov2):
                                k = h * tov2 + o
                                nc.vector.tensor_scalar(
                                    out=oh2[:, k * 128:(k + 1) * 128], in0=iota_sb[:],
                                    scalar1=rlov2_sb[:, (b * 2 + h) * tov2 + o:
                                                     (b * 2 + h) * tov2 + o + 1],
                                    scalar2=None, op0=OP.is_equal,
                                )
                                nc.tensor.matmul(
                                    pm[:], oh2[:, k * 128:(k + 1) * 128],
                                    g[:, (h * span + ng * 8 + br * tov2 + o) * 32:
                                      (h * span + ng * 8 + br * tov2 + o + 1) * 32],
                                    start=(k == 0), stop=(k == nmm - 1))
                        t_m = wb.tile([128, 32], F32, tag="tm")
                        nc.vector.tensor_tensor(
                            out=t_m[:], in0=m_sb[:], in1=pm[:], op=OP.add)
                        nc.scalar.activation(
                            outst[:, b * OUTP + 96:b * OUTP + 128], t_m[:],
                            ACT.Copy, scale=dinv_sb[:, b:b + 1])
                    nc.sync.dma_start(
                        outp[g0 * 128:(g0 + ng) * 128, :].rearrange(
                            "(q p) c -> p q c", p=128),
                        outst[:, g0 * OUTP:(g0 + ng) * OUTP].rearrange(
                            "p (q c) -> p q c", c=OUTP),
                    )
    nc.compile()
    return nc


def host_prep(cfg, x, edge_attrs, edge_index):
    """Shard + lay out inputs for the slot-grid kernel. Pure index work + O(N)
    scalar prep (degree normalizers); all O(E*H)/O(N*H) FP math runs on device."""
    BF = np.float16
    N, E, C, NBLK, TPB, t_ov, NPC, PADN = (cfg.N, cfg.E, cfg.C, cfg.NBLK,
                                           cfg.TPB, cfg.t_ov, cfg.NPC, cfg.PADN)
    row = np.asarray(edge_index[1]).astype(np.int64)
    col = np.asarray(edge_index[0]).astype(np.int64)
    ea = np.asarray(edge_attrs, dtype=np.float32)
    xf = np.asarray(x, dtype=np.float32)

    deg = np.bincount(row, minlength=N)
    degf = np.maximum(deg, 1).astype(np.float64)
    dinv = np.where(deg > 0, degf ** -0.5, 0.0).astype(np.float32)
    sqd = np.sqrt(deg.astype(np.float64)).astype(np.float32)

    core = row // NPC
    lrow = row - core * NPC
    blk = lrow // 128
    part = lrow % 128

    # rank of each edge within its destination node
    order = np.argsort(row, kind="stable")
    sorted_row = row[order]
    starts = np.searchsorted(sorted_row, np.arange(N), side="left")
    rank = np.empty(E, np.int64)
    rank[order] = np.arange(E) - starts[sorted_row]

    is_grid = rank < C
    ove = np.where(~is_grid)[0]
    ovkey = core[ove] * NBLK + blk[ove]
    o_order = np.argsort(ovkey, kind="stable")
    ove = ove[o_order]
    okey_sorted = ovkey[o_order]
    ostarts = np.searchsorted(okey_sorted, np.arange(NBLK * cfg.ncores), side="left")
    opos = np.arange(ove.size) - ostarts[okey_sorted]
    otile = C + opos // 128
    opart = opos % 128
    if ove.size and otile.max() >= TPB:
        raise ValueError("overflow tiles exceeded; raise t_ov")

    tile_idx = np.empty(E, np.int64)
    tpart = np.empty(E, np.int64)
    ge = np.where(is_grid)[0]
    tile_idx[ge] = blk[ge] * TPB + rank[ge]
    tpart[ge] = part[ge]
    tile_idx[ove] = blk[ove] * TPB + otile
    tpart[ove] = opart

    # ---- Phase-B dual gather grid: edges split by source half ----------
    # half h = src_core // 4; within-half row (p-major per core section):
    #   (src_core % 4) * PADN + (l % 128) * NBLK + l // 128,  l = col % NPC
    CH, t_ov2, THALF, IW = cfg.CH, cfg.t_ov2, cfg.THALF, cfg.IW
    src_core = col // NPC
    lcol = col % NPC
    halfs = src_core // (cfg.ncores // 2)
    zrow16 = ((src_core % (cfg.ncores // 2)) * PADN
              + (lcol % 128) * NBLK + lcol // 128)
    ZPADH = 127 * NBLK + (NBLK - 1)   # half-core-0 pad node -> zeros

    # rank of each edge within (dest node, half)
    key2 = row * 2 + halfs
    order2 = np.argsort(key2, kind="stable")
    sk2 = key2[order2]
    starts2 = np.searchsorted(sk2, np.arange(2 * N), side="left")
    rank2 = np.empty(E, np.int64)
    rank2[order2] = np.arange(E) - starts2[sk2]

    is_grid2 = rank2 < CH
    ove2 = np.where(~is_grid2)[0]
    ovkey2 = (core[ove2] * NBLK + blk[ove2]) * 2 + halfs[ove2]
    o_order2 = np.argsort(ovkey2, kind="stable")
    ove2 = ove2[o_order2]
    ok2_sorted = ovkey2[o_order2]
    ostarts2 = np.searchsorted(ok2_sorted, np.arange(NBLK * cfg.ncores * 2),
                               side="left")
    opos2 = np.arange(ove2.size) - ostarts2[ok2_sorted]
    otile2 = CH + opos2 // 128
    opart2 = opos2 % 128
    if ove2.size and otile2.max() >= THALF:
        raise ValueError("phase-B overflow tiles exceeded; raise t_ov2")

    tile2 = np.empty(E, np.int64)
    tpart2 = np.empty(E, np.int64)
    ge2 = np.where(is_grid2)[0]
    tile2[ge2] = rank2[ge2]
    tpart2[ge2] = part[ge2]
    tile2[ove2] = otile2
    tpart2[ove2] = opart2

    TOTC = NBLK * TPB
    in_maps = []
    for r in range(cfg.ncores):
        sel = core == r
        e_idx = np.where(sel)[0]
        t_i = tile_idx[e_idx]
        t_p = tpart[e_idx]
        colpos = t_i * 128 + t_p

        EAT = np.zeros((33, TOTC * 128), np.float32)
        EAT[:32, colpos] = ea[e_idx].T
        EAT[32, colpos] = 1.0

        # int16 gather indices, per (chunk, half): [main: block-major 8 tiles
        # | ov: block-major t_ov2 tiles]; each instruction's list is stored
        # 16-partition-wrapped [16g + i%16, i//16], replicated x8.
        GB2, NQ = cfg.GB, (NBLK + cfg.GB - 1) // cfg.GB
        eb = blk[e_idx]
        eq, ebr = eb // GB2, eb % GB2
        eh, et, ep = halfs[e_idx], tile2[e_idx], tpart2[e_idx]
        secsz = GB2 * THALF * 128            # idx slots per (chunk, half)
        main_sel = et < CH
        ipos = np.where(
            main_sel,
            (eq * 2 + eh) * secsz + (ebr * 8 + et) * 128 + ep,
            (eq * 2 + eh) * secsz + GB2 * 8 * 128
            + (ebr * t_ov2 + (et - CH)) * 128 + ep)
        flat = np.full(NQ * 2 * secsz, ZPADH, np.int32)
        flat[ipos] = zrow16[e_idx]
        wrap = (flat.reshape(NQ * 2 * GB2 * THALF, 8, 16).transpose(2, 0, 1)
                .reshape(16, NQ * 2 * GB2 * THALF * 8).astype(np.int16))
        GIDX16 = np.tile(wrap, (8, 1))

        RLOV = np.full((128, NBLK * t_ov), 200.0, np.float32)
        ov_r = ove[core[ove] == r]
        op_r = opart[core[ove] == r]
        ot_r = otile[core[ove] == r]
        ob_r = blk[ov_r]
        RLOV[op_r, ob_r * t_ov + (ot_r - C)] = part[ov_r].astype(np.float32)

        RLOV2 = np.full((128, NBLK * 2 * t_ov2), 200.0, np.float32)
        sel2 = core[ove2] == r
        ov2_r = ove2[sel2]
        RLOV2[opart2[sel2],
              (blk[ov2_r] * 2 + halfs[ov2_r]) * t_ov2
              + (otile2[sel2] - CH)] = part[ov2_r].astype(np.float32)

        lo, hi = r * NPC, (r + 1) * NPC
        XT = np.zeros((128, PADN), np.float32)
        XT[:, :NPC] = xf[lo:hi].T
        dl = np.zeros(PADN, np.float32)
        dl[:NPC] = dinv[lo:hi]
        sl = np.zeros(PADN, np.float32)
        sl[:NPC] = sqd[lo:hi]
        gl = np.zeros(PADN, np.float32)
        gl[:NPC] = dinv[lo:hi] ** 2          # 1/deg (0 for deg==0)
        DINV = dl.reshape(NBLK, 128).T.copy()
        SQD = sl.reshape(NBLK, 128).T.copy()
        DEGINV = gl.reshape(NBLK, 128).T.copy()

        m = {
            "eat": EAT.astype(BF), "gidx16": GIDX16, "rlov": RLOV,
            "rlov2": RLOV2, "xt": XT.astype(BF), "dinv": DINV, "sqd": SQD,
            "deginv": DEGINV,
        }
        in_maps.append(m)
    return in_maps


def make_consts(cfg, W_peer, b_peer, W_ego, b_ego, W_edge, b_edge):
    BF = np.float16
    RCONST = np.zeros((33, 64), np.float32)
    RCONST[:32, :32] = np.eye(32, dtype=np.float32)
    RCONST[:32, 32:64] = np.asarray(W_edge, np.float32).T
    RCONST[32, 32:64] = np.asarray(b_edge, np.float32)
    consts = {
        "rconst": RCONST.astype(BF),
        "wegot": np.ascontiguousarray(np.asarray(W_ego, np.float32).T).astype(BF),
        "wpxt": np.ascontiguousarray(np.asarray(W_peer, np.float32)[:, :128].T).astype(BF),
        "wpet": np.ascontiguousarray(np.asarray(W_peer, np.float32)[:, 128:].T).astype(BF),
        "bego": np.asarray(b_ego, np.float32).reshape(1, 64).astype(BF),
        "bpeer": np.asarray(b_peer, np.float32).reshape(1, 96).astype(BF),
        "iota": np.broadcast_to(np.arange(128, dtype=np.float32), (128, 128)).astype(BF).copy(),
        "ident": np.eye(128, dtype=np.float32).astype(BF),
    }
    return consts


_CACHE = {}
RUN_KWARGS = {}


def kernel(x, edge_attrs, W_peer, b_peer, W_ego, b_ego, W_edge, b_edge, edge_index):
    x = np.asarray(x)
    edge_attrs = np.asarray(edge_attrs)
    edge_index = np.asarray(edge_index)
    N, E = x.shape[0], edge_attrs.shape[0]

    row = edge_index[1].astype(np.int64)
    col = edge_index[0].astype(np.int64)
    C = 15
    CH = 8
    ncores = 8
    NPC = N // ncores
    NBLK = (NPC + 127) // 128
    deg = np.bincount(row, minlength=N)
    ovn = np.maximum(deg - C, 0)
    nodes = np.arange(N)
    bkey = (nodes // NPC) * NBLK + (nodes % NPC) // 128
    ovblk = np.bincount(bkey, weights=ovn.astype(np.float64), minlength=NBLK * ncores)
    t_ov = max(3, int(np.ceil(ovblk.max() / 128.0)))

    # phase-B dual-grid overflow sizing (per dest block x source half)
    halfs = (col // NPC) // (ncores // 2)
    degh = np.bincount(row * 2 + halfs, minlength=2 * N)
    ovn2 = np.maximum(degh - CH, 0)
    bkey2 = np.repeat(bkey, 2) * 2 + np.tile([0, 1], N)
    ovblk2 = np.bincount(bkey2, weights=ovn2.astype(np.float64),
                         minlength=2 * NBLK * ncores)
    t_ov2 = max(2, int(np.ceil(ovblk2.max() / 128.0)))

    cfg = Cfg(N=N, E=E, ncores=ncores, C=C, t_ov=t_ov, CH=CH, t_ov2=t_ov2)
    key = cfg.key()
    if key not in _CACHE:
        _CACHE[key] = build_program(cfg)
    nc = _CACHE[key]

    in_maps = host_prep(cfg, x, edge_attrs, edge_index)
    consts = make_consts(cfg, W_peer, b_peer, W_ego, b_ego, W_edge, b_edge)
    for m in in_maps:
        m.update(consts)

    res = run_bass_kernel_spmd(nc, in_maps, core_ids=list(range(cfg.ncores)),
                               **RUN_KWARGS)
    out = np.empty((N, cfg.OUTD), np.float32)
    for r in range(cfg.ncores):
        blk = np.asarray(res.results[r]["out"]).astype(np.float32)
        out[r * cfg.NPC:(r + 1) * cfg.NPC] = blk[:cfg.NPC, :cfg.OUTD]
    kernel.last_result = res
    return out


# revision 43
# speedup vs baseline: 3.4796x; 1.0245x over previous
"""Trainium2 Bass kernel: CANE FeatureEmbedding GNN message passing.

Strategy (node-range sharding, 8 cores), v2:
  - Nodes range-partitioned; edges assigned to the core owning their
    DESTINATION (row = edge_index[1]).
  - Algebraic collapse of the peer branch (gather and scatter both use `row`):
        h_peer[i] = relu( sqrt(deg_i)*(W_px x_i + b_p) + deg_i^-1/2 * (W_pe S_ea[i]) )
    removing the [E,160]x[160,96] per-edge MLP.
  - Remaining per-edge work: h_e = relu(ea @ W_edge.T + b_edge), segment sums
    of ea and h_e over row, and M[i] = sum_{e: row=i} h_e_agg[col[e]].
  - Slot grid: node v gets C slots; edge k of v goes to (block v//128,
    partition v%128, slot k). One bf16 matmul per slot tile accumulates S_ea
    in PSUM; h_e pre-acts land per-slot, relu on the Act engine, slot-sum on
    DVE straight into PSUM, overflow (deg>C) via one-hot scatter matmuls.
  - h_e_agg shared across cores via a compact fp16 AllGather, then locally
    re-padded into two 256B-stride half-tables (<=32767 rows each, int16
    indexable). M is built with dma_gather (64B payload / 256B stride, <=1024
    indices per instruction -- the SWDGE ring cap) over a second slot grid
    split by source half, reduced along (half, slot) on DVE into PSUM with
    one-hot matmul accumulation for overflow.
  - Everything streams in fp16 (4x finer mantissa than bf16 at identical
    cost); PSUM accumulation in fp32; output written as [PADN, 256] fp16 rows
    (cols 224:256 pad) and upcast host-side.
"""

import numpy as np

import concourse.bass as bass
import concourse.mybir as mybir
import concourse.tile as tile
from concourse import bacc
from concourse._compat import get_trn_type
from concourse.bass import IndirectOffsetOnAxis
from concourse.bass_utils import run_bass_kernel_spmd

F32 = mybir.dt.float32
BF16 = mybir.dt.float16  # fp16: 4x finer mantissa than bf16, same perf
I32 = mybir.dt.int32
I16 = mybir.dt.int16
AX = mybir.AxisListType
OP = mybir.AluOpType
ACT = mybir.ActivationFunctionType


class Cfg:
    def __init__(self, N=50000, E=800000, ncores=8, C=15, t_ov=3, GB=4, EB=4,
                 CH=8, t_ov2=2):
        self.N = N
        self.E = E
        self.ncores = ncores
        self.NPC = N // ncores            # nodes per core
        self.NBLK = (self.NPC + 127) // 128
        self.PADN = self.NBLK * 128       # padded nodes per core
        self.C = C                        # grid slots per node
        self.t_ov = t_ov                  # overflow tiles per block
        self.TPB = C + t_ov               # tiles per block
        self.GB = GB                      # blocks per gather instruction
        self.EB = EB                      # blocks per eat-load DMA
        self.CH = CH                      # phase-B main slots per (node, half)
        self.t_ov2 = t_ov2                # phase-B overflow tiles per (block, half)
        self.THALF = CH + t_ov2           # phase-B tiles per (block, half)
        self.IW = (self.THALF * 128 + 15) // 16   # int16 idx cols per gather
        self.ND = 128
        self.ED = 32
        self.OUTD = 224
        self.OUTP = 256                   # padded out row

    def key(self):
        return (self.N, self.E, self.ncores, self.C, self.t_ov, self.GB,
                self.EB, self.CH, self.t_ov2)


def raw_dma_gather(nc, out_ap, in_ap, idxs_ap, num_idxs, elem_size, elem_step):
    """nc.gpsimd.dma_gather minus the %256 payload assert (the 256B rule
    applies to the row STRIDE, which elem_step satisfies; the ucode packetizes
    the payload at any size)."""
    gp = nc.gpsimd
    stride_bytes = elem_step * mybir.dt.size(in_ap.dtype)
    assert stride_bytes % 256 == 0 and stride_bytes // 256 < 256
    _in_ap = gp.lower_ap_dma(in_ap, for_custom_bir_dma=True)
    _idxs_ap = gp.lower_ap(idxs_ap)
    _out_ap = gp.lower_ap(out_ap)
    return gp.add_instruction(
        mybir.InstDMAGatherAnt(
            name=f"I-{nc.next_id()}",
            ins=[*_in_ap, _idxs_ap, gp.lower_val_access(gp.to_reg(num_idxs))],
            outs=[_out_ap],
            transpose=False,
            num_idxs=num_idxs,
            elem_size=elem_size,
            stride_bytes_256=stride_bytes // 256,
            gen_mode=0,
            single_packet=True,
            queue_num=0,
            sbuf_tokens_per_rank=0,
            sbuf_free_dim_per_rank=0,
            sbuf_free_dim_pad_per_rank=0,
            sbuf_byte_offset=0,
        )
    )


def build_program(cfg, skip=()):
    """Build the SPMD Bass program (same NEFF for all cores).

    skip: {"gather", "slotmm", "overflow", "phasec", "collective"} — timing
    experiment knobs only (results are wrong when used)."""
    skip = set(skip)
    nc = bacc.Bacc(get_trn_type() or "TRN2", target_bir_lowering=False, debug=True)
    NBLK, TPB, C, t_ov, PADN = cfg.NBLK, cfg.TPB, cfg.C, cfg.t_ov, cfg.PADN
    GB, EB, OUTP = cfg.GB, cfg.EB, cfg.OUTP
    TOT = NBLK * TPB

    eat = nc.declare_dram_parameter("eat", [33, TOT * 128], BF16, isOutput=False)
    NQ = (NBLK + GB - 1) // GB
    gidx16 = nc.declare_dram_parameter(
        "gidx16", [128, NQ * 2 * GB * cfg.THALF * 8], I16, isOutput=False)
    rlov2 = nc.declare_dram_parameter("rlov2", [128, NBLK * 2 * cfg.t_ov2], F32,
                                      isOutput=False)
    rlov = nc.declare_dram_parameter("rlov", [128, NBLK * t_ov], F32, isOutput=False)
    xt = nc.declare_dram_parameter("xt", [128, PADN], BF16, isOutput=False)
    dinvp = nc.declare_dram_parameter("dinv", [128, NBLK], F32, isOutput=False)
    sqdp = nc.declare_dram_parameter("sqd", [128, NBLK], F32, isOutput=False)
    deginvp = nc.declare_dram_parameter("deginv", [128, NBLK], F32, isOutput=False)
    rconst = nc.declare_dram_parameter("rconst", [33, 64], BF16, isOutput=False)
    wegot = nc.declare_dram_parameter("wegot", [128, 64], BF16, isOutput=False)
    wpxt = nc.declare_dram_parameter("wpxt", [128, 96], BF16, isOutput=False)
    wpet = nc.declare_dram_parameter("wpet", [32, 96], BF16, isOutput=False)
    bego = nc.declare_dram_parameter("bego", [1, 64], BF16, isOutput=False)
    bpeer = nc.declare_dram_parameter("bpeer", [1, 96], BF16, isOutput=False)
    iota = nc.declare_dram_parameter("iota", [128, 128], BF16, isOutput=False)
    ident = nc.declare_dram_parameter("ident", [128, 128], BF16, isOutput=False)
    outp = nc.declare_dram_parameter("out", [PADN, OUTP], BF16, isOutput=True)

    with tile.TileContext(nc) as tc:
        with (
            tc.tile_pool(name="const", bufs=1) as cp,
            tc.tile_pool(name="resident", bufs=1) as rp,
            tc.tile_pool(name="dram", bufs=1, space="DRAM") as dp,
        ):
            rc = cp.tile([33, 64], BF16)
            nc.sync.dma_start(rc[:], rconst[:])
            wego_sb = cp.tile([128, 64], BF16)
            nc.sync.dma_start(wego_sb[:], wegot[:])
            wpx_sb = cp.tile([128, 96], BF16)
            nc.sync.dma_start(wpx_sb[:], wpxt[:])
            wpe_sb = cp.tile([32, 96], BF16)
            nc.sync.dma_start(wpe_sb[:], wpet[:])
            bego_sb = cp.tile([1, 64], BF16)
            nc.sync.dma_start(bego_sb[:], bego[:])
            bpeer_sb = cp.tile([1, 96], BF16)
            nc.sync.dma_start(bpeer_sb[:], bpeer[:])
            iota_sb = cp.tile([128, 128], BF16)
            nc.sync.dma_start(iota_sb[:], iota[:])
            ident_sb = cp.tile([128, 128], BF16)
            nc.sync.dma_start(ident_sb[:], ident[:])
            ones_sb = cp.tile([1, 128], BF16)
            nc.gpsimd.memset(ones_sb[:], 1.0)

            xt_sb = rp.tile([128, PADN], BF16)
            nc.sync.dma_start(xt_sb[:], xt[:])
            gidx16_sb = rp.tile([128, NQ * 2 * GB * cfg.THALF * 8], I16)
            nc.sync.dma_start(gidx16_sb[:], gidx16[:])
            rlov2_sb = rp.tile([128, NBLK * 2 * cfg.t_ov2], F32)
            nc.sync.dma_start(rlov2_sb[:], rlov2[:])
            rlov_sb = rp.tile([128, NBLK * t_ov], F32)
            nc.sync.dma_start(rlov_sb[:], rlov[:])
            dinv_sb = rp.tile([128, NBLK], F32)
            nc.sync.dma_start(dinv_sb[:], dinvp[:])
            sqd_sb = rp.tile([128, NBLK], F32)
            nc.sync.dma_start(sqd_sb[:], sqdp[:])
            deginv_sb = rp.tile([128, NBLK], F32)
            nc.sync.dma_start(deginv_sb[:], deginvp[:])

            heagg_bf = rp.tile([128, NBLK * 32], BF16)   # h_e_agg, bf16
            ts_cache = rp.tile([128, NBLK * 32], BF16)   # dinv * S_ea, bf16
            oh_cache = rp.tile([128, NBLK * t_ov * 128], BF16)  # one-hots
            outst = rp.tile([128, NBLK * OUTP], BF16)    # output staging

            # zag holds all cores' h_e_agg (p-major rows within each core
            # section); zagA/zagB are 256B-stride padded halves for dma_gather
            zloc = dp.tile([PADN, 32], BF16)
            zag = dp.tile([cfg.ncores * PADN, 32], BF16, addr_space="Shared")
            zagA = dp.tile([cfg.ncores // 2 * PADN, 128], BF16)
            zagB = dp.tile([cfg.ncores // 2 * PADN, 128], BF16)

            # ---------------- Phase A: per-edge MLP + segment sums ----------
            with (
                tc.tile_pool(name="eatp", bufs=3) as eatp,
                tc.tile_pool(name="workA", bufs=4) as wp,
                tc.tile_pool(name="psA", bufs=2, space="PSUM") as psA,
                tc.tile_pool(name="psB", bufs=2, space="PSUM") as psB,
                tc.tile_pool(name="psP", bufs=2, space="PSUM") as psP,
                tc.tile_pool(name="psO", bufs=2, space="PSUM") as psO,
            ):
                ech = None
                for b in range(NBLK):
                    if b % EB == 0:
                        nch = min(EB, NBLK - b)
                        ech = eatp.tile([33, nch * TPB * 128], BF16, tag="ech")
                        nc.sync.dma_start(
                            ech[:], eat[:, b * TPB * 128:(b + nch) * TPB * 128])
                    e0 = (b % EB) * TPB * 128
                    bankA = psA.tile([128, 32], F32, tag="bankA")
                    bankB = psB.tile([128, C * 32], F32, tag="bankB")
                    for j in range(C):
                        if "slotmm" in skip:
                            break
                        lh = ech[:, e0 + j * 128:e0 + (j + 1) * 128]
                        nc.tensor.matmul(bankA[:], lh, rc[:, 0:32],
                                         start=(j == 0), stop=False)
                        nc.tensor.matmul(bankB[:, j * 32:(j + 1) * 32], lh,
                                         rc[:, 32:64],
                                         start=(j == 0), stop=(j == C - 1))
                    # overflow: pre-act for t_ov tiles in one PSUM tile
                    pc = psP.tile([128, t_ov * 64], F32, tag="pc")
                    ov = wp.tile([128, t_ov * 64], BF16, tag="ov")
                    for o in range(t_ov):
                        lh = ech[:, e0 + (C + o) * 128:e0 + (C + o + 1) * 128]
                        nc.tensor.matmul(pc[:, o * 64:(o + 1) * 64], lh, rc[:, 0:64],
                                         start=(o == 0), stop=(o == t_ov - 1))
                    # ea part: copy (Act); h_e part: relu (DVE)
                    nc.scalar.copy(
                        ov[:].rearrange("p (t x) -> p t x", x=64)[:, :, 0:32],
                        pc[:].rearrange("p (t x) -> p t x", x=64)[:, :, 0:32])
                    nc.vector.tensor_scalar_max(
                        ov[:].rearrange("p (t x) -> p t x", x=64)[:, :, 32:64],
                        pc[:].rearrange("p (t x) -> p t x", x=64)[:, :, 32:64],
                        0.0)
                    # one-hot scatter matrices (cached for Phase B reuse)
                    for o in range(t_ov):
                        k = b * t_ov + o
                        nc.gpsimd.tensor_scalar(
                            out=oh_cache[:, k * 128:(k + 1) * 128], in0=iota_sb[:],
                            scalar1=rlov_sb[:, k:k + 1], scalar2=None,
                            op0=OP.is_equal,
                        )
                        # fold overflow ea into bankA accumulation group
                        nc.tensor.matmul(bankA[:], oh_cache[:, k * 128:(k + 1) * 128],
                                         ov[:, o * 64:o * 64 + 32],
                                         start=False, stop=(o == t_ov - 1))
                    # t_s = (1/deg) * S_ea (kept bf16 for Phase C transpose;
                    # the final sqd scale rides the peer relu)
                    nc.vector.tensor_scalar_mul(
                        ts_cache[:, b * 32:(b + 1) * 32], bankA[:],
                        deginv_sb[:, b:b + 1])
                    # h_e slots: relu on Act, slot-sum on DVE into PSUM
                    relu_st = wp.tile([128, C * 32], BF16, tag="relu")
                    nc.scalar.activation(relu_st[:], bankB[:], ACT.Relu)
                    t_he = wp.tile([128, 32], F32, tag="the")
                    nc.vector.tensor_reduce(
                        t_he[:],
                        relu_st[:].rearrange("p (j c) -> p c j", j=C),
                        axis=AX.X, op=OP.add,
                    )
                    bankO = psO.tile([128, 32], F32, tag="bankO")
                    for o in range(t_ov):
                        k = b * t_ov + o
                        nc.tensor.matmul(bankO[:], oh_cache[:, k * 128:(k + 1) * 128],
                                         ov[:, o * 64 + 32:(o + 1) * 64],
                                         start=(o == 0), stop=(o == t_ov - 1))
                    nc.vector.tensor_tensor(
                        out=heagg_bf[:, b * 32:(b + 1) * 32],
                        in0=t_he[:], in1=bankO[:], op=OP.add)
                    # outst[64:96] = h_e_agg; pad cols 224:256 need some value
                    nc.vector.tensor_copy(
                        outst[:, b * OUTP + 64:b * OUTP + 96],
                        heagg_bf[:, b * 32:(b + 1) * 32])
                    nc.vector.tensor_copy(
                        outst[:, b * OUTP + 224:b * OUTP + 256],
                        heagg_bf[:, b * 32:(b + 1) * 32])

            # Share h_e_agg via compact AllGather ("Shared" DRAM is NOT
            # coherently cross-core writable in this runtime, so direct
            # scatter-shares don't work); then locally re-pad each half into
            # a 256B-stride table for the int16 dma_gather.
            nc.sync.dma_start(
                zloc[:].rearrange("(p b) c -> p (b c)", p=128),
                heagg_bf[:],
            )
            if "collective" not in skip:
                nc.gpsimd.collective_compute(
                    "AllGather", OP.bypass,
                    ins=[zloc.opt()], outs=[zag.opt()],
                    replica_groups=[list(range(cfg.ncores))],
                )
            HROWS = cfg.ncores // 2 * PADN
            nc.sync.dma_start(zagA[:, 0:32], zag[0:HROWS, :])
            nc.sync.dma_start(zagB[:, 0:32], zag[HROWS:2 * HROWS, :])

            # ------------- Phase C: node MLPs (overlaps the collective) -----
            if "phasec" not in skip:
                with (
                    tc.tile_pool(name="workC", bufs=4) as wc,
                    tc.tile_pool(name="ps1", bufs=2, space="PSUM") as ps1,
                    tc.tile_pool(name="ps3", bufs=2, space="PSUM") as ps3,
                    tc.tile_pool(name="psT", bufs=2, space="PSUM") as psT,
                ):
                    for b in range(NBLK):
                        xb = xt_sb[:, b * 128:(b + 1) * 128]
                        # h_ego = relu(x W_ego^T + b_ego)
                        p1 = ps1.tile([128, 64], F32, tag="p1")
                        nc.tensor.matmul(p1[:], ones_sb[:], bego_sb[:],
                                         start=True, stop=False)
                        nc.tensor.matmul(p1[:], xb, wego_sb[:],
                                         start=False, stop=True)
                        nc.scalar.activation(
                            outst[:, b * OUTP:b * OUTP + 64], p1[:], ACT.Relu)
                        # h_peer = relu(sqd * (W_px x + b_p + W_pe ((1/deg) S_ea)))
                        pt = psT.tile([32, 128], BF16, tag="pt")
                        nc.tensor.matmul(pt[:], ts_cache[:, b * 32:(b + 1) * 32],
                                         ident_sb[:], is_transpose=True,
                                         start=True, stop=True)
                        seat = wc.tile([32, 128], BF16, tag="seat")
                        nc.scalar.copy(seat[:], pt[:])
                        p3 = ps3.tile([128, 96], F32, tag="p3")
                        nc.tensor.matmul(p3[:], ones_sb[:], bpeer_sb[:],
                                         start=True, stop=False)
                        nc.tensor.matmul(p3[:], xb, wpx_sb[:],
                                         start=False, stop=False)
                        nc.tensor.matmul(p3[:], seat[:], wpe_sb[:],
                                         start=False, stop=True)
                        nc.scalar.activation(
                            outst[:, b * OUTP + 128:b * OUTP + 224], p3[:],
                            ACT.Relu, scale=sqd_sb[:, b:b + 1])

            # ------------- Phase B: gather + M + output writes --------------
            # dma_gather caps at 1024 indices (8 tiles); CH=8 main tiles fill
            # one instruction per (block, half), and the per-block overflow
            # tiles of a whole chunk merge into ONE instruction per half.
            # g layout per chunk: [half][main: block-major 8 tiles][ov:
            # block-major t_ov2 tiles].
            CH2, tov2, THALF = cfg.CH, cfg.t_ov2, cfg.THALF
            assert CH2 == 8
            with (
                tc.tile_pool(name="workB", bufs=4) as wb,
                tc.tile_pool(name="psM", bufs=3, space="PSUM") as psM,
            ):
                for qi, g0 in enumerate(range(0, NBLK, GB)):
                    ng = min(GB, NBLK - g0)
                    span = ng * THALF
                    g = wb.tile([128, 2 * span * 32], BF16, tag="g")
                    if "gather" not in skip:
                        for h, ztab in ((0, zagA), (1, zagB)):
                            qh = (qi * 2 + h) * GB * THALF * 8
                            for br in range(ng):
                                nc_ = raw_dma_gather(
                                    nc,
                                    g[:, (h * span + br * 8) * 32:
                                      (h * span + br * 8 + 8) * 32].rearrange(
                                        "p (t e) -> p t e", e=32),
                                    ztab[:, 0:32],
                                    gidx16_sb[:, qh + br * 64:qh + br * 64 + 64],
                                    num_idxs=1024, elem_size=32, elem_step=128)
                            nov = ng * tov2
                            raw_dma_gather(
                                nc,
                                g[:, (h * span + ng * 8) * 32:
                                  (h * span + ng * 8 + nov) * 32].rearrange(
                                    "p (t e) -> p t e", e=32),
                                ztab[:, 0:32],
                                gidx16_sb[:, qh + GB * 64:qh + GB * 64 + nov * 8],
                                num_idxs=nov * 128, elem_size=32, elem_step=128)
                    gv = g[:].rearrange("p (h z c) -> p c h z", h=2, c=32)
                    for br in range(ng):
                        b = g0 + br
                        m_sb = wb.tile([128, 32], F32, tag="msb")
                        nc.vector.tensor_reduce(
                            m_sb[:],
                            gv[:, :, :, br * 8:br * 8 + 8],
                            axis=AX.XY, op=OP.add,
                        )
                        pm = psM.tile([128, 32], F32, tag="pm")
                        oh2 = wb.tile([128, 2 * tov2 * 128], BF16, tag="oh2")
                        nmm = 2 * tov2
                        for h in range(2):
                            for o in range(tov2):
                                k = h * tov2 + o
                                nc.vector.tensor_scalar(
                                    out=oh2[:, k * 128:(k + 1) * 128], in0=iota_sb[:],
                                    scalar1=rlov2_sb[:, (b * 2 + h) * tov2 + o:
                                                     (b * 2 + h) * tov2 + o + 1],
                                    scalar2=None, op0=OP.is_equal,
                                )
                                nc.tensor.matmul(
                                    pm[:], oh2[:, k * 128:(k + 1) * 128],
                                    g[:, (h * span + ng * 8 + br * tov2 + o) * 32:
                                      (h * span + ng * 8 + br * tov2 + o + 1) * 32],
                                    start=(k == 0), stop=(k == nmm - 1))
                        t_m = wb.tile([128, 32], F32, tag="tm")
                        nc.vector.tensor_tensor(
                            out=t_m[:], in0=m_sb[:], in1=pm[:], op=OP.add)
                        nc.scalar.activation(
                            outst[:, b * OUTP + 96:b * OUTP + 128], t_m[:],
                            ACT.Copy, scale=dinv_sb[:, b:b + 1])
                    nc.sync.dma_start(
                        outp[g0 * 128:(g0 + ng) * 128, :].rearrange(
                            "(q p) c -> p q c", p=128),
                        outst[:, g0 * OUTP:(g0 + ng) * OUTP].rearrange(
                            "p (q c) -> p q c", c=OUTP),
                    )
    nc.compile()
    return nc


def host_prep(cfg, x, edge_attrs, edge_index):
    """Shard + lay out inputs for the slot-grid kernel. Pure index work + O(N)
    scalar prep (degree normalizers); all O(E*H)/O(N*H) FP math runs on device."""
    BF = np.float16
    N, E, C, NBLK, TPB, t_ov, NPC, PADN = (cfg.N, cfg.E, cfg.C, cfg.NBLK,
                                           cfg.TPB, cfg.t_ov, cfg.NPC, cfg.PADN)
    row = np.asarray(edge_index[1]).astype(np.int64)
    col = np.asarray(edge_index[0]).astype(np.int64)
    ea = np.asarray(edge_attrs, dtype=np.float32)
    xf = np.asarray(x, dtype=np.float32)

    deg = np.bincount(row, minlength=N)
    degf = np.maximum(deg, 1).astype(np.float64)
    dinv = np.where(deg > 0, degf ** -0.5, 0.0).astype(np.float32)
    sqd = np.sqrt(deg.astype(np.float64)).astype(np.float32)

    core = row // NPC
    lrow = row - core * NPC
    blk = lrow // 128
    part = lrow % 128

    # rank of each edge within its destination node
    order = np.argsort(row, kind="stable")
    sorted_row = row[order]
    starts = np.searchsorted(sorted_row, np.arange(N), side="left")
    rank = np.empty(E, np.int64)
    rank[order] = np.arange(E) - starts[sorted_row]

    is_grid = rank < C
    ove = np.where(~is_grid)[0]
    ovkey = core[ove] * NBLK + blk[ove]
    o_order = np.argsort(ovkey, kind="stable")
    ove = ove[o_order]
    okey_sorted = ovkey[o_order]
    ostarts = np.searchsorted(okey_sorted, np.arange(NBLK * cfg.ncores), side="left")
    opos = np.arange(ove.size) - ostarts[okey_sorted]
    otile = C + opos // 128
    opart = opos % 128
    if ove.size and otile.max() >= TPB:
        raise ValueError("overflow tiles exceeded; raise t_ov")

    tile_idx = np.empty(E, np.int64)
    tpart = np.empty(E, np.int64)
    ge = np.where(is_grid)[0]
    tile_idx[ge] = blk[ge] * TPB + rank[ge]
    tpart[ge] = part[ge]
    tile_idx[ove] = blk[ove] * TPB + otile
    tpart[ove] = opart

    # ---- Phase-B dual gather grid: edges split by source half ----------
    # half h = src_core // 4; within-half row (p-major per core section):
    #   (src_core % 4) * PADN + (l % 128) * NBLK + l // 128,  l = col % NPC
    CH, t_ov2, THALF, IW = cfg.CH, cfg.t_ov2, cfg.THALF, cfg.IW
    src_core = col // NPC
    lcol = col % NPC
    halfs = src_core // (cfg.ncores // 2)
    zrow16 = ((src_core % (cfg.ncores // 2)) * PADN
              + (lcol % 128) * NBLK + lcol // 128)
    ZPADH = 127 * NBLK + (NBLK - 1)   # half-core-0 pad node -> zeros

    # rank of each edge within (dest node, half)
    key2 = row * 2 + halfs
    order2 = np.argsort(key2, kind="stable")
    sk2 = key2[order2]
    starts2 = np.searchsorted(sk2, np.arange(2 * N), side="left")
    rank2 = np.empty(E, np.int64)
    rank2[order2] = np.arange(E) - starts2[sk2]

    is_grid2 = rank2 < CH
    ove2 = np.where(~is_grid2)[0]
    ovkey2 = (core[ove2] * NBLK + blk[ove2]) * 2 + halfs[ove2]
    o_order2 = np.argsort(ovkey2, kind="stable")
    ove2 = ove2[o_order2]
    ok2_sorted = ovkey2[o_order2]
    ostarts2 = np.searchsorted(ok2_sorted, np.arange(NBLK * cfg.ncores * 2),
                               side="left")
    opos2 = np.arange(ove2.size) - ostarts2[ok2_sorted]
    otile2 = CH + opos2 // 128
    opart2 = opos2 % 128
    if ove2.size and otile2.max() >= THALF:
        raise ValueError("phase-B overflow tiles exceeded; raise t_ov2")

    tile2 = np.empty(E, np.int64)
    tpart2 = np.empty(E, np.int64)
    ge2 = np.where(is_grid2)[0]
    tile2[ge2] = rank2[ge2]
    tpart2[ge2] = part[ge2]
    tile2[ove2] = otile2
    tpart2[ove2] = opart2

    TOTC = NBLK * TPB
    in_maps = []
    for r in range(cfg.ncores):
        sel = core == r
        e_idx = np.where(sel)[0]
        t_i = tile_idx[e_idx]
        t_p = tpart[e_idx]
        colpos = t_i * 128 + t_p

        EAT = np.zeros((33, TOTC * 128), np.float32)
        EAT[:32, colpos] = ea[e_idx].T
        EAT[32, colpos] = 1.0

        # int16 gather indices, per (chunk, half): [main: block-major 8 tiles
        # | ov: block-major t_ov2 tiles]; each instruction's list is stored
        # 16-partition-wrapped [16g + i%16, i//16], replicated x8.
        GB2, NQ = cfg.GB, (NBLK + cfg.GB - 1) // cfg.GB
        eb = blk[e_idx]
        eq, ebr = eb // GB2, eb % GB2
        eh, et, ep = halfs[e_idx], tile2[e_idx], tpart2[e_idx]
        secsz = GB2 * THALF * 128            # idx slots per (chunk, half)
        main_sel = et < CH
        ipos = np.where(
            main_sel,
            (eq * 2 + eh) * secsz + (ebr * 8 + et) * 128 + ep,
            (eq * 2 + eh) * secsz + GB2 * 8 * 128
            + (ebr * t_ov2 + (et - CH)) * 128 + ep)
        flat = np.full(NQ * 2 * secsz, ZPADH, np.int32)
        flat[ipos] = zrow16[e_idx]
        wrap = (flat.reshape(NQ * 2 * GB2 * THALF, 8, 16).transpose(2, 0, 1)
                .reshape(16, NQ * 2 * GB2 * THALF * 8).astype(np.int16))
        GIDX16 = np.tile(wrap, (8, 1))

        RLOV = np.full((128, NBLK * t_ov), 200.0, np.float32)
        ov_r = ove[core[ove] == r]
        op_r = opart[core[ove] == r]
        ot_r = otile[core[ove] == r]
        ob_r = blk[ov_r]
        RLOV[op_r, ob_r * t_ov + (ot_r - C)] = part[ov_r].astype(np.float32)

        RLOV2 = np.full((128, NBLK * 2 * t_ov2), 200.0, np.float32)
        sel2 = core[ove2] == r
        ov2_r = ove2[sel2]
        RLOV2[opart2[sel2],
              (blk[ov2_r] * 2 + halfs[ov2_r]) * t_ov2
              + (otile2[sel2] - CH)] = part[ov2_r].astype(np.float32)

        lo, hi = r * NPC, (r + 1) * NPC
        XT = np.zeros((128, PADN), np.float32)
        XT[:, :NPC] = xf[lo:hi].T
        dl = np.zeros(PADN, np.float32)
        dl[:NPC] = dinv[lo:hi]
        sl = np.zeros(PADN, np.float32)
        sl[:NPC] = sqd[lo:hi]
        gl = np.zeros(PADN, np.float32)
        gl[:NPC] = dinv[lo:hi] ** 2          # 1/deg (0 for deg==0)
        DINV = dl.reshape(NBLK, 128).T.copy()
        SQD = sl.reshape(NBLK, 128).T.copy()
        DEGINV = gl.reshape(NBLK, 128).T.copy()

        m = {
            "eat": EAT.astype(BF), "gidx16": GIDX16, "rlov": RLOV,
            "rlov2": RLOV2, "xt": XT.astype(BF), "dinv": DINV, "sqd": SQD,
            "deginv": DEGINV,
        }
        in_maps.append(m)
    return in_maps


def make_consts(cfg, W_peer, b_peer, W_ego, b_ego, W_edge, b_edge):
    BF = np.float16
    RCONST = np.zeros((33, 64), np.float32)
    RCONST[:32, :32] = np.eye(32, dtype=np.float32)
    RCONST[:32, 32:64] = np.asarray(W_edge, np.float32).T
    RCONST[32, 32:64] = np.asarray(b_edge, np.float32)
    consts = {
        "rconst": RCONST.astype(BF),
        "wegot": np.ascontiguousarray(np.asarray(W_ego, np.float32).T).astype(BF),
        "wpxt": np.ascontiguousarray(np.asarray(W_peer, np.float32)[:, :128].T).astype(BF),
        "wpet": np.ascontiguousarray(np.asarray(W_peer, np.float32)[:, 128:].T).astype(BF),
        "bego": np.asarray(b_ego, np.float32).reshape(1, 64).astype(BF),
        "bpeer": np.asarray(b_peer, np.float32).reshape(1, 96).astype(BF),
        "iota": np.broadcast_to(np.arange(128, dtype=np.float32), (128, 128)).astype(BF).copy(),
        "ident": np.eye(128, dtype=np.float32).astype(BF),
    }
    return consts


_CACHE = {}
RUN_KWARGS = {}


def kernel(x, edge_attrs, W_peer, b_peer, W_ego, b_ego, W_edge, b_edge, edge_index):
    x = np.asarray(x)
    edge_attrs = np.asarray(edge_attrs)
    edge_index = np.asarray(edge_index)
    N, E = x.shape[0], edge_attrs.shape[0]

    row = edge_index[1].astype(np.int64)
    col = edge_index[0].astype(np.int64)
    C = 15
    CH = 8
    ncores = 8
    NPC = N // ncores
    NBLK = (NPC + 127) // 128
    deg = np.bincount(row, minlength=N)
    ovn = np.maximum(deg - C, 0)
    nodes = np.arange(N)
    bkey = (nodes // NPC) * NBLK + (nodes % NPC) // 128
    ovblk = np.bincount(bkey, weights=ovn.astype(np.float64), minlength=NBLK * ncores)
    t_ov = max(3, int(np.ceil(ovblk.max() / 128.0)))

    # phase-B dual-grid overflow sizing (per dest block x source half)
    halfs = (col // NPC) // (ncores // 2)
    degh = np.bincount(row * 2 + halfs, minlength=2 * N)
    ovn2 = np.maximum(degh - CH, 0)
    bkey2 = np.repeat(bkey, 2) * 2 + np.tile([0, 1], N)
    ovblk2 = np.bincount(bkey2, weights=ovn2.astype(np.float64),
                         minlength=2 * NBLK * ncores)
    t_ov2 = max(2, int(np.ceil(ovblk2.max() / 128.0)))

    cfg = Cfg(N=N, E=E, ncores=ncores, C=C, t_ov=t_ov, CH=CH, t_ov2=t_ov2)
    key = cfg.key()
    if key not in _CACHE:
        _CACHE[key] = build_program(cfg)
    nc = _CACHE[key]

    in_maps = host_prep(cfg, x, edge_attrs, edge_index)
    consts = make_consts(cfg, W_peer, b_peer, W_ego, b_ego, W_edge, b_edge)
    for m in in_maps:
        m.update(consts)

    res = run_bass_kernel_spmd(nc, in_maps, core_ids=list(range(cfg.ncores)),
                               **RUN_KWARGS)
    out = np.empty((N, cfg.OUTD), np.float32)
    for r in range(cfg.ncores):
        blk = np.asarray(res.results[r]["out"]).astype(np.float32)
        out[r * cfg.NPC:(r + 1) * cfg.NPC] = blk[:cfg.NPC, :cfg.OUTD]
    kernel.last_result = res
    return out


# revision 44
# speedup vs baseline: 3.5581x; 1.0226x over previous
"""Trainium2 Bass kernel: CANE FeatureEmbedding GNN message passing.

Strategy (node-range sharding, 8 cores), v2:
  - Nodes range-partitioned; edges assigned to the core owning their
    DESTINATION (row = edge_index[1]).
  - Algebraic collapse of the peer branch (gather and scatter both use `row`):
        h_peer[i] = relu( sqrt(deg_i)*(W_px x_i + b_p) + deg_i^-1/2 * (W_pe S_ea[i]) )
    removing the [E,160]x[160,96] per-edge MLP.
  - Remaining per-edge work: h_e = relu(ea @ W_edge.T + b_edge), segment sums
    of ea and h_e over row, and M[i] = sum_{e: row=i} h_e_agg[col[e]].
  - Slot grid: node v gets C slots; edge k of v goes to (block v//128,
    partition v%128, slot k). One bf16 matmul per slot tile accumulates S_ea
    in PSUM; h_e pre-acts land per-slot, relu on the Act engine, slot-sum on
    DVE straight into PSUM, overflow (deg>C) via one-hot scatter matmuls.
  - h_e_agg shared across cores via a compact fp16 AllGather, then locally
    re-padded into two 256B-stride half-tables (<=32767 rows each, int16
    indexable). M is built with dma_gather (64B payload / 256B stride, <=1024
    indices per instruction -- the SWDGE ring cap) over a second slot grid
    split by source half, reduced along (half, slot) on DVE into PSUM with
    one-hot matmul accumulation for overflow.
  - Everything streams in fp16 (4x finer mantissa than bf16 at identical
    cost); PSUM accumulation in fp32; output written as [PADN, 256] fp16 rows
    (cols 224:256 pad) and upcast host-side.
"""

import numpy as np

import concourse.bass as bass
import concourse.mybir as mybir
import concourse.tile as tile
from concourse import bacc
from concourse._compat import get_trn_type
from concourse.bass import IndirectOffsetOnAxis
from concourse.bass_utils import run_bass_kernel_spmd

F32 = mybir.dt.float32
BF16 = mybir.dt.float16  # fp16: 4x finer mantissa than bf16, same perf
I32 = mybir.dt.int32
I16 = mybir.dt.int16
AX = mybir.AxisListType
OP = mybir.AluOpType
ACT = mybir.ActivationFunctionType


class Cfg:
    def __init__(self, N=50000, E=800000, ncores=8, C=15, t_ov=3, GB=4, EB=4,
                 CH=8, t_ov2=2):
        self.N = N
        self.E = E
        self.ncores = ncores
        self.NPC = N // ncores            # nodes per core
        self.NBLK = (self.NPC + 127) // 128
        self.PADN = self.NBLK * 128       # padded nodes per core
        self.C = C                        # grid slots per node
        self.t_ov = t_ov                  # overflow tiles per block
        self.TPB = C + t_ov               # tiles per block
        self.GB = GB                      # blocks per gather instruction
        self.EB = EB                      # blocks per eat-load DMA
        self.CH = CH                      # phase-B main slots per (node, half)
        self.t_ov2 = t_ov2                # phase-B overflow tiles per (block, half)
        self.THALF = CH + t_ov2           # phase-B tiles per (block, half)
        self.IW = (self.THALF * 128 + 15) // 16   # int16 idx cols per gather
        self.ND = 128
        self.ED = 32
        self.OUTD = 224
        self.OUTP = 256                   # padded out row

    def key(self):
        return (self.N, self.E, self.ncores, self.C, self.t_ov, self.GB,
                self.EB, self.CH, self.t_ov2)


def raw_dma_gather(nc, out_ap, in_ap, idxs_ap, num_idxs, elem_size, elem_step):
    """nc.gpsimd.dma_gather minus the %256 payload assert (the 256B rule
    applies to the row STRIDE, which elem_step satisfies; the ucode packetizes
    the payload at any size)."""
    gp = nc.gpsimd
    stride_bytes = elem_step * mybir.dt.size(in_ap.dtype)
    assert stride_bytes % 256 == 0 and stride_bytes // 256 < 256
    _in_ap = gp.lower_ap_dma(in_ap, for_custom_bir_dma=True)
    _idxs_ap = gp.lower_ap(idxs_ap)
    _out_ap = gp.lower_ap(out_ap)
    return gp.add_instruction(
        mybir.InstDMAGatherAnt(
            name=f"I-{nc.next_id()}",
            ins=[*_in_ap, _idxs_ap, gp.lower_val_access(gp.to_reg(num_idxs))],
            outs=[_out_ap],
            transpose=False,
            num_idxs=num_idxs,
            elem_size=elem_size,
            stride_bytes_256=stride_bytes // 256,
            gen_mode=0,
            single_packet=True,
            queue_num=0,
            sbuf_tokens_per_rank=0,
            sbuf_free_dim_per_rank=0,
            sbuf_free_dim_pad_per_rank=0,
            sbuf_byte_offset=0,
        )
    )


def build_program(cfg, skip=()):
    """Build the SPMD Bass program (same NEFF for all cores).

    skip: {"gather", "slotmm", "overflow", "phasec", "collective"} — timing
    experiment knobs only (results are wrong when used)."""
    skip = set(skip)
    nc = bacc.Bacc(get_trn_type() or "TRN2", target_bir_lowering=False, debug=True)
    NBLK, TPB, C, t_ov, PADN = cfg.NBLK, cfg.TPB, cfg.C, cfg.t_ov, cfg.PADN
    GB, EB, OUTP = cfg.GB, cfg.EB, cfg.OUTP
    TOT = NBLK * TPB

    eat = nc.declare_dram_parameter("eat", [33, TOT * 128], BF16, isOutput=False)
    NQ = (NBLK + GB - 1) // GB
    gidx16 = nc.declare_dram_parameter(
        "gidx16", [128, NQ * 2 * GB * cfg.THALF * 8], I16, isOutput=False)
    rlov2 = nc.declare_dram_parameter("rlov2", [128, NBLK * 2 * cfg.t_ov2], F32,
                                      isOutput=False)
    rlov = nc.declare_dram_parameter("rlov", [128, NBLK * t_ov], F32, isOutput=False)
    xt = nc.declare_dram_parameter("xt", [128, PADN], BF16, isOutput=False)
    dinvp = nc.declare_dram_parameter("dinv", [128, NBLK], F32, isOutput=False)
    sqdp = nc.declare_dram_parameter("sqd", [128, NBLK], F32, isOutput=False)
    deginvp = nc.declare_dram_parameter("deginv", [128, NBLK], F32, isOutput=False)
    rconst = nc.declare_dram_parameter("rconst", [33, 64], BF16, isOutput=False)
    wegot = nc.declare_dram_parameter("wegot", [128, 64], BF16, isOutput=False)
    wpxt = nc.declare_dram_parameter("wpxt", [128, 96], BF16, isOutput=False)
    wpet = nc.declare_dram_parameter("wpet", [32, 96], BF16, isOutput=False)
    bego = nc.declare_dram_parameter("bego", [1, 64], BF16, isOutput=False)
    bpeer = nc.declare_dram_parameter("bpeer", [1, 96], BF16, isOutput=False)
    iota = nc.declare_dram_parameter("iota", [128, 128], BF16, isOutput=False)
    ident = nc.declare_dram_parameter("ident", [128, 128], BF16, isOutput=False)
    outp = nc.declare_dram_parameter("out", [PADN, OUTP], BF16, isOutput=True)

    with tile.TileContext(nc) as tc:
        with (
            tc.tile_pool(name="const", bufs=1) as cp,
            tc.tile_pool(name="resident", bufs=1) as rp,
            tc.tile_pool(name="dram", bufs=1, space="DRAM") as dp,
        ):
            rc = cp.tile([33, 64], BF16)
            nc.sync.dma_start(rc[:], rconst[:])
            wego_sb = cp.tile([128, 64], BF16)
            nc.sync.dma_start(wego_sb[:], wegot[:])
            wpx_sb = cp.tile([128, 96], BF16)
            nc.sync.dma_start(wpx_sb[:], wpxt[:])
            wpe_sb = cp.tile([32, 96], BF16)
            nc.sync.dma_start(wpe_sb[:], wpet[:])
            bego_sb = cp.tile([1, 64], BF16)
            nc.sync.dma_start(bego_sb[:], bego[:])
            bpeer_sb = cp.tile([1, 96], BF16)
            nc.sync.dma_start(bpeer_sb[:], bpeer[:])
            iota_sb = cp.tile([128, 128], BF16)
            nc.sync.dma_start(iota_sb[:], iota[:])
            ident_sb = cp.tile([128, 128], BF16)
            nc.sync.dma_start(ident_sb[:], ident[:])
            ones_sb = cp.tile([1, 128], BF16)
            nc.gpsimd.memset(ones_sb[:], 1.0)

            xt_sb = rp.tile([128, PADN], BF16)
            nc.sync.dma_start(xt_sb[:], xt[:])
            gidx16_sb = rp.tile([128, NQ * 2 * GB * cfg.THALF * 8], I16)
            nc.sync.dma_start(gidx16_sb[:], gidx16[:])
            rlov2_sb = rp.tile([128, NBLK * 2 * cfg.t_ov2], F32)
            nc.sync.dma_start(rlov2_sb[:], rlov2[:])
            rlov_sb = rp.tile([128, NBLK * t_ov], F32)
            nc.sync.dma_start(rlov_sb[:], rlov[:])
            dinv_sb = rp.tile([128, NBLK], F32)
            nc.sync.dma_start(dinv_sb[:], dinvp[:])
            sqd_sb = rp.tile([128, NBLK], F32)
            nc.sync.dma_start(sqd_sb[:], sqdp[:])
            deginv_sb = rp.tile([128, NBLK], F32)
            nc.sync.dma_start(deginv_sb[:], deginvp[:])

            heagg_bf = rp.tile([128, NBLK * 32], BF16)   # h_e_agg, bf16
            ts_cache = rp.tile([128, NBLK * 32], BF16)   # dinv * S_ea, bf16
            oh_cache = rp.tile([128, NBLK * t_ov * 128], BF16)  # one-hots
            outst = rp.tile([128, NBLK * OUTP], BF16)    # output staging

            # zag holds all cores' h_e_agg (p-major rows within each core
            # section); zagA/zagB are 256B-stride padded halves for dma_gather
            zloc = dp.tile([PADN, 32], BF16)
            SPLr = (NBLK // 2) * 128
            zag1 = dp.tile([cfg.ncores * SPLr, 32], BF16, addr_space="Shared")
            zag2 = dp.tile([cfg.ncores * (PADN - SPLr), 32], BF16,
                           addr_space="Shared")
            zagA = dp.tile([cfg.ncores // 2 * PADN, 128], BF16)
            zagB = dp.tile([cfg.ncores // 2 * PADN, 128], BF16)

            # ---------------- Phase A: per-edge MLP + segment sums ----------
            with (
                tc.tile_pool(name="eatp", bufs=3) as eatp,
                tc.tile_pool(name="workA", bufs=4) as wp,
                tc.tile_pool(name="psA", bufs=2, space="PSUM") as psA,
                tc.tile_pool(name="psB", bufs=2, space="PSUM") as psB,
                tc.tile_pool(name="psP", bufs=2, space="PSUM") as psP,
                tc.tile_pool(name="psO", bufs=2, space="PSUM") as psO,
            ):
                ech = None
                for b in range(NBLK):
                    if b % EB == 0:
                        nch = min(EB, NBLK - b)
                        ech = eatp.tile([33, nch * TPB * 128], BF16, tag="ech")
                        nc.sync.dma_start(
                            ech[:], eat[:, b * TPB * 128:(b + nch) * TPB * 128])
                    e0 = (b % EB) * TPB * 128
                    bankA = psA.tile([128, 32], F32, tag="bankA")
                    bankB = psB.tile([128, C * 32], F32, tag="bankB")
                    for j in range(C):
                        if "slotmm" in skip:
                            break
                        lh = ech[:, e0 + j * 128:e0 + (j + 1) * 128]
                        nc.tensor.matmul(bankA[:], lh, rc[:, 0:32],
                                         start=(j == 0), stop=False)
                        nc.tensor.matmul(bankB[:, j * 32:(j + 1) * 32], lh,
                                         rc[:, 32:64],
                                         start=(j == 0), stop=(j == C - 1))
                    # overflow: pre-act for t_ov tiles in one PSUM tile
                    pc = psP.tile([128, t_ov * 64], F32, tag="pc")
                    ov = wp.tile([128, t_ov * 64], BF16, tag="ov")
                    for o in range(t_ov):
                        lh = ech[:, e0 + (C + o) * 128:e0 + (C + o + 1) * 128]
                        nc.tensor.matmul(pc[:, o * 64:(o + 1) * 64], lh, rc[:, 0:64],
                                         start=(o == 0), stop=(o == t_ov - 1))
                    # ea part: copy (Act); h_e part: relu (DVE)
                    nc.scalar.copy(
                        ov[:].rearrange("p (t x) -> p t x", x=64)[:, :, 0:32],
                        pc[:].rearrange("p (t x) -> p t x", x=64)[:, :, 0:32])
                    nc.vector.tensor_scalar_max(
                        ov[:].rearrange("p (t x) -> p t x", x=64)[:, :, 32:64],
                        pc[:].rearrange("p (t x) -> p t x", x=64)[:, :, 32:64],
                        0.0)
                    # one-hot scatter matrices (cached for Phase B reuse)
                    for o in range(t_ov):
                        k = b * t_ov + o
                        nc.gpsimd.tensor_scalar(
                            out=oh_cache[:, k * 128:(k + 1) * 128], in0=iota_sb[:],
                            scalar1=rlov_sb[:, k:k + 1], scalar2=None,
                            op0=OP.is_equal,
                        )
                        # fold overflow ea into bankA accumulation group
                        nc.tensor.matmul(bankA[:], oh_cache[:, k * 128:(k + 1) * 128],
                                         ov[:, o * 64:o * 64 + 32],
                                         start=False, stop=(o == t_ov - 1))
                    # t_s = (1/deg) * S_ea (kept bf16 for Phase C transpose;
                    # the final sqd scale rides the peer relu)
                    nc.vector.tensor_scalar_mul(
                        ts_cache[:, b * 32:(b + 1) * 32], bankA[:],
                        deginv_sb[:, b:b + 1])
                    # h_e slots: relu on Act, slot-sum on DVE into PSUM
                    relu_st = wp.tile([128, C * 32], BF16, tag="relu")
                    nc.scalar.activation(relu_st[:], bankB[:], ACT.Relu)
                    t_he = wp.tile([128, 32], F32, tag="the")
                    nc.vector.tensor_reduce(
                        t_he[:],
                        relu_st[:].rearrange("p (j c) -> p c j", j=C),
                        axis=AX.X, op=OP.add,
                    )
                    bankO = psO.tile([128, 32], F32, tag="bankO")
                    for o in range(t_ov):
                        k = b * t_ov + o
                        nc.tensor.matmul(bankO[:], oh_cache[:, k * 128:(k + 1) * 128],
                                         ov[:, o * 64 + 32:(o + 1) * 64],
                                         start=(o == 0), stop=(o == t_ov - 1))
                    nc.vector.tensor_tensor(
                        out=heagg_bf[:, b * 32:(b + 1) * 32],
                        in0=t_he[:], in1=bankO[:], op=OP.add)
                    # outst[64:96] = h_e_agg; pad cols 224:256 need some value
                    nc.vector.tensor_copy(
                        outst[:, b * OUTP + 64:b * OUTP + 96],
                        heagg_bf[:, b * 32:(b + 1) * 32])
                    nc.vector.tensor_copy(
                        outst[:, b * OUTP + 224:b * OUTP + 256],
                        heagg_bf[:, b * 32:(b + 1) * 32])

            # Share h_e_agg via TWO pipelined compact AllGathers (b-major
            # rows, so the first fires as soon as blocks 0..SPL-1 are done,
            # overlapping the tail of Phase A), then locally re-pad each half
            # into a 256B-stride table for the int16 dma_gather.
            SPL = (NBLK // 2) * 128
            nc.sync.dma_start(
                zloc[0:SPL, :].rearrange("(b p) c -> p b c", p=128),
                heagg_bf[:, 0:(SPL // 128) * 32].rearrange(
                    "p (b c) -> p b c", c=32),
            )
            nc.sync.dma_start(
                zloc[SPL:PADN, :].rearrange("(b p) c -> p b c", p=128),
                heagg_bf[:, (SPL // 128) * 32:].rearrange(
                    "p (b c) -> p b c", c=32),
            )
            if "collective" not in skip:
                nc.gpsimd.collective_compute(
                    "AllGather", OP.bypass,
                    ins=[zloc[0:SPL, :]], outs=[zag1.opt()],
                    replica_groups=[list(range(cfg.ncores))],
                )
                nc.gpsimd.collective_compute(
                    "AllGather", OP.bypass,
                    ins=[zloc[SPL:PADN, :]], outs=[zag2.opt()],
                    replica_groups=[list(range(cfg.ncores))],
                )
            NH = cfg.ncores // 2
            SPL2 = PADN - SPL
            for hi, ztab in ((0, zagA), (1, zagB)):
                nc.sync.dma_start(
                    ztab[:, 0:32].rearrange("(r w) c -> r w c", r=NH)[:, 0:SPL, :],
                    zag1[hi * NH * SPL:(hi + 1) * NH * SPL, :].rearrange(
                        "(r w) c -> r w c", r=NH),
                )
                nc.sync.dma_start(
                    ztab[:, 0:32].rearrange("(r w) c -> r w c", r=NH)[:, SPL:, :],
                    zag2[hi * NH * SPL2:(hi + 1) * NH * SPL2, :].rearrange(
                        "(r w) c -> r w c", r=NH),
                )

            # ------------- Phase C: node MLPs (overlaps the collective) -----
            if "phasec" not in skip:
                with (
                    tc.tile_pool(name="workC", bufs=4) as wc,
                    tc.tile_pool(name="ps1", bufs=2, space="PSUM") as ps1,
                    tc.tile_pool(name="ps3", bufs=2, space="PSUM") as ps3,
                    tc.tile_pool(name="psT", bufs=2, space="PSUM") as psT,
                ):
                    for b in range(NBLK):
                        xb = xt_sb[:, b * 128:(b + 1) * 128]
                        # h_ego = relu(x W_ego^T + b_ego)
                        p1 = ps1.tile([128, 64], F32, tag="p1")
                        nc.tensor.matmul(p1[:], ones_sb[:], bego_sb[:],
                                         start=True, stop=False)
                        nc.tensor.matmul(p1[:], xb, wego_sb[:],
                                         start=False, stop=True)
                        nc.scalar.activation(
                            outst[:, b * OUTP:b * OUTP + 64], p1[:], ACT.Relu)
                        # h_peer = relu(sqd * (W_px x + b_p + W_pe ((1/deg) S_ea)))
                        pt = psT.tile([32, 128], BF16, tag="pt")
                        nc.tensor.matmul(pt[:], ts_cache[:, b * 32:(b + 1) * 32],
                                         ident_sb[:], is_transpose=True,
                                         start=True, stop=True)
                        seat = wc.tile([32, 128], BF16, tag="seat")
                        nc.scalar.copy(seat[:], pt[:])
                        p3 = ps3.tile([128, 96], F32, tag="p3")
                        nc.tensor.matmul(p3[:], ones_sb[:], bpeer_sb[:],
                                         start=True, stop=False)
                        nc.tensor.matmul(p3[:], xb, wpx_sb[:],
                                         start=False, stop=False)
                        nc.tensor.matmul(p3[:], seat[:], wpe_sb[:],
                                         start=False, stop=True)
                        nc.scalar.activation(
                            outst[:, b * OUTP + 128:b * OUTP + 224], p3[:],
                            ACT.Relu, scale=sqd_sb[:, b:b + 1])

            # ------------- Phase B: gather + M + output writes --------------
            # dma_gather caps at 1024 indices (8 tiles); CH=8 main tiles fill
            # one instruction per (block, half), and the per-block overflow
            # tiles of a whole chunk merge into ONE instruction per half.
            # g layout per chunk: [half][main: block-major 8 tiles][ov:
            # block-major t_ov2 tiles].
            CH2, tov2, THALF = cfg.CH, cfg.t_ov2, cfg.THALF
            assert CH2 == 8
            with (
                tc.tile_pool(name="workB", bufs=4) as wb,
                tc.tile_pool(name="psM", bufs=3, space="PSUM") as psM,
            ):
                for qi, g0 in enumerate(range(0, NBLK, GB)):
                    ng = min(GB, NBLK - g0)
                    span = ng * THALF
                    g = wb.tile([128, 2 * span * 32], BF16, tag="g")
                    if "gather" not in skip:
                        for h, ztab in ((0, zagA), (1, zagB)):
                            qh = (qi * 2 + h) * GB * THALF * 8
                            for br in range(ng):
                                nc_ = raw_dma_gather(
                                    nc,
                                    g[:, (h * span + br * 8) * 32:
                                      (h * span + br * 8 + 8) * 32].rearrange(
                                        "p (t e) -> p t e", e=32),
                                    ztab[:, 0:32],
                                    gidx16_sb[:, qh + br * 64:qh + br * 64 + 64],
                                    num_idxs=1024, elem_size=32, elem_step=128)
                            nov = ng * tov2
                            raw_dma_gather(
                                nc,
                                g[:, (h * span + ng * 8) * 32:
                                  (h * span + ng * 8 + nov) * 32].rearrange(
                                    "p (t e) -> p t e", e=32),
                                ztab[:, 0:32],
                                gidx16_sb[:, qh + GB * 64:qh + GB * 64 + nov * 8],
                                num_idxs=nov * 128, elem_size=32, elem_step=128)
                    gv = g[:].rearrange("p (h z c) -> p c h z", h=2, c=32)
                    for br in range(ng):
                        b = g0 + br
                        m_sb = wb.tile([128, 32], F32, tag="msb")
                        nc.vector.tensor_reduce(
                            m_sb[:],
                            gv[:, :, :, br * 8:br * 8 + 8],
                            axis=AX.XY, op=OP.add,
                        )
                        pm = psM.tile([128, 32], F32, tag="pm")
                        oh2 = wb.tile([128, 2 * tov2 * 128], BF16, tag="oh2")
                        nmm = 2 * tov2
                        for h in range(2):
                            for o in range(tov2):
                                k = h * tov2 + o
                                nc.vector.tensor_scalar(
                                    out=oh2[:, k * 128:(k + 1) * 128], in0=iota_sb[:],
                                    scalar1=rlov2_sb[:, (b * 2 + h) * tov2 + o:
                                                     (b * 2 + h) * tov2 + o + 1],
                                    scalar2=None, op0=OP.is_equal,
                                )
                                nc.tensor.matmul(
                                    pm[:], oh2[:, k * 128:(k + 1) * 128],
                                    g[:, (h * span + ng * 8 + br * tov2 + o) * 32:
                                      (h * span + ng * 8 + br * tov2 + o + 1) * 32],
                                    start=(k == 0), stop=(k == nmm - 1))
                        t_m = wb.tile([128, 32], F32, tag="tm")
                        nc.vector.tensor_tensor(
                            out=t_m[:], in0=m_sb[:], in1=pm[:], op=OP.add)
                        nc.scalar.activation(
                            outst[:, b * OUTP + 96:b * OUTP + 128], t_m[:],
                            ACT.Copy, scale=dinv_sb[:, b:b + 1])
                    nc.sync.dma_start(
                        outp[g0 * 128:(g0 + ng) * 128, :].rearrange(
                            "(q p) c -> p q c", p=128),
                        outst[:, g0 * OUTP:(g0 + ng) * OUTP].rearrange(
                            "p (q c) -> p q c", c=OUTP),
                    )
    nc.compile()
    return nc


def host_prep(cfg, x, edge_attrs, edge_index):
    """Shard + lay out inputs for the slot-grid kernel. Pure index work + O(N)
    scalar prep (degree normalizers); all O(E*H)/O(N*H) FP math runs on device."""
    BF = np.float16
    N, E, C, NBLK, TPB, t_ov, NPC, PADN = (cfg.N, cfg.E, cfg.C, cfg.NBLK,
                                           cfg.TPB, cfg.t_ov, cfg.NPC, cfg.PADN)
    row = np.asarray(edge_index[1]).astype(np.int64)
    col = np.asarray(edge_index[0]).astype(np.int64)
    ea = np.asarray(edge_attrs, dtype=np.float32)
    xf = np.asarray(x, dtype=np.float32)

    deg = np.bincount(row, minlength=N)
    degf = np.maximum(deg, 1).astype(np.float64)
    dinv = np.where(deg > 0, degf ** -0.5, 0.0).astype(np.float32)
    sqd = np.sqrt(deg.astype(np.float64)).astype(np.float32)

    core = row // NPC
    lrow = row - core * NPC
    blk = lrow // 128
    part = lrow % 128

    # rank of each edge within its destination node
    order = np.argsort(row, kind="stable")
    sorted_row = row[order]
    starts = np.searchsorted(sorted_row, np.arange(N), side="left")
    rank = np.empty(E, np.int64)
    rank[order] = np.arange(E) - starts[sorted_row]

    is_grid = rank < C
    ove = np.where(~is_grid)[0]
    ovkey = core[ove] * NBLK + blk[ove]
    o_order = np.argsort(ovkey, kind="stable")
    ove = ove[o_order]
    okey_sorted = ovkey[o_order]
    ostarts = np.searchsorted(okey_sorted, np.arange(NBLK * cfg.ncores), side="left")
    opos = np.arange(ove.size) - ostarts[okey_sorted]
    otile = C + opos // 128
    opart = opos % 128
    if ove.size and otile.max() >= TPB:
        raise ValueError("overflow tiles exceeded; raise t_ov")

    tile_idx = np.empty(E, np.int64)
    tpart = np.empty(E, np.int64)
    ge = np.where(is_grid)[0]
    tile_idx[ge] = blk[ge] * TPB + rank[ge]
    tpart[ge] = part[ge]
    tile_idx[ove] = blk[ove] * TPB + otile
    tpart[ove] = opart

    # ---- Phase-B dual gather grid: edges split by source half ----------
    # half h = src_core // 4; within-half row (p-major per core section):
    #   (src_core % 4) * PADN + (l % 128) * NBLK + l // 128,  l = col % NPC
    CH, t_ov2, THALF, IW = cfg.CH, cfg.t_ov2, cfg.THALF, cfg.IW
    src_core = col // NPC
    lcol = col % NPC
    halfs = src_core // (cfg.ncores // 2)
    zrow16 = (src_core % (cfg.ncores // 2)) * PADN + lcol  # b-major rows
    ZPADH = 127 * NBLK + (NBLK - 1)   # half-core-0 pad node -> zeros

    # rank of each edge within (dest node, half)
    key2 = row * 2 + halfs
    order2 = np.argsort(key2, kind="stable")
    sk2 = key2[order2]
    starts2 = np.searchsorted(sk2, np.arange(2 * N), side="left")
    rank2 = np.empty(E, np.int64)
    rank2[order2] = np.arange(E) - starts2[sk2]

    is_grid2 = rank2 < CH
    ove2 = np.where(~is_grid2)[0]
    ovkey2 = (core[ove2] * NBLK + blk[ove2]) * 2 + halfs[ove2]
    o_order2 = np.argsort(ovkey2, kind="stable")
    ove2 = ove2[o_order2]
    ok2_sorted = ovkey2[o_order2]
    ostarts2 = np.searchsorted(ok2_sorted, np.arange(NBLK * cfg.ncores * 2),
                               side="left")
    opos2 = np.arange(ove2.size) - ostarts2[ok2_sorted]
    otile2 = CH + opos2 // 128
    opart2 = opos2 % 128
    if ove2.size and otile2.max() >= THALF:
        raise ValueError("phase-B overflow tiles exceeded; raise t_ov2")

    tile2 = np.empty(E, np.int64)
    tpart2 = np.empty(E, np.int64)
    ge2 = np.where(is_grid2)[0]
    tile2[ge2] = rank2[ge2]
    tpart2[ge2] = part[ge2]
    tile2[ove2] = otile2
    tpart2[ove2] = opart2

    TOTC = NBLK * TPB
    in_maps = []
    for r in range(cfg.ncores):
        sel = core == r
        e_idx = np.where(sel)[0]
        t_i = tile_idx[e_idx]
        t_p = tpart[e_idx]
        colpos = t_i * 128 + t_p

        EAT = np.zeros((33, TOTC * 128), np.float32)
        EAT[:32, colpos] = ea[e_idx].T
        EAT[32, colpos] = 1.0

        # int16 gather indices, per (chunk, half): [main: block-major 8 tiles
        # | ov: block-major t_ov2 tiles]; each instruction's list is stored
        # 16-partition-wrapped [16g + i%16, i//16], replicated x8.
        GB2, NQ = cfg.GB, (NBLK + cfg.GB - 1) // cfg.GB
        eb = blk[e_idx]
        eq, ebr = eb // GB2, eb % GB2
        eh, et, ep = halfs[e_idx], tile2[e_idx], tpart2[e_idx]
        secsz = GB2 * THALF * 128            # idx slots per (chunk, half)
        main_sel = et < CH
        ipos = np.where(
            main_sel,
            (eq * 2 + eh) * secsz + (ebr * 8 + et) * 128 + ep,
            (eq * 2 + eh) * secsz + GB2 * 8 * 128
            + (ebr * t_ov2 + (et - CH)) * 128 + ep)
        flat = np.full(NQ * 2 * secsz, ZPADH, np.int32)
        flat[ipos] = zrow16[e_idx]
        wrap = (flat.reshape(NQ * 2 * GB2 * THALF, 8, 16).transpose(2, 0, 1)
                .reshape(16, NQ * 2 * GB2 * THALF * 8).astype(np.int16))
        GIDX16 = np.tile(wrap, (8, 1))

        RLOV = np.full((128, NBLK * t_ov), 200.0, np.float32)
        ov_r = ove[core[ove] == r]
        op_r = opart[core[ove] == r]
        ot_r = otile[core[ove] == r]
        ob_r = blk[ov_r]
        RLOV[op_r, ob_r * t_ov + (ot_r - C)] = part[ov_r].astype(np.float32)

        RLOV2 = np.full((128, NBLK * 2 * t_ov2), 200.0, np.float32)
        sel2 = core[ove2] == r
        ov2_r = ove2[sel2]
        RLOV2[opart2[sel2],
              (blk[ov2_r] * 2 + halfs[ov2_r]) * t_ov2
              + (otile2[sel2] - CH)] = part[ov2_r].astype(np.float32)

        lo, hi = r * NPC, (r + 1) * NPC
        XT = np.zeros((128, PADN), np.float32)
        XT[:, :NPC] = xf[lo:hi].T
        dl = np.zeros(PADN, np.float32)
        dl[:NPC] = dinv[lo:hi]
        sl = np.zeros(PADN, np.float32)
        sl[:NPC] = sqd[lo:hi]
        gl = np.zeros(PADN, np.float32)
        gl[:NPC] = dinv[lo:hi] ** 2          # 1/deg (0 for deg==0)
        DINV = dl.reshape(NBLK, 128).T.copy()
        SQD = sl.reshape(NBLK, 128).T.copy()
        DEGINV = gl.reshape(NBLK, 128).T.copy()

        m = {
            "eat": EAT.astype(BF), "gidx16": GIDX16, "rlov": RLOV,
            "rlov2": RLOV2, "xt": XT.astype(BF), "dinv": DINV, "sqd": SQD,
            "deginv": DEGINV,
        }
        in_maps.append(m)
    return in_maps


def make_consts(cfg, W_peer, b_peer, W_ego, b_ego, W_edge, b_edge):
    BF = np.float16
    RCONST = np.zeros((33, 64), np.float32)
    RCONST[:32, :32] = np.eye(32, dtype=np.float32)
    RCONST[:32, 32:64] = np.asarray(W_edge, np.float32).T
    RCONST[32, 32:64] = np.asarray(b_edge, np.float32)
    consts = {
        "rconst": RCONST.astype(BF),
        "wegot": np.ascontiguousarray(np.asarray(W_ego, np.float32).T).astype(BF),
        "wpxt": np.ascontiguousarray(np.asarray(W_peer, np.float32)[:, :128].T).astype(BF),
        "wpet": np.ascontiguousarray(np.asarray(W_peer, np.float32)[:, 128:].T).astype(BF),
        "bego": np.asarray(b_ego, np.float32).reshape(1, 64).astype(BF),
        "bpeer": np.asarray(b_peer, np.float32).reshape(1, 96).astype(BF),
        "iota": np.broadcast_to(np.arange(128, dtype=np.float32), (128, 128)).astype(BF).copy(),
        "ident": np.eye(128, dtype=np.float32).astype(BF),
    }
    return consts


_CACHE = {}
RUN_KWARGS = {}


def kernel(x, edge_attrs, W_peer, b_peer, W_ego, b_ego, W_edge, b_edge, edge_index):
    x = np.asarray(x)
    edge_attrs = np.asarray(edge_attrs)
    edge_index = np.asarray(edge_index)
    N, E = x.shape[0], edge_attrs.shape[0]

    row = edge_index[1].astype(np.int64)
    col = edge_index[0].astype(np.int64)
    C = 15
    CH = 8
    ncores = 8
    NPC = N // ncores
    NBLK = (NPC + 127) // 128
    deg = np.bincount(row, minlength=N)
    ovn = np.maximum(deg - C, 0)
    nodes = np.arange(N)
    bkey = (nodes // NPC) * NBLK + (nodes % NPC) // 128
    ovblk = np.bincount(bkey, weights=ovn.astype(np.float64), minlength=NBLK * ncores)
    t_ov = max(3, int(np.ceil(ovblk.max() / 128.0)))

    # phase-B dual-grid overflow sizing (per dest block x source half)
    halfs = (col // NPC) // (ncores // 2)
    degh = np.bincount(row * 2 + halfs, minlength=2 * N)
    ovn2 = np.maximum(degh - CH, 0)
    bkey2 = np.repeat(bkey, 2) * 2 + np.tile([0, 1], N)
    ovblk2 = np.bincount(bkey2, weights=ovn2.astype(np.float64),
                         minlength=2 * NBLK * ncores)
    t_ov2 = max(2, int(np.ceil(ovblk2.max() / 128.0)))

    cfg = Cfg(N=N, E=E, ncores=ncores, C=C, t_ov=t_ov, CH=CH, t_ov2=t_ov2)
    key = cfg.key()
    if key not in _CACHE:
        _CACHE[key] = build_program(cfg)
    nc = _CACHE[key]

    in_maps = host_prep(cfg, x, edge_attrs, edge_index)
    consts = make_consts(cfg, W_peer, b_peer, W_ego, b_ego, W_edge, b_edge)
    for m in in_maps:
        m.update(consts)

    res = run_bass_kernel_spmd(nc, in_maps, core_ids=list(range(cfg.ncores)),
                               **RUN_KWARGS)
    out = np.empty((N, cfg.OUTD), np.float32)
    for r in range(cfg.ncores):
        blk = np.asarray(res.results[r]["out"]).astype(np.float32)
        out[r * cfg.NPC:(r + 1) * cfg.NPC] = blk[:cfg.NPC, :cfg.OUTD]
    kernel.last_result = res
    return out


# revision 46
# speedup vs baseline: 3.5776x; 1.0055x over previous
"""Trainium2 Bass kernel: CANE FeatureEmbedding GNN message passing.

Strategy (node-range sharding, 8 cores), v2:
  - Nodes range-partitioned; edges assigned to the core owning their
    DESTINATION (row = edge_index[1]).
  - Algebraic collapse of the peer branch (gather and scatter both use `row`):
        h_peer[i] = relu( sqrt(deg_i)*(W_px x_i + b_p) + deg_i^-1/2 * (W_pe S_ea[i]) )
    removing the [E,160]x[160,96] per-edge MLP.
  - Remaining per-edge work: h_e = relu(ea @ W_edge.T + b_edge), segment sums
    of ea and h_e over row, and M[i] = sum_{e: row=i} h_e_agg[col[e]].
  - Slot grid: node v gets C slots; edge k of v goes to (block v//128,
    partition v%128, slot k). One bf16 matmul per slot tile accumulates S_ea
    in PSUM; h_e pre-acts land per-slot, relu on the Act engine, slot-sum on
    DVE straight into PSUM, overflow (deg>C) via one-hot scatter matmuls.
  - h_e_agg shared across cores via a compact fp16 AllGather, then locally
    re-padded into two 256B-stride half-tables (<=32767 rows each, int16
    indexable). M is built with dma_gather (64B payload / 256B stride, <=1024
    indices per instruction -- the SWDGE ring cap) over a second slot grid
    split by source half, reduced along (half, slot) on DVE into PSUM with
    one-hot matmul accumulation for overflow.
  - Everything streams in fp16 (4x finer mantissa than bf16 at identical
    cost); PSUM accumulation in fp32; output written as [PADN, 256] fp16 rows
    (cols 224:256 pad) and upcast host-side.
"""

import numpy as np

import concourse.bass as bass
import concourse.mybir as mybir
import concourse.tile as tile
from concourse import bacc
from concourse._compat import get_trn_type
from concourse.bass import IndirectOffsetOnAxis
from concourse.bass_utils import run_bass_kernel_spmd

F32 = mybir.dt.float32
BF16 = mybir.dt.float16  # fp16: 4x finer mantissa than bf16, same perf
I32 = mybir.dt.int32
I16 = mybir.dt.int16
AX = mybir.AxisListType
OP = mybir.AluOpType
ACT = mybir.ActivationFunctionType


class Cfg:
    def __init__(self, N=50000, E=800000, ncores=8, C=15, t_ov=3, GB=4, EB=4,
                 CH=8, t_ov2=2):
        self.N = N
        self.E = E
        self.ncores = ncores
        self.NPC = N // ncores            # nodes per core
        self.NBLK = (self.NPC + 127) // 128
        self.PADN = self.NBLK * 128       # padded nodes per core
        self.C = C                        # grid slots per node
        self.t_ov = t_ov                  # overflow tiles per block
        self.TPB = C + t_ov               # tiles per block
        self.GB = GB                      # blocks per gather instruction
        self.EB = EB                      # blocks per eat-load DMA
        self.CH = CH                      # phase-B main slots per (node, half)
        self.t_ov2 = t_ov2                # phase-B overflow tiles per (block, half)
        self.THALF = CH + t_ov2           # phase-B tiles per (block, half)
        self.IW = (self.THALF * 128 + 15) // 16   # int16 idx cols per gather
        self.SPLB = 36                    # blocks in the first AllGather
        self.ND = 128
        self.ED = 32
        self.OUTD = 224
        self.OUTP = 256                   # padded out row

    def key(self):
        return (self.N, self.E, self.ncores, self.C, self.t_ov, self.GB,
                self.EB, self.CH, self.t_ov2)


def raw_dma_gather(nc, out_ap, in_ap, idxs_ap, num_idxs, elem_size, elem_step):
    """nc.gpsimd.dma_gather minus the %256 payload assert (the 256B rule
    applies to the row STRIDE, which elem_step satisfies; the ucode packetizes
    the payload at any size)."""
    gp = nc.gpsimd
    stride_bytes = elem_step * mybir.dt.size(in_ap.dtype)
    assert stride_bytes % 256 == 0 and stride_bytes // 256 < 256
    _in_ap = gp.lower_ap_dma(in_ap, for_custom_bir_dma=True)
    _idxs_ap = gp.lower_ap(idxs_ap)
    _out_ap = gp.lower_ap(out_ap)
    return gp.add_instruction(
        mybir.InstDMAGatherAnt(
            name=f"I-{nc.next_id()}",
            ins=[*_in_ap, _idxs_ap, gp.lower_val_access(gp.to_reg(num_idxs))],
            outs=[_out_ap],
            transpose=False,
            num_idxs=num_idxs,
            elem_size=elem_size,
            stride_bytes_256=stride_bytes // 256,
            gen_mode=0,
            single_packet=True,
            queue_num=0,
            sbuf_tokens_per_rank=0,
            sbuf_free_dim_per_rank=0,
            sbuf_free_dim_pad_per_rank=0,
            sbuf_byte_offset=0,
        )
    )


def build_program(cfg, skip=()):
    """Build the SPMD Bass program (same NEFF for all cores).

    skip: {"gather", "slotmm", "overflow", "phasec", "collective"} — timing
    experiment knobs only (results are wrong when used)."""
    skip = set(skip)
    nc = bacc.Bacc(get_trn_type() or "TRN2", target_bir_lowering=False, debug=True)
    NBLK, TPB, C, t_ov, PADN = cfg.NBLK, cfg.TPB, cfg.C, cfg.t_ov, cfg.PADN
    GB, EB, OUTP = cfg.GB, cfg.EB, cfg.OUTP
    TOT = NBLK * TPB

    eat = nc.declare_dram_parameter("eat", [33, TOT * 128], BF16, isOutput=False)
    NQ = (NBLK + GB - 1) // GB
    gidx16 = nc.declare_dram_parameter(
        "gidx16", [128, NQ * 2 * GB * cfg.THALF * 8], I16, isOutput=False)
    rlov2 = nc.declare_dram_parameter("rlov2", [128, NBLK * 2 * cfg.t_ov2], F32,
                                      isOutput=False)
    rlov = nc.declare_dram_parameter("rlov", [128, NBLK * t_ov], F32, isOutput=False)
    xt = nc.declare_dram_parameter("xt", [128, PADN], BF16, isOutput=False)
    dinvp = nc.declare_dram_parameter("dinv", [128, NBLK], F32, isOutput=False)
    sqdp = nc.declare_dram_parameter("sqd", [128, NBLK], F32, isOutput=False)
    deginvp = nc.declare_dram_parameter("deginv", [128, NBLK], F32, isOutput=False)
    rconst = nc.declare_dram_parameter("rconst", [33, 64], BF16, isOutput=False)
    wegot = nc.declare_dram_parameter("wegot", [128, 64], BF16, isOutput=False)
    wpxt = nc.declare_dram_parameter("wpxt", [128, 96], BF16, isOutput=False)
    wpet = nc.declare_dram_parameter("wpet", [32, 96], BF16, isOutput=False)
    bego = nc.declare_dram_parameter("bego", [1, 64], BF16, isOutput=False)
    bpeer = nc.declare_dram_parameter("bpeer", [1, 96], BF16, isOutput=False)
    iota = nc.declare_dram_parameter("iota", [128, 128], BF16, isOutput=False)
    ident = nc.declare_dram_parameter("ident", [128, 128], BF16, isOutput=False)
    outp = nc.declare_dram_parameter("out", [PADN, OUTP], BF16, isOutput=True)

    with tile.TileContext(nc) as tc:
        with (
            tc.tile_pool(name="const", bufs=1) as cp,
            tc.tile_pool(name="resident", bufs=1) as rp,
            tc.tile_pool(name="dram", bufs=1, space="DRAM") as dp,
        ):
            rc = cp.tile([33, 64], BF16)
            nc.sync.dma_start(rc[:], rconst[:])
            wego_sb = cp.tile([128, 64], BF16)
            nc.sync.dma_start(wego_sb[:], wegot[:])
            wpx_sb = cp.tile([128, 96], BF16)
            nc.sync.dma_start(wpx_sb[:], wpxt[:])
            wpe_sb = cp.tile([32, 96], BF16)
            nc.sync.dma_start(wpe_sb[:], wpet[:])
            bego_sb = cp.tile([1, 64], BF16)
            nc.sync.dma_start(bego_sb[:], bego[:])
            bpeer_sb = cp.tile([1, 96], BF16)
            nc.sync.dma_start(bpeer_sb[:], bpeer[:])
            iota_sb = cp.tile([128, 128], BF16)
            nc.sync.dma_start(iota_sb[:], iota[:])
            ident_sb = cp.tile([128, 128], BF16)
            nc.sync.dma_start(ident_sb[:], ident[:])
            ones_sb = cp.tile([1, 128], BF16)
            nc.gpsimd.memset(ones_sb[:], 1.0)

            xt_sb = rp.tile([128, PADN], BF16)
            nc.sync.dma_start(xt_sb[:], xt[:])
            gidx16_sb = rp.tile([128, NQ * 2 * GB * cfg.THALF * 8], I16)
            nc.sync.dma_start(gidx16_sb[:], gidx16[:])
            rlov2_sb = rp.tile([128, NBLK * 2 * cfg.t_ov2], F32)
            nc.sync.dma_start(rlov2_sb[:], rlov2[:])
            rlov_sb = rp.tile([128, NBLK * t_ov], F32)
            nc.sync.dma_start(rlov_sb[:], rlov[:])
            dinv_sb = rp.tile([128, NBLK], F32)
            nc.sync.dma_start(dinv_sb[:], dinvp[:])
            sqd_sb = rp.tile([128, NBLK], F32)
            nc.sync.dma_start(sqd_sb[:], sqdp[:])
            deginv_sb = rp.tile([128, NBLK], F32)
            nc.sync.dma_start(deginv_sb[:], deginvp[:])

            heagg_bf = rp.tile([128, NBLK * 32], BF16)   # h_e_agg, bf16
            ts_cache = rp.tile([128, NBLK * 32], BF16)   # dinv * S_ea, bf16
            oh_cache = rp.tile([128, NBLK * t_ov * 128], BF16)  # one-hots
            outst = rp.tile([128, NBLK * OUTP], BF16)    # output staging

            # zag holds all cores' h_e_agg (p-major rows within each core
            # section); zagA/zagB are 256B-stride padded halves for dma_gather
            zloc = dp.tile([PADN, 32], BF16)
            SPLr = cfg.SPLB * 128
            zag1 = dp.tile([cfg.ncores * SPLr, 32], BF16, addr_space="Shared")
            zag2 = dp.tile([cfg.ncores * (PADN - SPLr), 32], BF16,
                           addr_space="Shared")
            zagA = dp.tile([cfg.ncores // 2 * PADN, 128], BF16)
            zagB = dp.tile([cfg.ncores // 2 * PADN, 128], BF16)

            # ---------------- Phase A: per-edge MLP + segment sums ----------
            with (
                tc.tile_pool(name="eatp", bufs=3) as eatp,
                tc.tile_pool(name="workA", bufs=4) as wp,
                tc.tile_pool(name="psA", bufs=2, space="PSUM") as psA,
                tc.tile_pool(name="psB", bufs=2, space="PSUM") as psB,
                tc.tile_pool(name="psP", bufs=2, space="PSUM") as psP,
                tc.tile_pool(name="psO", bufs=2, space="PSUM") as psO,
            ):
                ech = None
                for b in range(NBLK):
                    if b % EB == 0:
                        nch = min(EB, NBLK - b)
                        ech = eatp.tile([33, nch * TPB * 128], BF16, tag="ech")
                        nc.sync.dma_start(
                            ech[:], eat[:, b * TPB * 128:(b + nch) * TPB * 128])
                    e0 = (b % EB) * TPB * 128
                    bankA = psA.tile([128, 32], F32, tag="bankA")
                    bankB = psB.tile([128, C * 32], F32, tag="bankB")
                    for j in range(C):
                        if "slotmm" in skip:
                            break
                        lh = ech[:, e0 + j * 128:e0 + (j + 1) * 128]
                        nc.tensor.matmul(bankA[:], lh, rc[:, 0:32],
                                         start=(j == 0), stop=False)
                        nc.tensor.matmul(bankB[:, j * 32:(j + 1) * 32], lh,
                                         rc[:, 32:64],
                                         start=(j == 0), stop=(j == C - 1))
                    # overflow: pre-act for t_ov tiles in one PSUM tile
                    pc = psP.tile([128, t_ov * 64], F32, tag="pc")
                    ov = wp.tile([128, t_ov * 64], BF16, tag="ov")
                    for o in range(t_ov):
                        lh = ech[:, e0 + (C + o) * 128:e0 + (C + o + 1) * 128]
                        nc.tensor.matmul(pc[:, o * 64:(o + 1) * 64], lh, rc[:, 0:64],
                                         start=(o == 0), stop=(o == t_ov - 1))
                    # ea part: copy (Act); h_e part: relu (DVE)
                    nc.scalar.copy(
                        ov[:].rearrange("p (t x) -> p t x", x=64)[:, :, 0:32],
                        pc[:].rearrange("p (t x) -> p t x", x=64)[:, :, 0:32])
                    nc.vector.tensor_scalar_max(
                        ov[:].rearrange("p (t x) -> p t x", x=64)[:, :, 32:64],
                        pc[:].rearrange("p (t x) -> p t x", x=64)[:, :, 32:64],
                        0.0)
                    # one-hot scatter matrices (cached for Phase B reuse)
                    for o in range(t_ov):
                        k = b * t_ov + o
                        nc.gpsimd.tensor_scalar(
                            out=oh_cache[:, k * 128:(k + 1) * 128], in0=iota_sb[:],
                            scalar1=rlov_sb[:, k:k + 1], scalar2=None,
                            op0=OP.is_equal,
                        )
                        # fold overflow ea into bankA accumulation group
                        nc.tensor.matmul(bankA[:], oh_cache[:, k * 128:(k + 1) * 128],
                                         ov[:, o * 64:o * 64 + 32],
                                         start=False, stop=(o == t_ov - 1))
                    # t_s = (1/deg) * S_ea (kept bf16 for Phase C transpose;
                    # the final sqd scale rides the peer relu)
                    nc.vector.tensor_scalar_mul(
                        ts_cache[:, b * 32:(b + 1) * 32], bankA[:],
                        deginv_sb[:, b:b + 1])
                    # h_e slots: relu on Act, slot-sum on DVE into PSUM
                    relu_st = wp.tile([128, C * 32], BF16, tag="relu")
                    nc.scalar.activation(relu_st[:], bankB[:], ACT.Relu)
                    t_he = wp.tile([128, 32], F32, tag="the")
                    nc.vector.tensor_reduce(
                        t_he[:],
                        relu_st[:].rearrange("p (j c) -> p c j", j=C),
                        axis=AX.X, op=OP.add,
                    )
                    bankO = psO.tile([128, 32], F32, tag="bankO")
                    for o in range(t_ov):
                        k = b * t_ov + o
                        nc.tensor.matmul(bankO[:], oh_cache[:, k * 128:(k + 1) * 128],
                                         ov[:, o * 64 + 32:(o + 1) * 64],
                                         start=(o == 0), stop=(o == t_ov - 1))
                    nc.vector.tensor_tensor(
                        out=heagg_bf[:, b * 32:(b + 1) * 32],
                        in0=t_he[:], in1=bankO[:], op=OP.add)
                    # outst[64:96] = h_e_agg; pad cols 224:256 need some value
                    nc.vector.tensor_copy(
                        outst[:, b * OUTP + 64:b * OUTP + 96],
                        heagg_bf[:, b * 32:(b + 1) * 32])
                    nc.vector.tensor_copy(
                        outst[:, b * OUTP + 224:b * OUTP + 256],
                        heagg_bf[:, b * 32:(b + 1) * 32])

            # Share h_e_agg via TWO pipelined compact AllGathers (b-major
            # rows, so the first fires as soon as blocks 0..SPL-1 are done,
            # overlapping the tail of Phase A), then locally re-pad each half
            # into a 256B-stride table for the int16 dma_gather.
            SPL = cfg.SPLB * 128
            nc.sync.dma_start(
                zloc[0:SPL, :].rearrange("(b p) c -> p b c", p=128),
                heagg_bf[:, 0:(SPL // 128) * 32].rearrange(
                    "p (b c) -> p b c", c=32),
            )
            nc.sync.dma_start(
                zloc[SPL:PADN, :].rearrange("(b p) c -> p b c", p=128),
                heagg_bf[:, (SPL // 128) * 32:].rearrange(
                    "p (b c) -> p b c", c=32),
            )
            if "collective" not in skip:
                nc.gpsimd.collective_compute(
                    "AllGather", OP.bypass,
                    ins=[zloc[0:SPL, :]], outs=[zag1.opt()],
                    replica_groups=[list(range(cfg.ncores))],
                )
                nc.gpsimd.collective_compute(
                    "AllGather", OP.bypass,
                    ins=[zloc[SPL:PADN, :]], outs=[zag2.opt()],
                    replica_groups=[list(range(cfg.ncores))],
                )
            NH = cfg.ncores // 2
            SPL2 = PADN - SPL
            for hi, ztab in ((0, zagA), (1, zagB)):
                nc.sync.dma_start(
                    ztab[:, 0:32].rearrange("(r w) c -> r w c", r=NH)[:, 0:SPL, :],
                    zag1[hi * NH * SPL:(hi + 1) * NH * SPL, :].rearrange(
                        "(r w) c -> r w c", r=NH),
                )
                nc.sync.dma_start(
                    ztab[:, 0:32].rearrange("(r w) c -> r w c", r=NH)[:, SPL:, :],
                    zag2[hi * NH * SPL2:(hi + 1) * NH * SPL2, :].rearrange(
                        "(r w) c -> r w c", r=NH),
                )

            # ------------- Phase C: node MLPs (overlaps the collective) -----
            if "phasec" not in skip:
                with (
                    tc.tile_pool(name="workC", bufs=4) as wc,
                    tc.tile_pool(name="ps1", bufs=2, space="PSUM") as ps1,
                    tc.tile_pool(name="ps3", bufs=2, space="PSUM") as ps3,
                    tc.tile_pool(name="psT", bufs=2, space="PSUM") as psT,
                ):
                    for b in range(NBLK):
                        xb = xt_sb[:, b * 128:(b + 1) * 128]
                        # h_ego = relu(x W_ego^T + b_ego)
                        p1 = ps1.tile([128, 64], F32, tag="p1")
                        nc.tensor.matmul(p1[:], ones_sb[:], bego_sb[:],
                                         start=True, stop=False)
                        nc.tensor.matmul(p1[:], xb, wego_sb[:],
                                         start=False, stop=True)
                        nc.scalar.activation(
                            outst[:, b * OUTP:b * OUTP + 64], p1[:], ACT.Relu)
                        # h_peer = relu(sqd * (W_px x + b_p + W_pe ((1/deg) S_ea)))
                        pt = psT.tile([32, 128], BF16, tag="pt")
                        nc.tensor.matmul(pt[:], ts_cache[:, b * 32:(b + 1) * 32],
                                         ident_sb[:], is_transpose=True,
                                         start=True, stop=True)
                        seat = wc.tile([32, 128], BF16, tag="seat")
                        nc.scalar.copy(seat[:], pt[:])
                        p3 = ps3.tile([128, 96], F32, tag="p3")
                        nc.tensor.matmul(p3[:], ones_sb[:], bpeer_sb[:],
                                         start=True, stop=False)
                        nc.tensor.matmul(p3[:], xb, wpx_sb[:],
                                         start=False, stop=False)
                        nc.tensor.matmul(p3[:], seat[:], wpe_sb[:],
                                         start=False, stop=True)
                        nc.scalar.activation(
                            outst[:, b * OUTP + 128:b * OUTP + 224], p3[:],
                            ACT.Relu, scale=sqd_sb[:, b:b + 1])

            # ------------- Phase B: gather + M + output writes --------------
            # dma_gather caps at 1024 indices (8 tiles); CH=8 main tiles fill
            # one instruction per (block, half), and the per-block overflow
            # tiles of a whole chunk merge into ONE instruction per half.
            # g layout per chunk: [half][main: block-major 8 tiles][ov:
            # block-major t_ov2 tiles].
            CH2, tov2, THALF = cfg.CH, cfg.t_ov2, cfg.THALF
            assert CH2 == 8
            with (
                tc.tile_pool(name="workB", bufs=4) as wb,
                tc.tile_pool(name="psM", bufs=3, space="PSUM") as psM,
            ):
                for qi, g0 in enumerate(range(0, NBLK, GB)):
                    ng = min(GB, NBLK - g0)
                    span = ng * THALF
                    g = wb.tile([128, 2 * span * 32], BF16, tag="g")
                    if "gather" not in skip:
                        for h, ztab in ((0, zagA), (1, zagB)):
                            qh = (qi * 2 + h) * GB * THALF * 8
                            for br in range(ng):
                                nc_ = raw_dma_gather(
                                    nc,
                                    g[:, (h * span + br * 8) * 32:
                                      (h * span + br * 8 + 8) * 32].rearrange(
                                        "p (t e) -> p t e", e=32),
                                    ztab[:, 0:32],
                                    gidx16_sb[:, qh + br * 64:qh + br * 64 + 64],
                                    num_idxs=1024, elem_size=32, elem_step=128)
                            nov = ng * tov2
                            raw_dma_gather(
                                nc,
                                g[:, (h * span + ng * 8) * 32:
                                  (h * span + ng * 8 + nov) * 32].rearrange(
                                    "p (t e) -> p t e", e=32),
                                ztab[:, 0:32],
                                gidx16_sb[:, qh + GB * 64:qh + GB * 64 + nov * 8],
                                num_idxs=nov * 128, elem_size=32, elem_step=128)
                    gv = g[:].rearrange("p (h z c) -> p c h z", h=2, c=32)
                    for br in range(ng):
                        b = g0 + br
                        m_sb = wb.tile([128, 32], F32, tag="msb")
                        nc.vector.tensor_reduce(
                            m_sb[:],
                            gv[:, :, :, br * 8:br * 8 + 8],
                            axis=AX.XY, op=OP.add,
                        )
                        pm = psM.tile([128, 32], F32, tag="pm")
                        oh2 = wb.tile([128, 2 * tov2 * 128], BF16, tag="oh2")
                        nmm = 2 * tov2
                        for h in range(2):
                            for o in range(tov2):
                                k = h * tov2 + o
                                nc.vector.tensor_scalar(
                                    out=oh2[:, k * 128:(k + 1) * 128], in0=iota_sb[:],
                                    scalar1=rlov2_sb[:, (b * 2 + h) * tov2 + o:
                                                     (b * 2 + h) * tov2 + o + 1],
                                    scalar2=None, op0=OP.is_equal,
                                )
                                nc.tensor.matmul(
                                    pm[:], oh2[:, k * 128:(k + 1) * 128],
                                    g[:, (h * span + ng * 8 + br * tov2 + o) * 32:
                                      (h * span + ng * 8 + br * tov2 + o + 1) * 32],
                                    start=(k == 0), stop=(k == nmm - 1))
                        t_m = wb.tile([128, 32], F32, tag="tm")
                        nc.vector.tensor_tensor(
                            out=t_m[:], in0=m_sb[:], in1=pm[:], op=OP.add)
                        nc.scalar.activation(
                            outst[:, b * OUTP + 96:b * OUTP + 128], t_m[:],
                            ACT.Copy, scale=dinv_sb[:, b:b + 1])
                    nc.sync.dma_start(
                        outp[g0 * 128:(g0 + ng) * 128, :].rearrange(
                            "(q p) c -> p q c", p=128),
                        outst[:, g0 * OUTP:(g0 + ng) * OUTP].rearrange(
                            "p (q c) -> p q c", c=OUTP),
                    )
    nc.compile()
    return nc


def host_prep(cfg, x, edge_attrs, edge_index):
    """Shard + lay out inputs for the slot-grid kernel. Pure index work + O(N)
    scalar prep (degree normalizers); all O(E*H)/O(N*H) FP math runs on device."""
    BF = np.float16
    N, E, C, NBLK, TPB, t_ov, NPC, PADN = (cfg.N, cfg.E, cfg.C, cfg.NBLK,
                                           cfg.TPB, cfg.t_ov, cfg.NPC, cfg.PADN)
    row = np.asarray(edge_index[1]).astype(np.int64)
    col = np.asarray(edge_index[0]).astype(np.int64)
    ea = np.asarray(edge_attrs, dtype=np.float32)
    xf = np.asarray(x, dtype=np.float32)

    deg = np.bincount(row, minlength=N)
    degf = np.maximum(deg, 1).astype(np.float64)
    dinv = np.where(deg > 0, degf ** -0.5, 0.0).astype(np.float32)
    sqd = np.sqrt(deg.astype(np.float64)).astype(np.float32)

    core = row // NPC
    lrow = row - core * NPC
    blk = lrow // 128
    part = lrow % 128

    # rank of each edge within its destination node
    order = np.argsort(row, kind="stable")
    sorted_row = row[order]
    starts = np.searchsorted(sorted_row, np.arange(N), side="left")
    rank = np.empty(E, np.int64)
    rank[order] = np.arange(E) - starts[sorted_row]

    is_grid = rank < C
    ove = np.where(~is_grid)[0]
    ovkey = core[ove] * NBLK + blk[ove]
    o_order = np.argsort(ovkey, kind="stable")
    ove = ove[o_order]
    okey_sorted = ovkey[o_order]
    ostarts = np.searchsorted(okey_sorted, np.arange(NBLK * cfg.ncores), side="left")
    opos = np.arange(ove.size) - ostarts[okey_sorted]
    otile = C + opos // 128
    opart = opos % 128
    if ove.size and otile.max() >= TPB:
        raise ValueError("overflow tiles exceeded; raise t_ov")

    tile_idx = np.empty(E, np.int64)
    tpart = np.empty(E, np.int64)
    ge = np.where(is_grid)[0]
    tile_idx[ge] = blk[ge] * TPB + rank[ge]
    tpart[ge] = part[ge]
    tile_idx[ove] = blk[ove] * TPB + otile
    tpart[ove] = opart

    # ---- Phase-B dual gather grid: edges split by source half ----------
    # half h = src_core // 4; within-half row (p-major per core section):
    #   (src_core % 4) * PADN + (l % 128) * NBLK + l // 128,  l = col % NPC
    CH, t_ov2, THALF, IW = cfg.CH, cfg.t_ov2, cfg.THALF, cfg.IW
    src_core = col // NPC
    lcol = col % NPC
    halfs = src_core // (cfg.ncores // 2)
    zrow16 = (src_core % (cfg.ncores // 2)) * PADN + lcol  # b-major rows
    ZPADH = 127 * NBLK + (NBLK - 1)   # half-core-0 pad node -> zeros

    # rank of each edge within (dest node, half)
    key2 = row * 2 + halfs
    order2 = np.argsort(key2, kind="stable")
    sk2 = key2[order2]
    starts2 = np.searchsorted(sk2, np.arange(2 * N), side="left")
    rank2 = np.empty(E, np.int64)
    rank2[order2] = np.arange(E) - starts2[sk2]

    is_grid2 = rank2 < CH
    ove2 = np.where(~is_grid2)[0]
    ovkey2 = (core[ove2] * NBLK + blk[ove2]) * 2 + halfs[ove2]
    o_order2 = np.argsort(ovkey2, kind="stable")
    ove2 = ove2[o_order2]
    ok2_sorted = ovkey2[o_order2]
    ostarts2 = np.searchsorted(ok2_sorted, np.arange(NBLK * cfg.ncores * 2),
                               side="left")
    opos2 = np.arange(ove2.size) - ostarts2[ok2_sorted]
    otile2 = CH + opos2 // 128
    opart2 = opos2 % 128
    if ove2.size and otile2.max() >= THALF:
        raise ValueError("phase-B overflow tiles exceeded; raise t_ov2")

    tile2 = np.empty(E, np.int64)
    tpart2 = np.empty(E, np.int64)
    ge2 = np.where(is_grid2)[0]
    tile2[ge2] = rank2[ge2]
    tpart2[ge2] = part[ge2]
    tile2[ove2] = otile2
    tpart2[ove2] = opart2

    TOTC = NBLK * TPB
    in_maps = []
    for r in range(cfg.ncores):
        sel = core == r
        e_idx = np.where(sel)[0]
        t_i = tile_idx[e_idx]
        t_p = tpart[e_idx]
        colpos = t_i * 128 + t_p

        EAT = np.zeros((33, TOTC * 128), np.float32)
        EAT[:32, colpos] = ea[e_idx].T
        EAT[32, colpos] = 1.0

        # int16 gather indices, per (chunk, half): [main: block-major 8 tiles
        # | ov: block-major t_ov2 tiles]; each instruction's list is stored
        # 16-partition-wrapped [16g + i%16, i//16], replicated x8.
        GB2, NQ = cfg.GB, (NBLK + cfg.GB - 1) // cfg.GB
        eb = blk[e_idx]
        eq, ebr = eb // GB2, eb % GB2
        eh, et, ep = halfs[e_idx], tile2[e_idx], tpart2[e_idx]
        secsz = GB2 * THALF * 128            # idx slots per (chunk, half)
        main_sel = et < CH
        ipos = np.where(
            main_sel,
            (eq * 2 + eh) * secsz + (ebr * 8 + et) * 128 + ep,
            (eq * 2 + eh) * secsz + GB2 * 8 * 128
            + (ebr * t_ov2 + (et - CH)) * 128 + ep)
        flat = np.full(NQ * 2 * secsz, ZPADH, np.int32)
        flat[ipos] = zrow16[e_idx]
        wrap = (flat.reshape(NQ * 2 * GB2 * THALF, 8, 16).transpose(2, 0, 1)
                .reshape(16, NQ * 2 * GB2 * THALF * 8).astype(np.int16))
        GIDX16 = np.tile(wrap, (8, 1))

        RLOV = np.full((128, NBLK * t_ov), 200.0, np.float32)
        ov_r = ove[core[ove] == r]
        op_r = opart[core[ove] == r]
        ot_r = otile[core[ove] == r]
        ob_r = blk[ov_r]
        RLOV[op_r, ob_r * t_ov + (ot_r - C)] = part[ov_r].astype(np.float32)

        RLOV2 = np.full((128, NBLK * 2 * t_ov2), 200.0, np.float32)
        sel2 = core[ove2] == r
        ov2_r = ove2[sel2]
        RLOV2[opart2[sel2],
              (blk[ov2_r] * 2 + halfs[ov2_r]) * t_ov2
              + (otile2[sel2] - CH)] = part[ov2_r].astype(np.float32)

        lo, hi = r * NPC, (r + 1) * NPC
        XT = np.zeros((128, PADN), np.float32)
        XT[:, :NPC] = xf[lo:hi].T
        dl = np.zeros(PADN, np.float32)
        dl[:NPC] = dinv[lo:hi]
        sl = np.zeros(PADN, np.float32)
        sl[:NPC] = sqd[lo:hi]
        gl = np.zeros(PADN, np.float32)
        gl[:NPC] = dinv[lo:hi] ** 2          # 1/deg (0 for deg==0)
        DINV = dl.reshape(NBLK, 128).T.copy()
        SQD = sl.reshape(NBLK, 128).T.copy()
        DEGINV = gl.reshape(NBLK, 128).T.copy()

        m = {
            "eat": EAT.astype(BF), "gidx16": GIDX16, "rlov": RLOV,
            "rlov2": RLOV2, "xt": XT.astype(BF), "dinv": DINV, "sqd": SQD,
            "deginv": DEGINV,
        }
        in_maps.append(m)
    return in_maps


def make_consts(cfg, W_peer, b_peer, W_ego, b_ego, W_edge, b_edge):
    BF = np.float16
    RCONST = np.zeros((33, 64), np.float32)
    RCONST[:32, :32] = np.eye(32, dtype=np.float32)
    RCONST[:32, 32:64] = np.asarray(W_edge, np.float32).T
    RCONST[32, 32:64] = np.asarray(b_edge, np.float32)
    consts = {
        "rconst": RCONST.astype(BF),
        "wegot": np.ascontiguousarray(np.asarray(W_ego, np.float32).T).astype(BF),
        "wpxt": np.ascontiguousarray(np.asarray(W_peer, np.float32)[:, :128].T).astype(BF),
        "wpet": np.ascontiguousarray(np.asarray(W_peer, np.float32)[:, 128:].T).astype(BF),
        "bego": np.asarray(b_ego, np.float32).reshape(1, 64).astype(BF),
        "bpeer": np.asarray(b_peer, np.float32).reshape(1, 96).astype(BF),
        "iota": np.broadcast_to(np.arange(128, dtype=np.float32), (128, 128)).astype(BF).copy(),
        "ident": np.eye(128, dtype=np.float32).astype(BF),
    }
    return consts


_CACHE = {}
RUN_KWARGS = {}


def kernel(x, edge_attrs, W_peer, b_peer, W_ego, b_ego, W_edge, b_edge, edge_index):
    x = np.asarray(x)
    edge_attrs = np.asarray(edge_attrs)
    edge_index = np.asarray(edge_index)
    N, E = x.shape[0], edge_attrs.shape[0]

    row = edge_index[1].astype(np.int64)
    col = edge_index[0].astype(np.int64)
    C = 15
    CH = 8
    ncores = 8
    NPC = N // ncores
    NBLK = (NPC + 127) // 128
    deg = np.bincount(row, minlength=N)
    ovn = np.maximum(deg - C, 0)
    nodes = np.arange(N)
    bkey = (nodes // NPC) * NBLK + (nodes % NPC) // 128
    ovblk = np.bincount(bkey, weights=ovn.astype(np.float64), minlength=NBLK * ncores)
    t_ov = max(3, int(np.ceil(ovblk.max() / 128.0)))

    # phase-B dual-grid overflow sizing (per dest block x source half)
    halfs = (col // NPC) // (ncores // 2)
    degh = np.bincount(row * 2 + halfs, minlength=2 * N)
    ovn2 = np.maximum(degh - CH, 0)
    bkey2 = np.repeat(bkey, 2) * 2 + np.tile([0, 1], N)
    ovblk2 = np.bincount(bkey2, weights=ovn2.astype(np.float64),
                         minlength=2 * NBLK * ncores)
    t_ov2 = max(2, int(np.ceil(ovblk2.max() / 128.0)))

    cfg = Cfg(N=N, E=E, ncores=ncores, C=C, t_ov=t_ov, CH=CH, t_ov2=t_ov2)
    key = cfg.key()
    if key not in _CACHE:
        _CACHE[key] = build_program(cfg)
    nc = _CACHE[key]

    in_maps = host_prep(cfg, x, edge_attrs, edge_index)
    consts = make_consts(cfg, W_peer, b_peer, W_ego, b_ego, W_edge, b_edge)
    for m in in_maps:
        m.update(consts)

    res = run_bass_kernel_spmd(nc, in_maps, core_ids=list(range(cfg.ncores)),
                               **RUN_KWARGS)
    out = np.empty((N, cfg.OUTD), np.float32)
    for r in range(cfg.ncores):
        blk = np.asarray(res.results[r]["out"]).astype(np.float32)
        out[r * cfg.NPC:(r + 1) * cfg.NPC] = blk[:cfg.NPC, :cfg.OUTD]
    kernel.last_result = res
    return out


# revision 47
# speedup vs baseline: 3.6009x; 1.0065x over previous
"""Trainium2 Bass kernel: CANE FeatureEmbedding GNN message passing.

Strategy (node-range sharding, 8 cores), v2:
  - Nodes range-partitioned; edges assigned to the core owning their
    DESTINATION (row = edge_index[1]).
  - Algebraic collapse of the peer branch (gather and scatter both use `row`):
        h_peer[i] = relu( sqrt(deg_i)*(W_px x_i + b_p) + deg_i^-1/2 * (W_pe S_ea[i]) )
    removing the [E,160]x[160,96] per-edge MLP.
  - Remaining per-edge work: h_e = relu(ea @ W_edge.T + b_edge), segment sums
    of ea and h_e over row, and M[i] = sum_{e: row=i} h_e_agg[col[e]].
  - Slot grid: node v gets C slots; edge k of v goes to (block v//128,
    partition v%128, slot k). One bf16 matmul per slot tile accumulates S_ea
    in PSUM; h_e pre-acts land per-slot, relu on the Act engine, slot-sum on
    DVE straight into PSUM, overflow (deg>C) via one-hot scatter matmuls.
  - h_e_agg shared across cores via a compact fp16 AllGather, then locally
    re-padded into two 256B-stride half-tables (<=32767 rows each, int16
    indexable). M is built with dma_gather (64B payload / 256B stride, <=1024
    indices per instruction -- the SWDGE ring cap) over a second slot grid
    split by source half, reduced along (half, slot) on DVE into PSUM with
    one-hot matmul accumulation for overflow.
  - Everything streams in fp16 (4x finer mantissa than bf16 at identical
    cost); PSUM accumulation in fp32; output written as [PADN, 256] fp16 rows
    (cols 224:256 pad) and upcast host-side.
"""

import numpy as np

import concourse.bass as bass
import concourse.mybir as mybir
import concourse.tile as tile
from concourse import bacc
from concourse._compat import get_trn_type
from concourse.bass import IndirectOffsetOnAxis
from concourse.bass_utils import run_bass_kernel_spmd

F32 = mybir.dt.float32
BF16 = mybir.dt.float16  # fp16: 4x finer mantissa than bf16, same perf
I32 = mybir.dt.int32
I16 = mybir.dt.int16
AX = mybir.AxisListType
OP = mybir.AluOpType
ACT = mybir.ActivationFunctionType


class Cfg:
    def __init__(self, N=50000, E=800000, ncores=8, C=15, t_ov=3, GB=4, EB=7,
                 CH=8, t_ov2=2):
        self.N = N
        self.E = E
        self.ncores = ncores
        self.NPC = N // ncores            # nodes per core
        self.NBLK = (self.NPC + 127) // 128
        self.PADN = self.NBLK * 128       # padded nodes per core
        self.C = C                        # grid slots per node
        self.t_ov = t_ov                  # overflow tiles per block
        self.TPB = C + t_ov               # tiles per block
        self.GB = GB                      # blocks per gather instruction
        self.EB = EB                      # blocks per eat-load DMA
        self.CH = CH                      # phase-B main slots per (node, half)
        self.t_ov2 = t_ov2                # phase-B overflow tiles per (block, half)
        self.THALF = CH + t_ov2           # phase-B tiles per (block, half)
        self.IW = (self.THALF * 128 + 15) // 16   # int16 idx cols per gather
        self.SPLB = 36                    # blocks in the first AllGather
        self.ND = 128
        self.ED = 32
        self.OUTD = 224
        self.OUTP = 256                   # padded out row

    def key(self):
        return (self.N, self.E, self.ncores, self.C, self.t_ov, self.GB,
                self.EB, self.CH, self.t_ov2)


def raw_dma_gather(nc, out_ap, in_ap, idxs_ap, num_idxs, elem_size, elem_step):
    """nc.gpsimd.dma_gather minus the %256 payload assert (the 256B rule
    applies to the row STRIDE, which elem_step satisfies; the ucode packetizes
    the payload at any size)."""
    gp = nc.gpsimd
    stride_bytes = elem_step * mybir.dt.size(in_ap.dtype)
    assert stride_bytes % 256 == 0 and stride_bytes // 256 < 256
    _in_ap = gp.lower_ap_dma(in_ap, for_custom_bir_dma=True)
    _idxs_ap = gp.lower_ap(idxs_ap)
    _out_ap = gp.lower_ap(out_ap)
    return gp.add_instruction(
        mybir.InstDMAGatherAnt(
            name=f"I-{nc.next_id()}",
            ins=[*_in_ap, _idxs_ap, gp.lower_val_access(gp.to_reg(num_idxs))],
            outs=[_out_ap],
            transpose=False,
            num_idxs=num_idxs,
            elem_size=elem_size,
            stride_bytes_256=stride_bytes // 256,
            gen_mode=0,
            single_packet=True,
            queue_num=0,
            sbuf_tokens_per_rank=0,
            sbuf_free_dim_per_rank=0,
            sbuf_free_dim_pad_per_rank=0,
            sbuf_byte_offset=0,
        )
    )


def build_program(cfg, skip=()):
    """Build the SPMD Bass program (same NEFF for all cores).

    skip: {"gather", "slotmm", "overflow", "phasec", "collective"} — timing
    experiment knobs only (results are wrong when used)."""
    skip = set(skip)
    nc = bacc.Bacc(get_trn_type() or "TRN2", target_bir_lowering=False, debug=True)
    NBLK, TPB, C, t_ov, PADN = cfg.NBLK, cfg.TPB, cfg.C, cfg.t_ov, cfg.PADN
    GB, EB, OUTP = cfg.GB, cfg.EB, cfg.OUTP
    TOT = NBLK * TPB

    eat = nc.declare_dram_parameter("eat", [33, TOT * 128], BF16, isOutput=False)
    NQ = (NBLK + GB - 1) // GB
    gidx16 = nc.declare_dram_parameter(
        "gidx16", [128, NQ * 2 * GB * cfg.THALF * 8], I16, isOutput=False)
    rlov2 = nc.declare_dram_parameter("rlov2", [128, NBLK * 2 * cfg.t_ov2], F32,
                                      isOutput=False)
    rlov = nc.declare_dram_parameter("rlov", [128, NBLK * t_ov], F32, isOutput=False)
    xt = nc.declare_dram_parameter("xt", [128, PADN], BF16, isOutput=False)
    dinvp = nc.declare_dram_parameter("dinv", [128, NBLK], F32, isOutput=False)
    sqdp = nc.declare_dram_parameter("sqd", [128, NBLK], F32, isOutput=False)
    deginvp = nc.declare_dram_parameter("deginv", [128, NBLK], F32, isOutput=False)
    rconst = nc.declare_dram_parameter("rconst", [33, 64], BF16, isOutput=False)
    wegot = nc.declare_dram_parameter("wegot", [128, 64], BF16, isOutput=False)
    wpxt = nc.declare_dram_parameter("wpxt", [128, 96], BF16, isOutput=False)
    wpet = nc.declare_dram_parameter("wpet", [32, 96], BF16, isOutput=False)
    bego = nc.declare_dram_parameter("bego", [1, 64], BF16, isOutput=False)
    bpeer = nc.declare_dram_parameter("bpeer", [1, 96], BF16, isOutput=False)
    iota = nc.declare_dram_parameter("iota", [128, 128], BF16, isOutput=False)
    ident = nc.declare_dram_parameter("ident", [128, 128], BF16, isOutput=False)
    outp = nc.declare_dram_parameter("out", [PADN, OUTP], BF16, isOutput=True)

    with tile.TileContext(nc) as tc:
        with (
            tc.tile_pool(name="const", bufs=1) as cp,
            tc.tile_pool(name="resident", bufs=1) as rp,
            tc.tile_pool(name="dram", bufs=1, space="DRAM") as dp,
        ):
            rc = cp.tile([33, 64], BF16)
            nc.sync.dma_start(rc[:], rconst[:])
            wego_sb = cp.tile([128, 64], BF16)
            nc.sync.dma_start(wego_sb[:], wegot[:])
            wpx_sb = cp.tile([128, 96], BF16)
            nc.sync.dma_start(wpx_sb[:], wpxt[:])
            wpe_sb = cp.tile([32, 96], BF16)
            nc.sync.dma_start(wpe_sb[:], wpet[:])
            bego_sb = cp.tile([1, 64], BF16)
            nc.sync.dma_start(bego_sb[:], bego[:])
            bpeer_sb = cp.tile([1, 96], BF16)
            nc.sync.dma_start(bpeer_sb[:], bpeer[:])
            iota_sb = cp.tile([128, 128], BF16)
            nc.sync.dma_start(iota_sb[:], iota[:])
            ident_sb = cp.tile([128, 128], BF16)
            nc.sync.dma_start(ident_sb[:], ident[:])
            ones_sb = cp.tile([1, 128], BF16)
            nc.gpsimd.memset(ones_sb[:], 1.0)

            xt_sb = rp.tile([128, PADN], BF16)
            nc.sync.dma_start(xt_sb[:], xt[:])
            gidx16_sb = rp.tile([128, NQ * 2 * GB * cfg.THALF * 8], I16)
            nc.sync.dma_start(gidx16_sb[:], gidx16[:])
            rlov2_sb = rp.tile([128, NBLK * 2 * cfg.t_ov2], F32)
            nc.sync.dma_start(rlov2_sb[:], rlov2[:])
            rlov_sb = rp.tile([128, NBLK * t_ov], F32)
            nc.sync.dma_start(rlov_sb[:], rlov[:])
            dinv_sb = rp.tile([128, NBLK], F32)
            nc.sync.dma_start(dinv_sb[:], dinvp[:])
            sqd_sb = rp.tile([128, NBLK], F32)
            nc.sync.dma_start(sqd_sb[:], sqdp[:])
            deginv_sb = rp.tile([128, NBLK], F32)
            nc.sync.dma_start(deginv_sb[:], deginvp[:])

            heagg_bf = rp.tile([128, NBLK * 32], BF16)   # h_e_agg, bf16
            ts_cache = rp.tile([128, NBLK * 32], BF16)   # dinv * S_ea, bf16
            oh_cache = rp.tile([128, NBLK * t_ov * 128], BF16)  # one-hots
            outst = rp.tile([128, NBLK * OUTP], BF16)    # output staging

            # zag holds all cores' h_e_agg (p-major rows within each core
            # section); zagA/zagB are 256B-stride padded halves for dma_gather
            zloc = dp.tile([PADN, 32], BF16)
            SPLr = cfg.SPLB * 128
            zag1 = dp.tile([cfg.ncores * SPLr, 32], BF16, addr_space="Shared")
            zag2 = dp.tile([cfg.ncores * (PADN - SPLr), 32], BF16,
                           addr_space="Shared")
            zagA = dp.tile([cfg.ncores // 2 * PADN, 128], BF16)
            zagB = dp.tile([cfg.ncores // 2 * PADN, 128], BF16)

            # ---------------- Phase A: per-edge MLP + segment sums ----------
            with (
                tc.tile_pool(name="eatp", bufs=3) as eatp,
                tc.tile_pool(name="workA", bufs=4) as wp,
                tc.tile_pool(name="psA", bufs=2, space="PSUM") as psA,
                tc.tile_pool(name="psB", bufs=2, space="PSUM") as psB,
                tc.tile_pool(name="psP", bufs=2, space="PSUM") as psP,
                tc.tile_pool(name="psO", bufs=2, space="PSUM") as psO,
            ):
                ech = None
                for b in range(NBLK):
                    if b % EB == 0:
                        nch = min(EB, NBLK - b)
                        ech = eatp.tile([33, nch * TPB * 128], BF16, tag="ech")
                        nc.sync.dma_start(
                            ech[:], eat[:, b * TPB * 128:(b + nch) * TPB * 128])
                    e0 = (b % EB) * TPB * 128
                    bankA = psA.tile([128, 32], F32, tag="bankA")
                    bankB = psB.tile([128, C * 32], F32, tag="bankB")
                    for j in range(C):
                        if "slotmm" in skip:
                            break
                        lh = ech[:, e0 + j * 128:e0 + (j + 1) * 128]
                        nc.tensor.matmul(bankA[:], lh, rc[:, 0:32],
                                         start=(j == 0), stop=False)
                        nc.tensor.matmul(bankB[:, j * 32:(j + 1) * 32], lh,
                                         rc[:, 32:64],
                                         start=(j == 0), stop=(j == C - 1))
                    # overflow: pre-act for t_ov tiles in one PSUM tile
                    pc = psP.tile([128, t_ov * 64], F32, tag="pc")
                    ov = wp.tile([128, t_ov * 64], BF16, tag="ov")
                    for o in range(t_ov):
                        lh = ech[:, e0 + (C + o) * 128:e0 + (C + o + 1) * 128]
                        nc.tensor.matmul(pc[:, o * 64:(o + 1) * 64], lh, rc[:, 0:64],
                                         start=(o == 0), stop=(o == t_ov - 1))
                    # ea part: copy (Act); h_e part: relu (DVE)
                    nc.scalar.copy(
                        ov[:].rearrange("p (t x) -> p t x", x=64)[:, :, 0:32],
                        pc[:].rearrange("p (t x) -> p t x", x=64)[:, :, 0:32])
                    nc.vector.tensor_scalar_max(
                        ov[:].rearrange("p (t x) -> p t x", x=64)[:, :, 32:64],
                        pc[:].rearrange("p (t x) -> p t x", x=64)[:, :, 32:64],
                        0.0)
                    # one-hot scatter matrices (cached for Phase B reuse)
                    for o in range(t_ov):
                        k = b * t_ov + o
                        nc.gpsimd.tensor_scalar(
                            out=oh_cache[:, k * 128:(k + 1) * 128], in0=iota_sb[:],
                            scalar1=rlov_sb[:, k:k + 1], scalar2=None,
                            op0=OP.is_equal,
                        )
                        # fold overflow ea into bankA accumulation group
                        nc.tensor.matmul(bankA[:], oh_cache[:, k * 128:(k + 1) * 128],
                                         ov[:, o * 64:o * 64 + 32],
                                         start=False, stop=(o == t_ov - 1))
                    # t_s = (1/deg) * S_ea (kept bf16 for Phase C transpose;
                    # the final sqd scale rides the peer relu)
                    nc.vector.tensor_scalar_mul(
                        ts_cache[:, b * 32:(b + 1) * 32], bankA[:],
                        deginv_sb[:, b:b + 1])
                    # h_e slots: relu on Act, slot-sum on DVE into PSUM
                    relu_st = wp.tile([128, C * 32], BF16, tag="relu")
                    nc.scalar.activation(relu_st[:], bankB[:], ACT.Relu)
                    t_he = wp.tile([128, 32], F32, tag="the")
                    nc.vector.tensor_reduce(
                        t_he[:],
                        relu_st[:].rearrange("p (j c) -> p c j", j=C),
                        axis=AX.X, op=OP.add,
                    )
                    bankO = psO.tile([128, 32], F32, tag="bankO")
                    for o in range(t_ov):
                        k = b * t_ov + o
                        nc.tensor.matmul(bankO[:], oh_cache[:, k * 128:(k + 1) * 128],
                                         ov[:, o * 64 + 32:(o + 1) * 64],
                                         start=(o == 0), stop=(o == t_ov - 1))
                    nc.vector.tensor_tensor(
                        out=heagg_bf[:, b * 32:(b + 1) * 32],
                        in0=t_he[:], in1=bankO[:], op=OP.add)
                    # outst[64:96] = h_e_agg; pad cols 224:256 need some value
                    nc.vector.tensor_copy(
                        outst[:, b * OUTP + 64:b * OUTP + 96],
                        heagg_bf[:, b * 32:(b + 1) * 32])
                    nc.vector.tensor_copy(
                        outst[:, b * OUTP + 224:b * OUTP + 256],
                        heagg_bf[:, b * 32:(b + 1) * 32])

            # Share h_e_agg via TWO pipelined compact AllGathers (b-major
            # rows, so the first fires as soon as blocks 0..SPL-1 are done,
            # overlapping the tail of Phase A), then locally re-pad each half
            # into a 256B-stride table for the int16 dma_gather.
            SPL = cfg.SPLB * 128
            nc.sync.dma_start(
                zloc[0:SPL, :].rearrange("(b p) c -> p b c", p=128),
                heagg_bf[:, 0:(SPL // 128) * 32].rearrange(
                    "p (b c) -> p b c", c=32),
            )
            nc.sync.dma_start(
                zloc[SPL:PADN, :].rearrange("(b p) c -> p b c", p=128),
                heagg_bf[:, (SPL // 128) * 32:].rearrange(
                    "p (b c) -> p b c", c=32),
            )
            if "collective" not in skip:
                nc.gpsimd.collective_compute(
                    "AllGather", OP.bypass,
                    ins=[zloc[0:SPL, :]], outs=[zag1.opt()],
                    replica_groups=[list(range(cfg.ncores))],
                )
                nc.gpsimd.collective_compute(
                    "AllGather", OP.bypass,
                    ins=[zloc[SPL:PADN, :]], outs=[zag2.opt()],
                    replica_groups=[list(range(cfg.ncores))],
                )
            NH = cfg.ncores // 2
            SPL2 = PADN - SPL
            for hi, ztab in ((0, zagA), (1, zagB)):
                nc.sync.dma_start(
                    ztab[:, 0:32].rearrange("(r w) c -> r w c", r=NH)[:, 0:SPL, :],
                    zag1[hi * NH * SPL:(hi + 1) * NH * SPL, :].rearrange(
                        "(r w) c -> r w c", r=NH),
                )
                nc.sync.dma_start(
                    ztab[:, 0:32].rearrange("(r w) c -> r w c", r=NH)[:, SPL:, :],
                    zag2[hi * NH * SPL2:(hi + 1) * NH * SPL2, :].rearrange(
                        "(r w) c -> r w c", r=NH),
                )

            # ------------- Phase C: node MLPs (overlaps the collective) -----
            if "phasec" not in skip:
                with (
                    tc.tile_pool(name="workC", bufs=4) as wc,
                    tc.tile_pool(name="ps1", bufs=2, space="PSUM") as ps1,
                    tc.tile_pool(name="ps3", bufs=2, space="PSUM") as ps3,
                    tc.tile_pool(name="psT", bufs=2, space="PSUM") as psT,
                ):
                    for b in range(NBLK):
                        xb = xt_sb[:, b * 128:(b + 1) * 128]
                        # h_ego = relu(x W_ego^T + b_ego)
                        p1 = ps1.tile([128, 64], F32, tag="p1")
                        nc.tensor.matmul(p1[:], ones_sb[:], bego_sb[:],
                                         start=True, stop=False)
                        nc.tensor.matmul(p1[:], xb, wego_sb[:],
                                         start=False, stop=True)
                        nc.scalar.activation(
                            outst[:, b * OUTP:b * OUTP + 64], p1[:], ACT.Relu)
                        # h_peer = relu(sqd * (W_px x + b_p + W_pe ((1/deg) S_ea)))
                        pt = psT.tile([32, 128], BF16, tag="pt")
                        nc.tensor.matmul(pt[:], ts_cache[:, b * 32:(b + 1) * 32],
                                         ident_sb[:], is_transpose=True,
                                         start=True, stop=True)
                        seat = wc.tile([32, 128], BF16, tag="seat")
                        nc.scalar.copy(seat[:], pt[:])
                        p3 = ps3.tile([128, 96], F32, tag="p3")
                        nc.tensor.matmul(p3[:], ones_sb[:], bpeer_sb[:],
                                         start=True, stop=False)
                        nc.tensor.matmul(p3[:], xb, wpx_sb[:],
                                         start=False, stop=False)
                        nc.tensor.matmul(p3[:], seat[:], wpe_sb[:],
                                         start=False, stop=True)
                        nc.scalar.activation(
                            outst[:, b * OUTP + 128:b * OUTP + 224], p3[:],
                            ACT.Relu, scale=sqd_sb[:, b:b + 1])

            # ------------- Phase B: gather + M + output writes --------------
            # dma_gather caps at 1024 indices (8 tiles); CH=8 main tiles fill
            # one instruction per (block, half), and the per-block overflow
            # tiles of a whole chunk merge into ONE instruction per half.
            # g layout per chunk: [half][main: block-major 8 tiles][ov:
            # block-major t_ov2 tiles].
            CH2, tov2, THALF = cfg.CH, cfg.t_ov2, cfg.THALF
            assert CH2 == 8
            with (
                tc.tile_pool(name="workB", bufs=4) as wb,
                tc.tile_pool(name="psM", bufs=3, space="PSUM") as psM,
            ):
                for qi, g0 in enumerate(range(0, NBLK, GB)):
                    ng = min(GB, NBLK - g0)
                    span = ng * THALF
                    g = wb.tile([128, 2 * span * 32], BF16, tag="g")
                    if "gather" not in skip:
                        for h, ztab in ((0, zagA), (1, zagB)):
                            qh = (qi * 2 + h) * GB * THALF * 8
                            for br in range(ng):
                                nc_ = raw_dma_gather(
                                    nc,
                                    g[:, (h * span + br * 8) * 32:
                                      (h * span + br * 8 + 8) * 32].rearrange(
                                        "p (t e) -> p t e", e=32),
                                    ztab[:, 0:32],
                                    gidx16_sb[:, qh + br * 64:qh + br * 64 + 64],
                                    num_idxs=1024, elem_size=32, elem_step=128)
                            nov = ng * tov2
                            raw_dma_gather(
                                nc,
                                g[:, (h * span + ng * 8) * 32:
                                  (h * span + ng * 8 + nov) * 32].rearrange(
                                    "p (t e) -> p t e", e=32),
                                ztab[:, 0:32],
                                gidx16_sb[:, qh + GB * 64:qh + GB * 64 + nov * 8],
                                num_idxs=nov * 128, elem_size=32, elem_step=128)
                    gv = g[:].rearrange("p (h z c) -> p c h z", h=2, c=32)
                    for br in range(ng):
                        b = g0 + br
                        m_sb = wb.tile([128, 32], F32, tag="msb")
                        nc.vector.tensor_reduce(
                            m_sb[:],
                            gv[:, :, :, br * 8:br * 8 + 8],
                            axis=AX.XY, op=OP.add,
                        )
                        pm = psM.tile([128, 32], F32, tag="pm")
                        oh2 = wb.tile([128, 2 * tov2 * 128], BF16, tag="oh2")
                        nmm = 2 * tov2
                        for h in range(2):
                            for o in range(tov2):
                                k = h * tov2 + o
                                nc.vector.tensor_scalar(
                                    out=oh2[:, k * 128:(k + 1) * 128], in0=iota_sb[:],
                                    scalar1=rlov2_sb[:, (b * 2 + h) * tov2 + o:
                                                     (b * 2 + h) * tov2 + o + 1],
                                    scalar2=None, op0=OP.is_equal,
                                )
                                nc.tensor.matmul(
                                    pm[:], oh2[:, k * 128:(k + 1) * 128],
                                    g[:, (h * span + ng * 8 + br * tov2 + o) * 32:
                                      (h * span + ng * 8 + br * tov2 + o + 1) * 32],
                                    start=(k == 0), stop=(k == nmm - 1))
                        t_m = wb.tile([128, 32], F32, tag="tm")
                        nc.vector.tensor_tensor(
                            out=t_m[:], in0=m_sb[:], in1=pm[:], op=OP.add)
                        nc.scalar.activation(
                            outst[:, b * OUTP + 96:b * OUTP + 128], t_m[:],
                            ACT.Copy, scale=dinv_sb[:, b:b + 1])
                    nc.sync.dma_start(
                        outp[g0 * 128:(g0 + ng) * 128, :].rearrange(
                            "(q p) c -> p q c", p=128),
                        outst[:, g0 * OUTP:(g0 + ng) * OUTP].rearrange(
                            "p (q c) -> p q c", c=OUTP),
                    )
    nc.compile()
    return nc


def host_prep(cfg, x, edge_attrs, edge_index):
    """Shard + lay out inputs for the slot-grid kernel. Pure index work + O(N)
    scalar prep (degree normalizers); all O(E*H)/O(N*H) FP math runs on device."""
    BF = np.float16
    N, E, C, NBLK, TPB, t_ov, NPC, PADN = (cfg.N, cfg.E, cfg.C, cfg.NBLK,
                                           cfg.TPB, cfg.t_ov, cfg.NPC, cfg.PADN)
    row = np.asarray(edge_index[1]).astype(np.int64)
    col = np.asarray(edge_index[0]).astype(np.int64)
    ea = np.asarray(edge_attrs, dtype=np.float32)
    xf = np.asarray(x, dtype=np.float32)

    deg = np.bincount(row, minlength=N)
    degf = np.maximum(deg, 1).astype(np.float64)
    dinv = np.where(deg > 0, degf ** -0.5, 0.0).astype(np.float32)
    sqd = np.sqrt(deg.astype(np.float64)).astype(np.float32)

    core = row // NPC
    lrow = row - core * NPC
    blk = lrow // 128
    part = lrow % 128

    # rank of each edge within its destination node
    order = np.argsort(row, kind="stable")
    sorted_row = row[order]
    starts = np.searchsorted(sorted_row, np.arange(N), side="left")
    rank = np.empty(E, np.int64)
    rank[order] = np.arange(E) - starts[sorted_row]

    is_grid = rank < C
    ove = np.where(~is_grid)[0]
    ovkey = core[ove] * NBLK + blk[ove]
    o_order = np.argsort(ovkey, kind="stable")
    ove = ove[o_order]
    okey_sorted = ovkey[o_order]
    ostarts = np.searchsorted(okey_sorted, np.arange(NBLK * cfg.ncores), side="left")
    opos = np.arange(ove.size) - ostarts[okey_sorted]
    otile = C + opos // 128
    opart = opos % 128
    if ove.size and otile.max() >= TPB:
        raise ValueError("overflow tiles exceeded; raise t_ov")

    tile_idx = np.empty(E, np.int64)
    tpart = np.empty(E, np.int64)
    ge = np.where(is_grid)[0]
    tile_idx[ge] = blk[ge] * TPB + rank[ge]
    tpart[ge] = part[ge]
    tile_idx[ove] = blk[ove] * TPB + otile
    tpart[ove] = opart

    # ---- Phase-B dual gather grid: edges split by source half ----------
    # half h = src_core // 4; within-half row (p-major per core section):
    #   (src_core % 4) * PADN + (l % 128) * NBLK + l // 128,  l = col % NPC
    CH, t_ov2, THALF, IW = cfg.CH, cfg.t_ov2, cfg.THALF, cfg.IW
    src_core = col // NPC
    lcol = col % NPC
    halfs = src_core // (cfg.ncores // 2)
    zrow16 = (src_core % (cfg.ncores // 2)) * PADN + lcol  # b-major rows
    ZPADH = 127 * NBLK + (NBLK - 1)   # half-core-0 pad node -> zeros

    # rank of each edge within (dest node, half)
    key2 = row * 2 + halfs
    order2 = np.argsort(key2, kind="stable")
    sk2 = key2[order2]
    starts2 = np.searchsorted(sk2, np.arange(2 * N), side="left")
    rank2 = np.empty(E, np.int64)
    rank2[order2] = np.arange(E) - starts2[sk2]

    is_grid2 = rank2 < CH
    ove2 = np.where(~is_grid2)[0]
    ovkey2 = (core[ove2] * NBLK + blk[ove2]) * 2 + halfs[ove2]
    o_order2 = np.argsort(ovkey2, kind="stable")
    ove2 = ove2[o_order2]
    ok2_sorted = ovkey2[o_order2]
    ostarts2 = np.searchsorted(ok2_sorted, np.arange(NBLK * cfg.ncores * 2),
                               side="left")
    opos2 = np.arange(ove2.size) - ostarts2[ok2_sorted]
    otile2 = CH + opos2 // 128
    opart2 = opos2 % 128
    if ove2.size and otile2.max() >= THALF:
        raise ValueError("phase-B overflow tiles exceeded; raise t_ov2")

    tile2 = np.empty(E, np.int64)
    tpart2 = np.empty(E, np.int64)
    ge2 = np.where(is_grid2)[0]
    tile2[ge2] = rank2[ge2]
    tpart2[ge2] = part[ge2]
    tile2[ove2] = otile2
    tpart2[ove2] = opart2

    TOTC = NBLK * TPB
    in_maps = []
    for r in range(cfg.ncores):
        sel = core == r
        e_idx = np.where(sel)[0]
        t_i = tile_idx[e_idx]
        t_p = tpart[e_idx]
        colpos = t_i * 128 + t_p

        EAT = np.zeros((33, TOTC * 128), np.float32)
        EAT[:32, colpos] = ea[e_idx].T
        EAT[32, colpos] = 1.0

        # int16 gather indices, per (chunk, half): [main: block-major 8 tiles
        # | ov: block-major t_ov2 tiles]; each instruction's list is stored
        # 16-partition-wrapped [16g + i%16, i//16], replicated x8.
        GB2, NQ = cfg.GB, (NBLK + cfg.GB - 1) // cfg.GB
        eb = blk[e_idx]
        eq, ebr = eb // GB2, eb % GB2
        eh, et, ep = halfs[e_idx], tile2[e_idx], tpart2[e_idx]
        secsz = GB2 * THALF * 128            # idx slots per (chunk, half)
        main_sel = et < CH
        ipos = np.where(
            main_sel,
            (eq * 2 + eh) * secsz + (ebr * 8 + et) * 128 + ep,
            (eq * 2 + eh) * secsz + GB2 * 8 * 128
            + (ebr * t_ov2 + (et - CH)) * 128 + ep)
        flat = np.full(NQ * 2 * secsz, ZPADH, np.int32)
        flat[ipos] = zrow16[e_idx]
        wrap = (flat.reshape(NQ * 2 * GB2 * THALF, 8, 16).transpose(2, 0, 1)
                .reshape(16, NQ * 2 * GB2 * THALF * 8).astype(np.int16))
        GIDX16 = np.tile(wrap, (8, 1))

        RLOV = np.full((128, NBLK * t_ov), 200.0, np.float32)
        ov_r = ove[core[ove] == r]
        op_r = opart[core[ove] == r]
        ot_r = otile[core[ove] == r]
        ob_r = blk[ov_r]
        RLOV[op_r, ob_r * t_ov + (ot_r - C)] = part[ov_r].astype(np.float32)

        RLOV2 = np.full((128, NBLK * 2 * t_ov2), 200.0, np.float32)
        sel2 = core[ove2] == r
        ov2_r = ove2[sel2]
        RLOV2[opart2[sel2],
              (blk[ov2_r] * 2 + halfs[ov2_r]) * t_ov2
              + (otile2[sel2] - CH)] = part[ov2_r].astype(np.float32)

        lo, hi = r * NPC, (r + 1) * NPC
        XT = np.zeros((128, PADN), np.float32)
        XT[:, :NPC] = xf[lo:hi].T
        dl = np.zeros(PADN, np.float32)
        dl[:NPC] = dinv[lo:hi]
        sl = np.zeros(PADN, np.float32)
        sl[:NPC] = sqd[lo:hi]
        gl = np.zeros(PADN, np.float32)
        gl[:NPC] = dinv[lo:hi] ** 2          # 1/deg (0 for deg==0)
        DINV = dl.reshape(NBLK, 128).T.copy()
        SQD = sl.reshape(NBLK, 128).T.copy()
        DEGINV = gl.reshape(NBLK, 128).T.copy()

        m = {
            "eat": EAT.astype(BF), "gidx16": GIDX16, "rlov": RLOV,
            "rlov2": RLOV2, "xt": XT.astype(BF), "dinv": DINV, "sqd": SQD,
            "deginv": DEGINV,
        }
        in_maps.append(m)
    return in_maps


def make_consts(cfg, W_peer, b_peer, W_ego, b_ego, W_edge, b_edge):
    BF = np.float16
    RCONST = np.zeros((33, 64), np.float32)
    RCONST[:32, :32] = np.eye(32, dtype=np.float32)
    RCONST[:32, 32:64] = np.asarray(W_edge, np.float32).T
    RCONST[32, 32:64] = np.asarray(b_edge, np.float32)
    consts = {
        "rconst": RCONST.astype(BF),
        "wegot": np.ascontiguousarray(np.asarray(W_ego, np.float32).T).astype(BF),
        "wpxt": np.ascontiguousarray(np.asarray(W_peer, np.float32)[:, :128].T).astype(BF),
        "wpet": np.ascontiguousarray(np.asarray(W_peer, np.float32)[:, 128:].T).astype(BF),
        "bego": np.asarray(b_ego, np.float32).reshape(1, 64).astype(BF),
        "bpeer": np.asarray(b_peer, np.float32).reshape(1, 96).astype(BF),
        "iota": np.broadcast_to(np.arange(128, dtype=np.float32), (128, 128)).astype(BF).copy(),
        "ident": np.eye(128, dtype=np.float32).astype(BF),
    }
    return consts


_CACHE = {}
RUN_KWARGS = {}


def kernel(x, edge_attrs, W_peer, b_peer, W_ego, b_ego, W_edge, b_edge, edge_index):
    x = np.asarray(x)
    edge_attrs = np.asarray(edge_attrs)
    edge_index = np.asarray(edge_index)
    N, E = x.shape[0], edge_attrs.shape[0]

    row = edge_index[1].astype(np.int64)
    col = edge_index[0].astype(np.int64)
    C = 15
    CH = 8
    ncores = 8
    NPC = N // ncores
    NBLK = (NPC + 127) // 128
    deg = np.bincount(row, minlength=N)
    ovn = np.maximum(deg - C, 0)
    nodes = np.arange(N)
    bkey = (nodes // NPC) * NBLK + (nodes % NPC) // 128
    ovblk = np.bincount(bkey, weights=ovn.astype(np.float64), minlength=NBLK * ncores)
    t_ov = max(3, int(np.ceil(ovblk.max() / 128.0)))

    # phase-B dual-grid overflow sizing (per dest block x source half)
    halfs = (col // NPC) // (ncores // 2)
    degh = np.bincount(row * 2 + halfs, minlength=2 * N)
    ovn2 = np.maximum(degh - CH, 0)
    bkey2 = np.repeat(bkey, 2) * 2 + np.tile([0, 1], N)
    ovblk2 = np.bincount(bkey2, weights=ovn2.astype(np.float64),
                         minlength=2 * NBLK * ncores)
    t_ov2 = max(2, int(np.ceil(ovblk2.max() / 128.0)))

    cfg = Cfg(N=N, E=E, ncores=ncores, C=C, t_ov=t_ov, CH=CH, t_ov2=t_ov2)
    key = cfg.key()
    if key not in _CACHE:
        _CACHE[key] = build_program(cfg)
    nc = _CACHE[key]

    in_maps = host_prep(cfg, x, edge_attrs, edge_index)
    consts = make_consts(cfg, W_peer, b_peer, W_ego, b_ego, W_edge, b_edge)
    for m in in_maps:
        m.update(consts)

    res = run_bass_kernel_spmd(nc, in_maps, core_ids=list(range(cfg.ncores)),
                               **RUN_KWARGS)
    out = np.empty((N, cfg.OUTD), np.float32)
    for r in range(cfg.ncores):
        blk = np.asarray(res.results[r]["out"]).astype(np.float32)
        out[r * cfg.NPC:(r + 1) * cfg.NPC] = blk[:cfg.NPC, :cfg.OUTD]
    kernel.last_result = res
    return out
